# revision 1
# baseline (speedup 1.0000x reference)
"""GCNConv Trainium2 kernel.

Per (b, p) slice of Ans [B, P, n, n] the reference computes
    deg[m]  = sum_i A[i, m]                 (column sums)
    dhat    = 1 / (sqrt(deg) + eps)
    L       = diag(dhat) (diag(deg) - A) diag(dhat)
    out_bp  = h_p @ L          where h_p = ((X W)^T)[16p:16p+16, :]
which expands to
    out[c, m] = g[c, m] * deg[m] * dhat[m] - dhat[m] * (g @ A)[c, m]
with g = h_p * dhat (broadcast along c).  This lets the kernel stream A
in its natural row-major layout as the moving operand of the PE matmul
(contraction over A's rows), with no transpose and no materialized
Laplacian.  A is read from HBM exactly once: each 16 MiB slice is kept
SBUF-resident, column sums are computed from SBUF while it loads, and the
main matmul re-reads it from SBUF.

Sharding: core b <- batch b (8 cores).  weight/bias are replicated; each
core gets Ans[b] ([4, 2048, 2048]) and X[b].  No collectives.

Matmuls over A run in float32r (relaxed fp32, full PE rate); the tiny
X@W / broadcast matmuls run in exact fp32.  A loads as column strips
[512,512,512,256,256]; partial matmuls are emitted per (output strip,
row block) as soon as their dependencies (tiles + that column strip's
degree/dhat) are satisfied, so only the last 256 columns' worth of work
trails the final DMA.  Modeled per-core time: ~206.5us vs a ~190us
HBM-stream floor (64 MiB/core at ~358 GB/s).
"""

import numpy as np

import concourse.bacc as bacc
import concourse.mybir as mybir
import concourse.tile as tile
from concourse.bass_utils import run_bass_kernel_spmd
from concourse.masks import make_identity

F32 = mybir.dt.float32
F32R = mybir.dt.float32r
MULT = mybir.AluOpType.mult
ADD = mybir.AluOpType.add

U = 64
UP = 16  # U // P


def build(n=2048, n_slices=4, a_bufs=14):
    """Build the per-core SPMD program.

    n: graph size (multiple of 512), n_slices: number of P slices per core.
    """
    assert n % 512 == 0
    n_strips = n // 512  # output column strips
    n_blocks = n // 128  # 128-row blocks (also m-blocks)

    nc = bacc.Bacc("TRN2", target_bir_lowering=False, debug=False)

    a_in = nc.dram_tensor("a_in", [n_slices, n, n], F32, kind="ExternalInput")
    x_in = nc.dram_tensor("x_in", [n, U], F32, kind="ExternalInput")
    w_in = nc.dram_tensor("w_in", [U, U], F32, kind="ExternalInput")
    b_in = nc.dram_tensor("b_in", [U], F32, kind="ExternalInput")
    out_d = nc.dram_tensor("out", [n, U], F32, kind="ExternalOutput")

    with tile.TileContext(nc) as tc:
        with (
            tc.tile_pool(name="consts", bufs=1) as consts,
            tc.tile_pool(name="work", bufs=2) as work,
            tc.tile_pool(name="apool", bufs=min(a_bufs, 4 * n_strips + 2)) as apool,
        ):
            identity = consts.tile([128, 128], F32)
            make_identity(nc, identity[:])
            ones_col = consts.tile([128, 1], F32)
            nc.vector.memset(ones_col[:], 1.0)
            ones_r = consts.tile([128, 1], F32R)
            nc.vector.tensor_copy(ones_r[:], ones_col[:])
            ones_row = consts.tile([1, 128], F32)
            nc.vector.memset(ones_row[:], 1.0)

            # Issue the first A strip's DMAs ahead of the setup loads so
            # the big stream starts immediately (XW isn't needed for ~15us).
            pre_ats = []
            for q in range(n_strips):
                at = apool.tile([128, 4, 512], F32R, tag="A512", bufs=a_bufs, name=f"at_0_0_{q}")
                src = (
                    a_in[0, 512 * q : 512 * q + 512, 0:512]
                    .rearrange("(j r) c -> r j c", r=128)
                    .bitcast(F32R)
                )
                nc.sync.dma_start(at[:], src)
                pre_ats.append(at)

            w_sb = consts.tile([U, U], F32)
            nc.sync.dma_start(w_sb[:], w_in[:])
            bias_row = consts.tile([1, U], F32)
            nc.sync.dma_start(bias_row[:], b_in[:].unsqueeze(0))

            # xw_sb column block kb holds (X @ W)[128*kb : 128*kb+128, :]
            xw_sb = consts.tile([128, n_blocks * U], F32)
            bias_t = consts.tile([128, U], F32)
            # out staging: column block mb holds out[128*mb : 128*mb+128, :]
            out_sb = consts.tile([128, n_blocks * U], F32)

            with tc.tile_pool(name="psetup", bufs=2, space="PSUM") as psetup:
                for kb in range(n_blocks):
                    xt = work.tile([128, U], F32, tag="xt")
                    nc.sync.dma_start(xt[:], x_in[128 * kb : 128 * kb + 128, :])
                    pxt = psetup.tile([U, 128], F32, tag="pxt")
                    nc.tensor.transpose(pxt[:], xt[:], identity[:])
                    xts = work.tile([U, 128], F32, tag="xts")
                    nc.vector.tensor_copy(xts[:], pxt[:])
                    pxw = psetup.tile([128, U], F32, tag="pxw")
                    nc.tensor.matmul(pxw[:], xts[:], w_sb[:], start=True, stop=True)
                    nc.vector.tensor_copy(xw_sb[:, U * kb : U * kb + U], pxw[:])
                # bias broadcast across partitions: ones_row^T @ bias_row
                pb = psetup.tile([128, U], F32, tag="pb")
                nc.tensor.matmul(pb[:], ones_row[:], bias_row[:], start=True, stop=True)
                nc.vector.tensor_copy(bias_t[:], pb[:])

            with tc.tile_pool(name="pmain", bufs=2, space="PSUM") as pmain:
                # Column strips; the last strips are narrower so only a small
                # amount of deg/matmul work depends on the final DMAs.
                if n >= 2048:
                    widths = [512] * (n // 512 - 1) + [256, 256]
                else:
                    widths = [512] * (n // 512)
                offs = [sum(widths[:i]) for i in range(len(widths))]
                n_strip_list = list(zip(offs, widths))
                n_quads = n // 512  # 512-row groups

                # One PSUM bank per output strip: sharing a bank would
                # serialize the second accumulation group behind the first
                # group's stop (which lands in the tail).
                packs = [(i, 0) for i in range(len(n_strip_list))]
                bank_used = [w for _, w in n_strip_list]

                for p in range(n_slices):
                    # sqrt(deg) in m-on-partition layout: column kb holds
                    # sqrt(deg)[128*kb : 128*kb+128]
                    sq_cols = work.tile([128, n_blocks], F32, tag="sq_cols")
                    dhat = work.tile([128, n_blocks], F32, tag="dhat")
                    ndhat = work.tile([128, n_blocks], F32, tag="ndhat")
                    gT = work.tile([128, n_blocks * UP], F32R, tag="gT")
                    t1 = work.tile([128, n_blocks * UP], F32, tag="t1")
                    atiles = []
                    banks = [
                        pmain.tile(
                            [UP, 512], F32, tag=f"pmmb{bi}", bufs=1,
                            name=f"pmmb_{p}_{bi}",
                        )
                        for bi in range(len(bank_used))
                    ]

                    def pmm_view(t):
                        bi, c0 = packs[t]
                        return banks[bi][:, c0 : c0 + n_strip_list[t][1]]

                    started = [False] * len(n_strip_list)
                    emitted = [0] * len(n_strip_list)

                    def emit_mm(t, nb):
                        # pmm_t += gT[block nb].T @ A[rows nb, strip t cols]
                        emitted[t] += 1
                        nc.tensor.matmul(
                            pmm_view(t),
                            gT[:, UP * nb : UP * nb + UP],
                            atiles[t][nb // 4][:, nb % 4],
                            start=not started[t],
                            stop=(emitted[t] == n_blocks),
                        )
                        started[t] = True

                    def emit_scale(t):
                        # out strip t: out = t1 - dhat * M^T
                        off, w = n_strip_list[t]
                        msb = work.tile([UP, 512], F32, tag="msb", bufs=5, name=f"msb_{p}_{t}")
                        nc.scalar.copy(msb[0:UP, 0:w], pmm_view(t))
                        for j in range(w // 128):
                            mb = off // 128 + j
                            # rotate a third slot through the pdeg bank (free
                            # after the last sqrt) to loosen the transpose->
                            # stt ping-pong in the tail
                            ptag, pbufs = ("pdeg", 1) if (off // 128 + j) % 3 == 2 else ("ptr", 2)
                            pmt = pmain.tile(
                                [128, UP], F32, tag=ptag, bufs=pbufs,
                                name=f"pmt_{p}_{t}_{j}",
                            )
                            nc.tensor.transpose(
                                pmt[:],
                                msb[0:UP, 128 * j : 128 * j + 128],
                                identity[0:UP, 0:UP],
                            )
                            nc.vector.scalar_tensor_tensor(
                                out_sb[:, U * mb + UP * p : U * mb + UP * p + UP],
                                pmt[:],
                                ndhat[:, mb : mb + 1],
                                t1[:, UP * mb : UP * mb + UP],
                                MULT,
                                ADD,
                            )
                        if p == n_slices - 1:
                            # store this output strip with one strided DMA
                            dst = out_d[off : off + w, :].rearrange(
                                "(j r) u -> r j u", r=128
                            )
                            src_sb = out_sb[
                                :, (off // 128) * U : (off // 128) * U + (w // 128) * U
                            ].rearrange("r (j u) -> r j u", j=w // 128)
                            nc.sync.dma_start(dst, src_sb)

                    ready_blocks = []
                    for si, (off, w) in enumerate(n_strip_list):
                        last_strip = si == len(n_strip_list) - 1
                        if p == 0 and si == 0:
                            ats = pre_ats
                        else:
                            ats = []
                            for q in range(n_quads):
                                at = apool.tile(
                                    [128, 4, w], F32R, tag=f"A{w}",
                                    bufs=(a_bufs if w == 512 else 8),
                                    name=f"at_{p}_{si}_{q}",
                                )
                                src = (
                                    a_in[
                                        p,
                                        512 * q : 512 * q + 512,
                                        off : off + w,
                                    ]
                                    .rearrange("(j r) c -> r j c", r=128)
                                    .bitcast(F32R)
                                )
                                if (
                                    p == n_slices - 1
                                    and last_strip
                                    and q == n_quads - 1
                                ):
                                    # split the final transfer so the last deg
                                    # matmul waits on a quarter tile only
                                    for jj in range(4):
                                        nc.sync.dma_start(
                                            at[:, jj : jj + 1, :],
                                            src[:, jj : jj + 1, :],
                                        )
                                else:
                                    nc.sync.dma_start(at[:], src)
                                ats.append(at)
                        atiles.append(ats)

                        # deg -> dhat -> gT chain: latency-critical (gates all
                        # partial matmuls of this strip), so high priority.
                        with tc.high_priority():
                            pdeg = pmain.tile(
                                [1, w], F32, tag="pdeg", bufs=1,
                                padded_shape=[1, 512],
                                name=f"pdeg_{p}_{si}",
                            )
                            for q in range(n_quads):
                                for j in range(4):
                                    nc.tensor.matmul(
                                        pdeg[:],
                                        ones_r[:],
                                        ats[q][:, j],
                                        start=(q == 0 and j == 0),
                                        stop=(q == n_quads - 1 and j == 3),
                                    )
                            sq_row = work.tile(
                                [1, w], F32, tag="sq_row",
                                padded_shape=[1, 512],
                                name=f"sq_row_{p}_{si}",
                            )
                            nc.scalar.sqrt(sq_row[:], pdeg[:])
                            pt = pmain.tile(
                                [128, w // 128], F32, tag="ptr", bufs=2,
                                padded_shape=[128, UP],
                                name=f"pt_{p}_{si}",
                            )
                            for j4 in range(w // 128):
                                nc.tensor.transpose(
                                    pt[:, j4 : j4 + 1],
                                    sq_row[0:1, 128 * j4 : 128 * j4 + 128],
                                    identity[0:1, 0:1],
                                )
                            b0 = off // 128
                            cs = slice(b0, b0 + w // 128)
                            nc.vector.tensor_copy(sq_cols[:, cs], pt[:])

                            # dhat = 1/(sqrt(deg)+eps); s1 = deg*dhat;
                            # ndhat = -dhat -- per strip so partials start
                            # before the rest of the slice arrives.
                            # reference adds EPS=1e-7 to sqrt(deg)~30 before
                            # the reciprocal; that is a ~3e-9 relative shift,
                            # far below the f32r matmul noise, so skip it.
                            nc.vector.reciprocal(dhat[:, cs], sq_cols[:, cs])
                            nc.vector.tensor_scalar_mul(ndhat[:, cs], dhat[:, cs], -1.0)
                            # s1 = deg*dhat = deg/sqrt(deg) = sqrt(deg), which
                            # is sq_cols itself (exact once eps is dropped)
                            new_blocks = list(range(b0, b0 + w // 128))
                            for kb in new_blocks:
                                nc.vector.tensor_scalar_mul(
                                    gT[:, UP * kb : UP * kb + UP],
                                    xw_sb[:, U * kb + UP * p : U * kb + UP * p + UP],
                                    dhat[:, kb : kb + 1],
                                )
                                nc.vector.scalar_tensor_tensor(
                                    t1[:, UP * kb : UP * kb + UP],
                                    gT[:, UP * kb : UP * kb + UP].bitcast(F32),
                                    sq_cols[:, kb : kb + 1],
                                    bias_t[:, UP * p : UP * p + UP],
                                    MULT,
                                    ADD,
                                )

                        # Partial matmuls that just became ready.  The new
                        # strip's backlog (old gT blocks x new tiles) only
                        # needs the tiles, so emit it before the matmuls
                        # gated on this strip's deg chain.
                        for nb in ready_blocks:
                            emit_mm(si, nb)
                        if si == len(n_strip_list) - 1:
                            for nb in new_blocks:
                                emit_mm(si, nb)
                            for t in range(si):
                                for nb in new_blocks:
                                    emit_mm(t, nb)
                        else:
                            for t in range(si):
                                for nb in new_blocks:
                                    emit_mm(t, nb)
                            for nb in new_blocks:
                                emit_mm(si, nb)
                        ready_blocks += new_blocks
                        with tc.high_priority():
                            for t in [si] + list(range(si)):
                                if emitted[t] == n_blocks:
                                    emit_scale(t)

    nc.compile()
    return nc


_NC_CACHE = {}


def _get_nc():
    if "nc" not in _NC_CACHE:
        _NC_CACHE["nc"] = build()
    return _NC_CACHE["nc"]


def kernel(Ans, X, weight, bias):
    Ans = np.ascontiguousarray(Ans, dtype=np.float32)
    X = np.ascontiguousarray(X, dtype=np.float32)
    weight = np.ascontiguousarray(weight, dtype=np.float32)
    bias = np.ascontiguousarray(bias, dtype=np.float32)

    nc = _get_nc()
    in_maps = [
        {"a_in": Ans[b], "x_in": X[b], "w_in": weight, "b_in": bias}
        for b in range(Ans.shape[0])
    ]
    res = run_bass_kernel_spmd(nc, in_maps, core_ids=list(range(len(in_maps))))
    return np.stack([r["out"] for r in res.results], axis=0)



# revision 5
# speedup vs baseline: 1.0156x; 1.0156x over previous
"""GCNConv Trainium2 kernel.

Per (b, p) slice of Ans [B, P, n, n] the reference computes
    deg[m]  = sum_i A[i, m]                 (column sums)
    dhat    = 1 / (sqrt(deg) + eps)
    L       = diag(dhat) (diag(deg) - A) diag(dhat)
    out_bp  = h_p @ L          where h_p = ((X W)^T)[16p:16p+16, :]
With eps dropped (a ~3e-9 relative shift, far below the f32r matmul
noise), deg*dhat^2 == 1 exactly, so the diagonal term collapses:
    out[c, m] = (XW)[m, c] + bias[c] + dhat[m] * (gn @ A)[c, m]
with gn = -(XW)^T * dhat (broadcast along c).  t1 = XW + bias is a
setup-time constant; the per-strip critical chain is only
deg -> sqrt -> transpose -> (-1/sq) -> gn.  A streams from HBM exactly
once in column strips; column sums are computed from SBUF while it
loads, and the main matmul re-reads it from SBUF.

Output rows are staged in SBUF in a row-pair layout (partition r of
256-row group G holds rows 256G+2r and 256G+2r+1) so the store DMAs
move 512-byte contiguous chunks (256B chunks pay a 2x DMA penalty).
t1 is built in the same pair layout at setup time via strided-weight
matmuls.

Sharding: core b <- batch b (8 cores).  weight/bias are replicated; each
core gets Ans[b] ([4, 2048, 2048]) and X[b].  No collectives.

Matmuls over A run in float32r (relaxed fp32, full PE rate; output
free-dim kept >= 256 to stay at the 1 cycle/row rate); the tiny X@W /
broadcast matmuls run in exact fp32.  A loads as column strips
[512,512,512,256,256]; partial matmuls are emitted per (output strip,
row block) as soon as their dependencies (tiles + that column strip's
degree/dhat) are satisfied, so only the last 256 columns' worth of work
trails the final DMA.  deg accumulators are double-buffered in PSUM so
consecutive strips' column sums overlap, and in the endgame the last
strip's deg matmuls are ordered ahead of its backlog matmuls so the
scale chain starts as early as possible.
"""

import numpy as np

import concourse.bacc as bacc
import concourse.mybir as mybir
import concourse.tile as tile
from concourse.bass_utils import run_bass_kernel_spmd
from concourse.masks import make_identity
from concourse.tile_rust import add_dep_helper

F32 = mybir.dt.float32
F32R = mybir.dt.float32r
MULT = mybir.AluOpType.mult
ADD = mybir.AluOpType.add

U = 64
UP = 16  # U // P


def build(n=2048, n_slices=4, a_bufs=14):
    """Build the per-core SPMD program.

    n: graph size (multiple of 512), n_slices: number of P slices per core.
    """
    assert n % 512 == 0
    n_blocks = n // 128  # 128-row blocks (also m-blocks)
    n_groups = n // 256  # 256-row groups (store pair-layout units)

    nc = bacc.Bacc("TRN2", target_bir_lowering=False, debug=False)

    a_in = nc.dram_tensor("a_in", [n_slices, n, n], F32, kind="ExternalInput")
    x_in = nc.dram_tensor("x_in", [n, U], F32, kind="ExternalInput")
    w_in = nc.dram_tensor("w_in", [U, U], F32, kind="ExternalInput")
    b_in = nc.dram_tensor("b_in", [U], F32, kind="ExternalInput")
    out_d = nc.dram_tensor("out", [n, U], F32, kind="ExternalOutput")

    with tile.TileContext(nc) as tc:
        with (
            tc.tile_pool(name="consts", bufs=1) as consts,
            tc.tile_pool(name="work", bufs=2) as work,
            tc.tile_pool(name="apool", bufs=a_bufs) as apool,
        ):
            identity = consts.tile([128, 128], F32)
            make_identity(nc, identity[:])
            ones_col = consts.tile([128, 1], F32)
            nc.vector.memset(ones_col[:], 1.0)
            ones_r = consts.tile([128, 1], F32R)
            nc.vector.tensor_copy(ones_r[:], ones_col[:])
            ones_row = consts.tile([1, 128], F32)
            nc.vector.memset(ones_row[:], 1.0)
            # Issue the first A strip's DMAs ahead of the setup loads so
            # the big stream starts immediately (XW isn't needed for ~15us).
            pre_ats = []
            for q in range(n // 512):
                at = apool.tile([128, 4, 512], F32R, tag="A512", bufs=a_bufs, name=f"at_0_0_{q}")
                src = (
                    a_in[0, 512 * q : 512 * q + 512, 0:512]
                    .rearrange("(j r) c -> r j c", r=128)
                    .bitcast(F32R)
                )
                nc.sync.dma_start(at[:], src)
                pre_ats.append(at)

            # X in block layout with a single DMA: partition r of column
            # group kb holds X[128*kb + r, :].
            xtile = consts.tile([128, n_blocks, U], F32)
            nc.sync.dma_start(
                xtile[:], x_in[:].rearrange("(j r) u -> r j u", r=128)
            )
            w_sb = consts.tile([U, U], F32)
            nc.sync.dma_start(w_sb[:], w_in[:])
            bias_row = consts.tile([1, U], F32)
            nc.sync.dma_start(bias_row[:], b_in[:].unsqueeze(0))

            # xw_sb column block kb holds (X @ W)[128*kb : 128*kb+128, :]
            # (standard block layout, feeds the gn scaling).
            # t1p column group G holds (XW + bias) rows 256G+2r+s in the
            # row-pair layout: t1p[r, 128G + 64s + u].
            xw_sb = consts.tile([128, n_blocks * U], F32)
            t1p = consts.tile([128, n_blocks * U], F32)
            bias_t = consts.tile([128, U], F32)
            # out staging, same pair layout as t1p
            out_sb = consts.tile([128, n_blocks * U], F32)
            # X^T staging: xts_wide[u, m] = X[m, u]
            xts_wide = work.tile([U, n], F32, tag="xts", bufs=1)

            with tc.tile_pool(name="psetup", bufs=2, space="PSUM") as psetup:
                # bias broadcast across partitions: ones_row^T @ bias_row
                pb = psetup.tile([128, U], F32, tag="pb")
                nc.tensor.matmul(pb[:], ones_row[:], bias_row[:], start=True, stop=True)
                nc.vector.tensor_copy(bias_t[:], pb[:])
                for kb in range(n_blocks):
                    pxt = psetup.tile([U, 128], F32, tag="pxt")
                    nc.tensor.transpose(pxt[:], xtile[:, kb], identity[:])
                    nc.vector.tensor_copy(
                        xts_wide[:, 128 * kb : 128 * kb + 128], pxt[:]
                    )
                    pxw = psetup.tile([128, U], F32, tag="pxw")
                    nc.tensor.matmul(
                        pxw[:],
                        xts_wide[:, 128 * kb : 128 * kb + 128],
                        w_sb[:],
                        start=True,
                        stop=True,
                    )
                    nc.vector.tensor_copy(xw_sb[:, U * kb : U * kb + U], pxw[:])
                # pair-layout XW + bias: for 256-row group G, parity s the
                # stationary operand is X^T columns 256G+2x+s (x = 0..127),
                # so the matmul output partition r holds row 256G+2r+s.
                for G in range(n_groups):
                    xv = xts_wide[:, 256 * G : 256 * G + 256].rearrange(
                        "u (x s) -> u s x", s=2
                    )
                    for s in range(2):
                        pxw = psetup.tile([128, U], F32, tag="pxw")
                        nc.tensor.matmul(
                            pxw[:], xv[:, s], w_sb[:], start=True, stop=True
                        )
                        nc.vector.tensor_tensor(
                            t1p[:, 128 * G + 64 * s : 128 * G + 64 * s + 64],
                            pxw[:],
                            bias_t[:],
                            ADD,
                        )

            with tc.tile_pool(name="pmain", bufs=2, space="PSUM") as pmain:
                # Column strips; the last strips are narrower so only a small
                # amount of deg/matmul work depends on the final DMAs.  All
                # widths stay >= 256 to keep f32r matmuls at full rate.
                if n >= 2048:
                    widths = [512] * (n // 512 - 1) + [256, 256]
                else:
                    widths = [512] * (n // 512)
                offs = [sum(widths[:i]) for i in range(len(widths))]
                n_strip_list = list(zip(offs, widths))
                n_quads = n // 512  # 512-row groups
                n_strips = len(n_strip_list)

                for p in range(n_slices):
                    # ndhat: column kb holds -1/sqrt(deg) for rows 128kb+r
                    # (standard layout, feeds gn).  dhat_pr: column 2G+s
                    # holds +1/sqrt(deg) for rows 256G+2r+s (pair layout,
                    # feeds the output stt).
                    ndhat = work.tile([128, n_blocks], F32, tag="ndhat")
                    dhat_pr = work.tile([128, n_groups * 2], F32, tag="dhat_pr")
                    gT = work.tile([128, n_blocks * UP], F32R, tag="gT")
                    atiles = []
                    banks = [
                        pmain.tile(
                            [UP, 512], F32, tag=f"pmmb{bi}", bufs=1,
                            name=f"pmmb_{p}_{bi}",
                        )
                        for bi in range(n_strips)
                    ]

                    def pmm_view(t):
                        return banks[t][:, 0 : n_strip_list[t][1]]

                    started = [False] * n_strips
                    emitted = [0] * n_strips

                    def emit_mm(t, nb):
                        # pmm_t += gT[block nb].T @ A[rows nb, strip t cols]
                        emitted[t] += 1
                        mm = nc.tensor.matmul(
                            pmm_view(t),
                            gT[:, UP * nb : UP * nb + UP],
                            atiles[t][nb // 4][:, nb % 4],
                            start=not started[t],
                            stop=(emitted[t] == n_blocks),
                        )
                        started[t] = True
                        return mm

                    pmt_count = [0]

                    def emit_scale(t):
                        # out strip t: out = t1 + dhat * Mneg^T  (gT carries
                        # the -1/sqrt(deg) factor, so Mneg = -g @ A), built
                        # per 256-row group G and row parity s in the pair
                        # layout.
                        off, w = n_strip_list[t]
                        msb = work.tile([UP, 512], F32, tag="msb", bufs=5, name=f"msb_{p}_{t}")
                        nc.scalar.copy(msb[0:UP, 0:w], pmm_view(t))
                        for gl in range(w // 256):
                            G = off // 256 + gl
                            mv = msb[0:UP, 256 * gl : 256 * gl + 256].rearrange(
                                "c (x s) -> c s x", s=2
                            )
                            for s in range(2):
                                k = pmt_count[0]
                                pmt_count[0] += 1
                                ptag, pbufs = ("ptr", 1) if k % 3 == 0 else ("pdeg", 2)
                                pmt = pmain.tile(
                                    [128, UP], F32, tag=ptag, bufs=pbufs,
                                    name=f"pmt_{p}_{t}_{gl}_{s}",
                                )
                                nc.tensor.transpose(
                                    pmt[:], mv[:, s], identity[0:UP, 0:UP]
                                )
                                col = 128 * G + 64 * s + UP * p
                                nc.vector.scalar_tensor_tensor(
                                    out_sb[:, col : col + UP],
                                    pmt[:],
                                    dhat_pr[:, 2 * G + s : 2 * G + s + 1],
                                    t1p[:, col : col + UP],
                                    MULT,
                                    ADD,
                                )

                    ready_blocks = []
                    for si, (off, w) in enumerate(n_strip_list):
                        last_strip = si == n_strips - 1
                        endgame = last_strip and p == n_slices - 1
                        if p == 0 and si == 0:
                            ats = pre_ats
                        else:
                            ats = []
                            for q in range(n_quads):
                                at = apool.tile(
                                    [128, 4, w], F32R, tag=f"A{w}",
                                    bufs=(a_bufs if w == 512 else 8),
                                    name=f"at_{p}_{si}_{q}",
                                )
                                src = (
                                    a_in[
                                        p,
                                        512 * q : 512 * q + 512,
                                        off : off + w,
                                    ]
                                    .rearrange("(j r) c -> r j c", r=128)
                                    .bitcast(F32R)
                                )
                                if endgame and q == n_quads - 1:
                                    # split the final transfer so the last deg
                                    # matmul waits on a quarter tile only
                                    for jj in range(4):
                                        nc.sync.dma_start(
                                            at[:, jj : jj + 1, :],
                                            src[:, jj : jj + 1, :],
                                        )
                                else:
                                    nc.sync.dma_start(at[:], src)
                                ats.append(at)
                        atiles.append(ats)

                        # deg -> dhat -> gn chain: latency-critical (gates all
                        # partial matmuls of this strip), so high priority.
                        with tc.high_priority():
                            pdeg = pmain.tile(
                                [1, w], F32, tag="pdeg", bufs=2,
                                padded_shape=[1, 512],
                                name=f"pdeg_{p}_{si}",
                            )
                            last_deg = None
                            for q in range(n_quads):
                                for j in range(4):
                                    last_deg = nc.tensor.matmul(
                                        pdeg[:],
                                        ones_r[:],
                                        ats[q][:, j],
                                        start=(q == 0 and j == 0),
                                        stop=(q == n_quads - 1 and j == 3),
                                    )
                            sq_row = work.tile(
                                [1, w], F32, tag="sq_row",
                                padded_shape=[1, 512],
                                name=f"sq_row_{p}_{si}",
                            )
                            nc.scalar.sqrt(sq_row[:], pdeg[:])
                            nj = w // 128
                            pt = pmain.tile(
                                [128, 2 * nj], F32, tag="ptr", bufs=1,
                                padded_shape=[128, UP],
                                name=f"pt_{p}_{si}",
                            )
                            # standard-layout columns 0..nj: partition r of
                            # column j holds sqrt(deg)[off + 128j + r]
                            for j4 in range(nj):
                                nc.tensor.transpose(
                                    pt[:, j4 : j4 + 1],
                                    sq_row[0:1, 128 * j4 : 128 * j4 + 128],
                                    identity[0:1, 0:1],
                                )
                            # pair-layout columns: nj + 2*gl + s holds
                            # sqrt(deg)[off + 256gl + 2r + s]
                            for gl in range(w // 256):
                                sv = sq_row[
                                    0:1, 256 * gl : 256 * gl + 256
                                ].rearrange("o (x s) -> o s x", s=2)
                                for s in range(2):
                                    nc.tensor.transpose(
                                        pt[:, nj + 2 * gl + s : nj + 2 * gl + s + 1],
                                        sv[:, s],
                                        identity[0:1, 0:1],
                                    )
                            b0 = off // 128
                            cs = slice(b0, b0 + nj)
                            # dhat = +1/sqrt(deg) gates gn: emit its consumers
                            # (the gn muls) before the pair-layout reciprocal
                            # so DVE order favours the critical path.
                            nc.vector.reciprocal(ndhat[:, cs], pt[:, 0:nj])
                            new_blocks = list(range(b0, b0 + nj))
                            for kb in new_blocks:
                                nc.vector.tensor_scalar_mul(
                                    gT[:, UP * kb : UP * kb + UP],
                                    xw_sb[:, U * kb + UP * p : U * kb + UP * p + UP],
                                    ndhat[:, kb : kb + 1],
                                )
                            # pair-layout -1/sqrt(deg) (off the critical
                            # path; only the late stt reads it)
                            g0 = off // 256
                            pv = dhat_pr[:, 2 * g0 : 2 * g0 + w // 128]
                            nc.vector.reciprocal(pv, pt[:, nj : 2 * nj])
                            nc.vector.tensor_scalar_mul(pv, pv, -1.0)

                        # Partial matmuls that just became ready.  The new
                        # strip's backlog (old gT blocks x new tiles) only
                        # needs the tiles, so emit it before the matmuls
                        # gated on this strip's deg chain.
                        for nb in ready_blocks:
                            mm = emit_mm(si, nb)
                            if endgame:
                                # keep the engine order deg-first so the
                                # scale chain starts as soon as the last
                                # quarter tile lands
                                add_dep_helper(
                                    mm.ins, last_deg.ins, sync=False,
                                    reason="endgame: deg partials before backlog",
                                )
                        if last_strip:
                            for t in range(n_strips):
                                for nb in new_blocks:
                                    emit_mm(t, nb)
                        else:
                            for t in range(si):
                                for nb in new_blocks:
                                    emit_mm(t, nb)
                            for nb in new_blocks:
                                emit_mm(si, nb)
                        ready_blocks += new_blocks
                        with tc.high_priority():
                            for t in range(n_strips):
                                if emitted[t] == n_blocks:
                                    emit_scale(t)
                        if p == n_slices - 1 and last_strip:
                            # store 512-row groups in scale-completion order;
                            # 512B contiguous chunks thanks to the pair layout
                            for g in range(n // 512):
                                dst = out_d[512 * g : 512 * g + 512, :].rearrange(
                                    "(j r s) u -> r j s u", r=128, s=2
                                )
                                src_sb = out_sb[
                                    :, 256 * g : 256 * g + 256
                                ].rearrange("r (j s u) -> r j s u", j=2, s=2)
                                nc.sync.dma_start(dst, src_sb)

    nc.compile()
    return nc


_NC_CACHE = {}


def _get_nc():
    if "nc" not in _NC_CACHE:
        _NC_CACHE["nc"] = build()
    return _NC_CACHE["nc"]


def kernel(Ans, X, weight, bias):
    Ans = np.ascontiguousarray(Ans, dtype=np.float32)
    X = np.ascontiguousarray(X, dtype=np.float32)
    weight = np.ascontiguousarray(weight, dtype=np.float32)
    bias = np.ascontiguousarray(bias, dtype=np.float32)

    nc = _get_nc()
    in_maps = [
        {"a_in": Ans[b], "x_in": X[b], "w_in": weight, "b_in": bias}
        for b in range(Ans.shape[0])
    ]
    res = run_bass_kernel_spmd(nc, in_maps, core_ids=list(range(len(in_maps))))
    return np.stack([r["out"] for r in res.results], axis=0)


# revision 15
# speedup vs baseline: 1.0205x; 1.0049x over previous
"""GCNConv Trainium2 kernel.

Per (b, p) slice of Ans [B, P, n, n] the reference computes
    deg[m]  = sum_i A[i, m]                 (column sums)
    dhat    = 1 / (sqrt(deg) + eps)
    L       = diag(dhat) (diag(deg) - A) diag(dhat)
    out_bp  = h_p @ L          where h_p = ((X W)^T)[16p:16p+16, :]
With eps dropped (a ~3e-9 relative shift, far below the f32r matmul
noise), deg*dhat^2 == 1 exactly, so the diagonal term collapses:
    out[c, m] = (XW)[m, c] + bias[c] + dhat[m] * (gn @ A)[c, m]
with gn = -(XW)^T * dhat (broadcast along c).  t1 = XW + bias is a
setup-time constant; the per-strip critical chain is only
deg -> sqrt -> transpose -> (-1/sq) -> gn.  A streams from HBM exactly
once in column strips; column sums are computed from SBUF while it
loads, and the main matmul re-reads it from SBUF.

Output rows are staged in SBUF in a row-pair layout (partition r of
256-row group G holds rows 256G+2r and 256G+2r+1) so the store DMAs
move 512-byte contiguous chunks (256B chunks pay a 2x DMA penalty).
t1 is built in the same pair layout at setup time via strided-weight
matmuls.

Sharding: core b <- batch b (8 cores).  weight/bias are replicated; each
core gets Ans[b] ([4, 2048, 2048]) and X[b].  No collectives.

Matmuls over A run in float32r (relaxed fp32, full PE rate; output
free-dim kept >= 256 to stay at the 1 cycle/row rate); the tiny X@W /
broadcast matmuls run in exact fp32.  A loads as column strips
[512,512,512,256,256]; partial matmuls are emitted per (output strip,
row block) as soon as their dependencies (tiles + that column strip's
degree/dhat) are satisfied, so only the last 256 columns' worth of work
trails the final DMA.  deg accumulators are double-buffered in PSUM so
consecutive strips' column sums overlap, and in the endgame the last
strip's deg matmuls are ordered ahead of its backlog matmuls so the
scale chain starts as early as possible.
"""

import numpy as np

import concourse.bacc as bacc
import concourse.mybir as mybir
import concourse.tile as tile
from concourse.bass_utils import run_bass_kernel_spmd
from concourse.masks import make_identity
from concourse.tile_rust import add_dep_helper

F32 = mybir.dt.float32
F32R = mybir.dt.float32r
MULT = mybir.AluOpType.mult
ADD = mybir.AluOpType.add

U = 64
UP = 16  # U // P

# build-time instruction labels for trace attribution (no program effect)
MM_LABELS = {}


def _lab(bi, label):
    MM_LABELS[bi.ins.name] = label
    return bi


def build(n=2048, n_slices=4, a_bufs=14):
    """Build the per-core SPMD program.

    n: graph size (multiple of 512), n_slices: number of P slices per core.
    """
    assert n % 512 == 0
    n_blocks = n // 128  # 128-row blocks (also m-blocks)
    n_groups = n // 256  # 256-row groups (store pair-layout units)

    nc = bacc.Bacc("TRN2", target_bir_lowering=False, debug=False)

    a_in = nc.dram_tensor("a_in", [n_slices, n, n], F32, kind="ExternalInput")
    x_in = nc.dram_tensor("x_in", [n, U], F32, kind="ExternalInput")
    w_in = nc.dram_tensor("w_in", [U, U], F32, kind="ExternalInput")
    b_in = nc.dram_tensor("b_in", [U], F32, kind="ExternalInput")
    out_d = nc.dram_tensor("out", [n, U], F32, kind="ExternalOutput")

    with tile.TileContext(nc) as tc:
        with (
            tc.tile_pool(name="consts", bufs=1) as consts,
            tc.tile_pool(name="work", bufs=2) as work,
            tc.tile_pool(name="apool", bufs=a_bufs) as apool,
        ):
            identity = consts.tile([128, 128], F32)
            make_identity(nc, identity[:])
            ones_col = consts.tile([128, 1], F32)
            nc.vector.memset(ones_col[:], 1.0)
            ones_r = consts.tile([128, 1], F32R)
            nc.vector.tensor_copy(ones_r[:], ones_col[:])
            ones_row = consts.tile([1, 128], F32)
            nc.vector.memset(ones_row[:], 1.0)
            # Issue the first A strip's DMAs ahead of the setup loads so
            # the big stream starts immediately (XW isn't needed for ~15us).
            pre_ats = []
            for q in range(n // 512):
                at = apool.tile([128, 4, 512], F32R, tag="A512", bufs=a_bufs, name=f"at_0_0_{q}")
                src = (
                    a_in[0, 512 * q : 512 * q + 512, 0:512]
                    .rearrange("(j r) c -> r j c", r=128)
                    .bitcast(F32R)
                )
                nc.sync.dma_start(at[:], src)
                pre_ats.append(at)

            # X in block layout with a single DMA: partition r of column
            # group kb holds X[128*kb + r, :].
            xtile = consts.tile([128, n_blocks, U], F32)
            nc.sync.dma_start(
                xtile[:], x_in[:].rearrange("(j r) u -> r j u", r=128)
            )
            w_sb = consts.tile([U, U], F32)
            nc.sync.dma_start(w_sb[:], w_in[:])
            bias_row = consts.tile([1, U], F32)
            nc.sync.dma_start(bias_row[:], b_in[:].unsqueeze(0))

            # xw_sb column block kb holds (X @ W)[128*kb : 128*kb+128, :]
            # (standard block layout, feeds the gn scaling).
            # t1p column group G holds (XW + bias) rows 256G+2r+s in the
            # row-pair layout: t1p[r, 128G + 64s + u].
            xw_sb = consts.tile([128, n_blocks * U], F32)
            t1p = consts.tile([128, n_blocks * U], F32)
            bias_t = consts.tile([128, U], F32)
            # out staging, same pair layout as t1p
            out_sb = consts.tile([128, n_blocks * U], F32)
            # X^T staging: xts_wide[u, m] = X[m, u]
            xts_wide = work.tile([U, n], F32, tag="xts", bufs=1)

            with tc.tile_pool(name="psetup", bufs=2, space="PSUM") as psetup:
                # bias broadcast across partitions: ones_row^T @ bias_row
                pb = psetup.tile([128, U], F32, tag="pb")
                nc.tensor.matmul(pb[:], ones_row[:], bias_row[:], start=True, stop=True)
                nc.vector.tensor_copy(bias_t[:], pb[:])
                for kb in range(n_blocks):
                    pxt = psetup.tile([U, 128], F32, tag="pxt")
                    nc.tensor.transpose(pxt[:], xtile[:, kb], identity[:])
                    nc.vector.tensor_copy(
                        xts_wide[:, 128 * kb : 128 * kb + 128], pxt[:]
                    )
                    pxw = psetup.tile([128, U], F32, tag="pxw")
                    nc.tensor.matmul(
                        pxw[:],
                        xts_wide[:, 128 * kb : 128 * kb + 128],
                        w_sb[:],
                        start=True,
                        stop=True,
                    )
                    nc.vector.tensor_copy(xw_sb[:, U * kb : U * kb + U], pxw[:])
                # pair-layout XW + bias: for 256-row group G, parity s the
                # stationary operand is X^T columns 256G+2x+s (x = 0..127),
                # so the matmul output partition r holds row 256G+2r+s.
                for G in range(n_groups):
                    xv = xts_wide[:, 256 * G : 256 * G + 256].rearrange(
                        "u (x s) -> u s x", s=2
                    )
                    for s in range(2):
                        pxw = psetup.tile([128, U], F32, tag="pxw")
                        nc.tensor.matmul(
                            pxw[:], xv[:, s], w_sb[:], start=True, stop=True
                        )
                        nc.vector.tensor_tensor(
                            t1p[:, 128 * G + 64 * s : 128 * G + 64 * s + 64],
                            pxw[:],
                            bias_t[:],
                            ADD,
                        )

            with tc.tile_pool(name="pmain", bufs=2, space="PSUM") as pmain:
                # Column strips; the last strips are narrower so only a small
                # amount of deg/matmul work depends on the final DMAs.  All
                # widths stay >= 256 to keep f32r matmuls at full rate.
                if n >= 2048:
                    widths = [512] * (n // 512 - 1) + [256, 256]
                else:
                    widths = [512] * (n // 512)
                offs = [sum(widths[:i]) for i in range(len(widths))]
                n_strip_list = list(zip(offs, widths))
                n_quads = n // 512  # 512-row groups
                n_strips = len(n_strip_list)

                for p in range(n_slices):
                    # ndhat: column kb holds -1/sqrt(deg) for rows 128kb+r
                    # (standard layout, feeds gn).  dhat_pr: column 2G+s
                    # holds +1/sqrt(deg) for rows 256G+2r+s (pair layout,
                    # feeds the output stt).
                    ndhat = work.tile([128, n_blocks], F32, tag="ndhat")
                    dhat_pr = work.tile([128, n_groups * 2], F32, tag="dhat_pr")
                    gT = work.tile([128, n_blocks * UP], F32R, tag="gT")
                    atiles = []
                    banks = [
                        pmain.tile(
                            [UP, 512], F32, tag=f"pmmb{bi}", bufs=1,
                            name=f"pmmb_{p}_{bi}",
                        )
                        for bi in range(n_strips)
                    ]

                    def pmm_view(t):
                        return banks[t][:, 0 : n_strip_list[t][1]]

                    started = [False] * n_strips
                    emitted = [0] * n_strips

                    def emit_mm(t, nb):
                        # pmm_t += gT[block nb].T @ A[rows nb, strip t cols]
                        emitted[t] += 1
                        mm = _lab(nc.tensor.matmul(
                            pmm_view(t),
                            gT[:, UP * nb : UP * nb + UP],
                            atiles[t][nb // 4][:, nb % 4],
                            start=not started[t],
                            stop=(emitted[t] == n_blocks),
                        ), f"mm_p{p}_t{t}_nb{nb}")
                        started[t] = True
                        return mm

                    pmt_count = [0]

                    def emit_scale(t):
                        # out strip t: out = t1 + dhat * Mneg^T  (gT carries
                        # the -1/sqrt(deg) factor, so Mneg = -g @ A), built
                        # per 256-row group G and row parity s in the pair
                        # layout.
                        off, w = n_strip_list[t]
                        msb = work.tile([UP, 512], F32, tag="msb", bufs=5, name=f"msb_{p}_{t}")
                        nc.scalar.copy(msb[0:UP, 0:w], pmm_view(t))
                        for gl in range(w // 256):
                            G = off // 256 + gl
                            mv = msb[0:UP, 256 * gl : 256 * gl + 256].rearrange(
                                "c (x s) -> c s x", s=2
                            )
                            for s in range(2):
                                k = pmt_count[0]
                                pmt_count[0] += 1
                                ptag, pbufs = ("ptr", 1) if k % 3 == 0 else ("pdeg", 2)
                                pmt = pmain.tile(
                                    [128, UP], F32, tag=ptag, bufs=pbufs,
                                    name=f"pmt_{p}_{t}_{gl}_{s}",
                                )
                                _lab(nc.tensor.transpose(
                                    pmt[:], mv[:, s], identity[0:UP, 0:UP]
                                ), f"pmtT_p{p}_t{t}_{gl}_{s}")
                                col = 128 * G + 64 * s + UP * p
                                nc.vector.scalar_tensor_tensor(
                                    out_sb[:, col : col + UP],
                                    pmt[:],
                                    dhat_pr[:, 2 * G + s : 2 * G + s + 1],
                                    t1p[:, col : col + UP],
                                    MULT,
                                    ADD,
                                )

                    ready_blocks = []
                    for si, (off, w) in enumerate(n_strip_list):
                        last_strip = si == n_strips - 1
                        endgame = last_strip and p == n_slices - 1
                        if p == 0 and si == 0:
                            ats = pre_ats
                        else:
                            ats = []
                            for q in range(n_quads):
                                at = apool.tile(
                                    [128, 4, w], F32R, tag=f"A{w}",
                                    bufs=(a_bufs if w == 512 else 8),
                                    name=f"at_{p}_{si}_{q}",
                                )
                                src = (
                                    a_in[
                                        p,
                                        512 * q : 512 * q + 512,
                                        off : off + w,
                                    ]
                                    .rearrange("(j r) c -> r j c", r=128)
                                    .bitcast(F32R)
                                )
                                if endgame and q == n_quads - 1:
                                    # split the final quad by column halves so
                                    # the first half's column sums close while
                                    # the second half still streams
                                    h = w // 2
                                    for jj in range(2):
                                        nc.sync.dma_start(
                                            at[:, :, jj * h : jj * h + h],
                                            src[:, :, jj * h : jj * h + h],
                                        )
                                else:
                                    nc.sync.dma_start(at[:], src)
                                ats.append(at)
                        atiles.append(ats)

                        # Column sums directly in column layout: for each
                        # 128-column chunk, deg_chunk[m] = A[:, chunk]^T @ ones
                        # with the A piece as the (cost-free) stationary
                        # operand and a single ones column moving.  pdeg
                        # columns 0..nj hold sqrt-input in standard layout
                        # (partition r of column c = deg[off+128c+r]); columns
                        # nj..2nj hold the row-pair layout (deg[off+256gl+2r+s]
                        # at column nj+2gl+s).  The accumulation groups on the
                        # shared tile serialize in emission order, which also
                        # pins the PE order deg-first in the endgame.
                        nj = w // 128
                        with tc.high_priority():
                            pdeg = pmain.tile(
                                [128, 2 * nj], F32, tag="pdeg", bufs=2,
                                padded_shape=[128, UP],
                                name=f"pdeg_{p}_{si}",
                            )

                        def deg_group(col, view_fn, label):
                            # exact-f32 matmuls: the fp32r path has ISA
                            # restrictions that reject a 1-column moving
                            # operand
                            with tc.high_priority():
                                for i in range(16):
                                    _lab(nc.tensor.matmul(
                                        pdeg[:, col : col + 1],
                                        view_fn(i).bitcast(F32),
                                        ones_col[:],
                                        start=(i == 0),
                                        stop=(i == 15),
                                    ), f"{label}_i{i}")

                        def std_view(c):
                            return lambda i: ats[i // 4][:, i % 4, 128 * c : 128 * c + 128]

                        def pair_view(gl, s):
                            def f(i):
                                return ats[i // 4][
                                    :, i % 4, 256 * gl : 256 * gl + 256
                                ].rearrange("r (x s) -> r s x", s=2)[:, s]

                            return f

                        if endgame:
                            # interleave the (tiny) deg groups with the
                            # strip's backlog matmuls in quad arrival order
                            deg_group(0, std_view(0), f"degS_p{p}_s{si}_c0")
                            for nb in ready_blocks:
                                emit_mm(si, nb)
                            for c in range(1, nj):
                                deg_group(c, std_view(c), f"degS_p{p}_s{si}_c{c}")
                            for gl in range(w // 256):
                                for s in range(2):
                                    deg_group(
                                        nj + 2 * gl + s, pair_view(gl, s),
                                        f"degP_p{p}_s{si}_g{gl}_{s}",
                                    )
                        else:
                            for c in range(nj):
                                deg_group(c, std_view(c), f"degS_p{p}_s{si}_c{c}")
                            for gl in range(w // 256):
                                for s in range(2):
                                    deg_group(
                                        nj + 2 * gl + s, pair_view(gl, s),
                                        f"degP_p{p}_s{si}_g{gl}_{s}",
                                    )
                        with tc.high_priority():
                            sq_cols = work.tile(
                                [128, 2 * nj], F32, tag="sq_cols",
                                padded_shape=[128, UP],
                                name=f"sq_cols_{p}_{si}",
                            )
                            nc.scalar.sqrt(sq_cols[:], pdeg[:])
                            b0 = off // 128
                            cs = slice(b0, b0 + nj)
                            # dhat = +1/sqrt(deg) gates gn: emit its consumers
                            # (the gn muls) before the pair-layout reciprocal
                            # so DVE order favours the critical path.
                            nc.vector.reciprocal(ndhat[:, cs], sq_cols[:, 0:nj])
                            new_blocks = list(range(b0, b0 + nj))
                            for kb in new_blocks:
                                nc.vector.tensor_scalar_mul(
                                    gT[:, UP * kb : UP * kb + UP],
                                    xw_sb[:, U * kb + UP * p : U * kb + UP * p + UP],
                                    ndhat[:, kb : kb + 1],
                                )
                            # pair-layout -1/sqrt(deg) (off the critical
                            # path; only the late stt reads it)
                            g0 = off // 256
                            pv = dhat_pr[:, 2 * g0 : 2 * g0 + w // 128]
                            nc.vector.reciprocal(pv, sq_cols[:, nj : 2 * nj])
                            nc.vector.tensor_scalar_mul(pv, pv, -1.0)

                        # Partial matmuls that just became ready.  The new
                        # strip's backlog (old gT blocks x new tiles) only
                        # needs the tiles, so emit it before the matmuls
                        # gated on this strip's deg chain.
                        if not endgame:
                            for nb in ready_blocks:
                                emit_mm(si, nb)
                        if last_strip:
                            # narrow strips first so their scale chains (and
                            # the last store group) start early
                            for t in [si, si - 1] + list(range(si - 1)):
                                for nb in new_blocks:
                                    emit_mm(t, nb)
                        else:
                            for t in range(si):
                                for nb in new_blocks:
                                    emit_mm(t, nb)
                            for nb in new_blocks:
                                emit_mm(si, nb)
                        ready_blocks += new_blocks
                        with tc.high_priority():
                            scale_order = (
                                [si, si - 1] + list(range(si - 1))
                                if last_strip
                                else range(n_strips)
                            )
                            for t in scale_order:
                                if emitted[t] == n_blocks:
                                    emit_scale(t)
                        if p == n_slices - 1 and last_strip:
                            # store 512-row groups in scale-completion order;
                            # 512B contiguous chunks thanks to the pair layout
                            for g in [3, 0, 1, 2] if n == 2048 else range(n // 512):
                                dst = out_d[512 * g : 512 * g + 512, :].rearrange(
                                    "(j r s) u -> r j s u", r=128, s=2
                                )
                                src_sb = out_sb[
                                    :, 256 * g : 256 * g + 256
                                ].rearrange("r (j s u) -> r j s u", j=2, s=2)
                                nc.sync.dma_start(dst, src_sb)

    nc.compile()
    return nc


_NC_CACHE = {}


def _get_nc():
    if "nc" not in _NC_CACHE:
        _NC_CACHE["nc"] = build()
    return _NC_CACHE["nc"]


def kernel(Ans, X, weight, bias):
    Ans = np.ascontiguousarray(Ans, dtype=np.float32)
    X = np.ascontiguousarray(X, dtype=np.float32)
    weight = np.ascontiguousarray(weight, dtype=np.float32)
    bias = np.ascontiguousarray(bias, dtype=np.float32)

    nc = _get_nc()
    in_maps = [
        {"a_in": Ans[b], "x_in": X[b], "w_in": weight, "b_in": bias}
        for b in range(Ans.shape[0])
    ]
    res = run_bass_kernel_spmd(nc, in_maps, core_ids=list(range(len(in_maps))))
    return np.stack([r["out"] for r in res.results], axis=0)


# revision 26
# speedup vs baseline: 1.0334x; 1.0127x over previous
"""GCNConv Trainium2 kernel.

Per (b, p) slice of Ans [B, P, n, n] the reference computes
    deg[m]  = sum_i A[i, m]                 (column sums)
    dhat    = 1 / (sqrt(deg) + eps)
    L       = diag(dhat) (diag(deg) - A) diag(dhat)
    out_bp  = h_p @ L          where h_p = ((X W)^T)[16p:16p+16, :]
With eps dropped (a ~3e-9 relative shift, far below the f32r matmul
noise), deg*dhat^2 == 1 exactly, so the diagonal term collapses:
    out[c, m] = (XW)[m, c] + bias[c] + dhat[m] * (gn @ A)[c, m]
with gn = -(XW)^T * dhat (broadcast along c).  t1 = XW + bias is a
setup-time constant; the per-strip critical chain is only
deg -> sqrt -> transpose -> (-1/sq) -> gn.  A streams from HBM exactly
once in column strips; column sums are computed from SBUF while it
loads, and the main matmul re-reads it from SBUF.

Output rows are staged in SBUF in a row-pair layout (partition r of
256-row group G holds rows 256G+2r and 256G+2r+1) so the store DMAs
move 512-byte contiguous chunks (256B chunks pay a 2x DMA penalty).
t1 is built in the same pair layout at setup time via strided-weight
matmuls.

Sharding: core b <- batch b (8 cores).  weight/bias are replicated; each
core gets Ans[b] ([4, 2048, 2048]) and X[b].  No collectives.

Matmuls over A run in float32r (relaxed fp32, full PE rate; output
free-dim kept >= 256 to stay at the 1 cycle/row rate); the tiny X@W /
broadcast matmuls run in exact fp32.  A loads as column strips
[512,512,512,256,256]; partial matmuls are emitted per (output strip,
row block) as soon as their dependencies (tiles + that column strip's
degree/dhat) are satisfied, so only the last 256 columns' worth of work
trails the final DMA.  deg accumulators are double-buffered in PSUM so
consecutive strips' column sums overlap, and in the endgame the last
strip's deg matmuls are ordered ahead of its backlog matmuls so the
scale chain starts as early as possible.
"""

import numpy as np

import concourse.bacc as bacc
import concourse.mybir as mybir
import concourse.tile as tile
from concourse.bass_utils import run_bass_kernel_spmd
from concourse.masks import make_identity
from concourse.tile_rust import add_dep_helper

F32 = mybir.dt.float32
F32R = mybir.dt.float32r
MULT = mybir.AluOpType.mult
ADD = mybir.AluOpType.add

U = 64
UP = 16  # U // P

# build-time instruction labels for trace attribution (no program effect)
MM_LABELS = {}


def _lab(bi, label):
    MM_LABELS[bi.ins.name] = label
    return bi


def build(n=2048, n_slices=4, a_bufs=14):
    """Build the per-core SPMD program.

    n: graph size (multiple of 512), n_slices: number of P slices per core.
    """
    assert n % 512 == 0
    n_blocks = n // 128  # 128-row blocks (also m-blocks)
    n_groups = n // 256  # 256-row groups (store pair-layout units)

    nc = bacc.Bacc("TRN2", target_bir_lowering=False, debug=False)

    a_in = nc.dram_tensor("a_in", [n_slices, n, n], F32, kind="ExternalInput")
    x_in = nc.dram_tensor("x_in", [n, U], F32, kind="ExternalInput")
    w_in = nc.dram_tensor("w_in", [U, U], F32, kind="ExternalInput")
    b_in = nc.dram_tensor("b_in", [U], F32, kind="ExternalInput")
    out_d = nc.dram_tensor("out", [n, U], F32, kind="ExternalOutput")

    with tile.TileContext(nc) as tc:
        with (
            tc.tile_pool(name="consts", bufs=1) as consts,
            tc.tile_pool(name="work", bufs=2) as work,
            tc.tile_pool(name="apool", bufs=a_bufs) as apool,
        ):
            identity = consts.tile([128, 128], F32)
            make_identity(nc, identity[:])
            ones_col = consts.tile([128, 1], F32)
            nc.vector.memset(ones_col[:], 1.0)
            ones_r = consts.tile([128, 1], F32R)
            nc.vector.tensor_copy(ones_r[:], ones_col[:])
            ones_row = consts.tile([1, 128], F32)
            nc.vector.memset(ones_row[:], 1.0)
            # Issue the first A strip's DMAs ahead of the setup loads so
            # the big stream starts immediately (XW isn't needed for ~15us).
            pre_ats = []
            for q in range(n // 512):
                at = apool.tile([128, 4, 512], F32R, tag="A512", bufs=a_bufs, name=f"at_0_0_{q}")
                src = (
                    a_in[0, 512 * q : 512 * q + 512, 0:512]
                    .rearrange("(j r) c -> r j c", r=128)
                    .bitcast(F32R)
                )
                nc.sync.dma_start(at[:], src)
                pre_ats.append(at)

            # X loaded r-major (partition r holds rows 16r..16r+15) so the
            # DMA moves 4KiB contiguous chunks; the block-layout fixup
            # happens in the setup transposes below.
            xtile = consts.tile([128, n_blocks, U], F32)
            nc.sync.dma_start(
                xtile[:], x_in[:].rearrange("(r j) u -> r j u", j=n_blocks)
            )
            w_sb = consts.tile([U, U], F32)
            nc.sync.dma_start(w_sb[:], w_in[:])
            bias_row = consts.tile([1, U], F32)
            nc.sync.dma_start(bias_row[:], b_in[:].unsqueeze(0))

            # xw_sb column block kb holds (X @ W)[128*kb : 128*kb+128, :]
            # (standard block layout, feeds the gn scaling).
            # t1p column group G holds (XW + bias) rows 256G+2r+s in the
            # row-pair layout: t1p[r, 128G + 64s + u].
            xw_sb = consts.tile([128, n_blocks * U], F32)
            t1p = consts.tile([128, n_blocks * U], F32)
            bias_t = consts.tile([128, U], F32)
            # out staging, same pair layout as t1p
            out_sb = consts.tile([128, n_blocks * U], F32)
            # X^T staging: xts_wide[u, m] = X[m, u]
            xts_wide = work.tile([U, n], F32, tag="xts", bufs=1)

            with tc.tile_pool(name="psetup", bufs=2, space="PSUM") as psetup:
                # bias broadcast across partitions: ones_row^T @ bias_row
                pb = psetup.tile([128, U], F32, tag="pb")
                nc.tensor.matmul(pb[:], ones_row[:], bias_row[:], start=True, stop=True)
                nc.vector.tensor_copy(bias_t[:], pb[:])
                # X^T assembly: transposing the r-major chunk j gives X^T
                # columns {16r+j}; a stride-16 copy scatters them into place.
                xtv = xts_wide[:].rearrange("u (r j) -> u j r", j=n_blocks)
                for j in range(n_blocks):
                    pxt = psetup.tile([U, 128], F32, tag="pxt")
                    nc.tensor.transpose(pxt[:], xtile[:, j], identity[:])
                    nc.vector.tensor_copy(xtv[:, j], pxt[:])
                for kb in range(n_blocks):
                    pxw = psetup.tile([128, U], F32, tag="pxw")
                    nc.tensor.matmul(
                        pxw[:],
                        xts_wide[:, 128 * kb : 128 * kb + 128],
                        w_sb[:],
                        start=True,
                        stop=True,
                    )
                    nc.vector.tensor_copy(xw_sb[:, U * kb : U * kb + U], pxw[:])
                # pair-layout XW + bias: for 256-row group G, parity s the
                # stationary operand is X^T columns 256G+2x+s (x = 0..127),
                # so the matmul output partition r holds row 256G+2r+s.
                for G in range(n_groups):
                    xv = xts_wide[:, 256 * G : 256 * G + 256].rearrange(
                        "u (x s) -> u s x", s=2
                    )
                    for s in range(2):
                        pxw = psetup.tile([128, U], F32, tag="pxw")
                        nc.tensor.matmul(
                            pxw[:], xv[:, s], w_sb[:], start=True, stop=True
                        )
                        nc.vector.tensor_tensor(
                            t1p[:, 128 * G + 64 * s : 128 * G + 64 * s + 64],
                            pxw[:],
                            bias_t[:],
                            ADD,
                        )

            with tc.tile_pool(name="pmain", bufs=2, space="PSUM") as pmain:
                # Column strips; the last strips are narrower so only a small
                # amount of deg/matmul work depends on the final DMAs.  All
                # widths stay >= 256 to keep f32r matmuls at full rate.
                if n >= 2048:
                    widths = [512] * (n // 512 - 1) + [256, 256]
                else:
                    widths = [512] * (n // 512)
                offs = [sum(widths[:i]) for i in range(len(widths))]
                n_strip_list = list(zip(offs, widths))
                n_quads = n // 512  # 512-row groups
                n_strips = len(n_strip_list)

                for p in range(n_slices):
                    # ndhat: column kb holds -1/sqrt(deg) for rows 128kb+r
                    # (standard layout, feeds gn).  dhat_pr: column 2G+s
                    # holds +1/sqrt(deg) for rows 256G+2r+s (pair layout,
                    # feeds the output stt).
                    ndhat = work.tile([128, n_blocks], F32, tag="ndhat")
                    dhat_pr = work.tile([128, n_groups * 2], F32, tag="dhat_pr")
                    gT = work.tile([128, n_blocks * UP], F32R, tag="gT")
                    atiles = []
                    banks = [
                        pmain.tile(
                            [UP, 512], F32, tag=f"pmmb{bi}", bufs=1,
                            name=f"pmmb_{p}_{bi}",
                        )
                        for bi in range(n_strips)
                    ]

                    def pmm_view(t):
                        return banks[t][:, 0 : n_strip_list[t][1]]

                    started = [False] * n_strips
                    emitted = [0] * n_strips
                    scaled = [False] * n_strips
                    # In the last slice, strips other than the final one close
                    # their accumulation at block 13 so their scale work runs
                    # while the final strip still streams; blocks 14/15 are
                    # applied later as an A-stationary correction.
                    last_slice = p == n_slices - 1
                    stop_at = [
                        n_blocks - 2 if last_slice and t < n_strips - 1 else n_blocks
                        for t in range(n_strips)
                    ]

                    def emit_mm(t, nb):
                        # pmm_t += gT[block nb].T @ A[rows nb, strip t cols]
                        emitted[t] += 1
                        mm = _lab(nc.tensor.matmul(
                            pmm_view(t),
                            gT[:, UP * nb : UP * nb + UP],
                            atiles[t][nb // 4][:, nb % 4],
                            start=not started[t],
                            stop=(emitted[t] == stop_at[t]),
                        ), f"mm_p{p}_t{t}_nb{nb}")
                        started[t] = True
                        return mm

                    pmt_count = [0]

                    def emit_scale(t):
                        # out strip t: out = t1 + dhat * Mneg^T  (gT carries
                        # the -1/sqrt(deg) factor, so Mneg = -g @ A), built
                        # per 256-row group G and row parity s in the pair
                        # layout.
                        off, w = n_strip_list[t]
                        msb = work.tile([UP, 512], F32, tag="msb", bufs=5, name=f"msb_{p}_{t}")
                        nc.scalar.copy(msb[0:UP, 0:w], pmm_view(t))
                        for gl in range(w // 256):
                            G = off // 256 + gl
                            mv = msb[0:UP, 256 * gl : 256 * gl + 256].rearrange(
                                "c (x s) -> c s x", s=2
                            )
                            for s in range(2):
                                k = pmt_count[0]
                                pmt_count[0] += 1
                                ptag, pbufs = ("ptr", 1) if k % 3 == 0 else ("pdeg", 2)
                                pmt = pmain.tile(
                                    [128, UP], F32, tag=ptag, bufs=pbufs,
                                    name=f"pmt_{p}_{t}_{gl}_{s}",
                                )
                                _lab(nc.tensor.transpose(
                                    pmt[:], mv[:, s], identity[0:UP, 0:UP]
                                ), f"pmtT_p{p}_t{t}_{gl}_{s}")
                                col = 128 * G + 64 * s + UP * p
                                nc.vector.scalar_tensor_tensor(
                                    out_sb[:, col : col + UP],
                                    pmt[:],
                                    dhat_pr[:, 2 * G + s : 2 * G + s + 1],
                                    t1p[:, col : col + UP],
                                    MULT,
                                    ADD,
                                )

                    ready_blocks = []
                    for si, (off, w) in enumerate(n_strip_list):
                        last_strip = si == n_strips - 1
                        endgame = last_strip and p == n_slices - 1
                        if p == 0 and si == 0:
                            ats = pre_ats
                        else:
                            ats = []
                            for q in range(n_quads):
                                at = apool.tile(
                                    [128, 4, w], F32R, tag=f"A{w}",
                                    bufs=(a_bufs if w == 512 else 8),
                                    name=f"at_{p}_{si}_{q}",
                                )
                                src = (
                                    a_in[
                                        p,
                                        512 * q : 512 * q + 512,
                                        off : off + w,
                                    ]
                                    .rearrange("(j r) c -> r j c", r=128)
                                    .bitcast(F32R)
                                )
                                if endgame and q == n_quads - 1:
                                    # split the final quad by column halves so
                                    # the first half's column sums close while
                                    # the second half still streams
                                    h = w // 2
                                    for jj in range(2):
                                        nc.sync.dma_start(
                                            at[:, :, jj * h : jj * h + h],
                                            src[:, :, jj * h : jj * h + h],
                                        )
                                else:
                                    nc.sync.dma_start(at[:], src)
                                ats.append(at)
                        atiles.append(ats)

                        # Column sums directly in column layout: for each
                        # 128-column chunk, deg_chunk[m] = A[:, chunk]^T @ ones
                        # with the A piece as the (cost-free) stationary
                        # operand and a single ones column moving.  pdeg
                        # columns 0..nj hold sqrt-input in standard layout
                        # (partition r of column c = deg[off+128c+r]); columns
                        # nj..2nj hold the row-pair layout (deg[off+256gl+2r+s]
                        # at column nj+2gl+s).  The accumulation groups on the
                        # shared tile serialize in emission order, which also
                        # pins the PE order deg-first in the endgame.
                        nj = w // 128
                        with tc.high_priority():
                            pdeg = pmain.tile(
                                [128, 2 * nj], F32, tag="pdeg", bufs=2,
                                padded_shape=[128, UP],
                                name=f"pdeg_{p}_{si}",
                            )

                        def deg_group(col, view_fn, label):
                            # exact-f32 matmuls: the fp32r path has ISA
                            # restrictions that reject a 1-column moving
                            # operand
                            with tc.high_priority():
                                for i in range(16):
                                    _lab(nc.tensor.matmul(
                                        pdeg[:, col : col + 1],
                                        view_fn(i).bitcast(F32),
                                        ones_col[:],
                                        start=(i == 0),
                                        stop=(i == 15),
                                    ), f"{label}_i{i}")

                        def std_view(c):
                            return lambda i: ats[i // 4][:, i % 4, 128 * c : 128 * c + 128]

                        def pair_view(gl, s):
                            def f(i):
                                return ats[i // 4][
                                    :, i % 4, 256 * gl : 256 * gl + 256
                                ].rearrange("r (x s) -> r s x", s=2)[:, s]

                            return f

                        if endgame:
                            # interleave the (tiny) deg groups with the
                            # strip's backlog matmuls in quad arrival order
                            deg_group(0, std_view(0), f"degS_p{p}_s{si}_c0")
                            for nb in ready_blocks:
                                emit_mm(si, nb)
                            for c in range(1, nj):
                                deg_group(c, std_view(c), f"degS_p{p}_s{si}_c{c}")
                            for gl in range(w // 256):
                                for s in range(2):
                                    deg_group(
                                        nj + 2 * gl + s, pair_view(gl, s),
                                        f"degP_p{p}_s{si}_g{gl}_{s}",
                                    )
                        else:
                            for c in range(nj):
                                deg_group(c, std_view(c), f"degS_p{p}_s{si}_c{c}")
                            for gl in range(w // 256):
                                for s in range(2):
                                    deg_group(
                                        nj + 2 * gl + s, pair_view(gl, s),
                                        f"degP_p{p}_s{si}_g{gl}_{s}",
                                    )
                        with tc.high_priority():
                            sq_cols = work.tile(
                                [128, 2 * nj], F32, tag="sq_cols",
                                padded_shape=[128, UP],
                                name=f"sq_cols_{p}_{si}",
                            )
                            nc.scalar.sqrt(sq_cols[:], pdeg[:])
                            b0 = off // 128
                            cs = slice(b0, b0 + nj)
                            # dhat = +1/sqrt(deg) gates gn: emit its consumers
                            # (the gn muls) before the pair-layout reciprocal
                            # so DVE order favours the critical path.
                            nc.vector.reciprocal(ndhat[:, cs], sq_cols[:, 0:nj])
                            new_blocks = list(range(b0, b0 + nj))
                            for kb in new_blocks:
                                nc.vector.tensor_scalar_mul(
                                    gT[:, UP * kb : UP * kb + UP],
                                    xw_sb[:, U * kb + UP * p : U * kb + UP * p + UP],
                                    ndhat[:, kb : kb + 1],
                                )
                        # pair-layout -1/sqrt(deg) (off the critical
                        # path; only the late stt reads it)
                        g0 = off // 256
                        pv = dhat_pr[:, 2 * g0 : 2 * g0 + w // 128]
                        nc.vector.reciprocal(pv, sq_cols[:, nj : 2 * nj])
                        nc.vector.tensor_scalar_mul(pv, pv, -1.0)

                        # Partial matmuls that just became ready.  The new
                        # strip's backlog (old gT blocks x new tiles) only
                        # needs the tiles, so emit it before the matmuls
                        # gated on this strip's deg chain.
                        if not endgame:
                            for nb in ready_blocks:
                                emit_mm(si, nb)
                        if endgame:
                            # only the final strip still accumulates the last
                            # two blocks in its main (gn-stationary) group
                            for nb in new_blocks:
                                emit_mm(si, nb)
                        elif last_strip:
                            for t in [si, si - 1] + list(range(si - 1)):
                                for nb in new_blocks:
                                    emit_mm(t, nb)
                        else:
                            for t in range(si):
                                for nb in new_blocks:
                                    emit_mm(t, nb)
                            for nb in new_blocks:
                                emit_mm(si, nb)
                        ready_blocks += new_blocks
                        if not endgame:
                            with tc.high_priority():
                                for t in range(n_strips):
                                    if emitted[t] == stop_at[t] and not scaled[t]:
                                        scaled[t] = True
                                        emit_scale(t)

                        if endgame:
                            # Correction pass: add dhat * (gn[14:16] @ A)^T for
                            # strips 0..3, computed per 256-row pair group with
                            # the A piece as the (cost-free) stationary operand
                            # so the output lands directly in the pair layout.
                            # PSUM comes from the now-free main banks.
                            def corr_unit(G, s, bank):
                                t = next(
                                    tt
                                    for tt, (o, ww) in enumerate(n_strip_list)
                                    if o <= 256 * G < o + ww
                                )
                                o, ww = n_strip_list[t]
                                gl = (256 * G - o) // 256
                                corr = pmain.tile(
                                    [128, UP], F32, tag=f"pmmb{bank}", bufs=1,
                                    name=f"corr_{G}_{s}",
                                )
                                for k, i in enumerate((n_blocks - 2, n_blocks - 1)):
                                    lhs = (
                                        atiles[t][i // 4][
                                            :, i % 4, 256 * gl : 256 * gl + 256
                                        ]
                                        .rearrange("r (x s) -> r s x", s=2)[:, s]
                                        .bitcast(F32)
                                    )
                                    _lab(nc.tensor.matmul(
                                        corr[:],
                                        lhs,
                                        gT[:, UP * i : UP * i + UP].bitcast(F32),
                                        start=(k == 0),
                                        stop=(k == 1),
                                    ), f"corr_p{p}_G{G}_s{s}_i{i}")
                                col = 128 * G + 64 * s + UP * p
                                nc.vector.scalar_tensor_tensor(
                                    out_sb[:, col : col + UP],
                                    corr[:],
                                    dhat_pr[:, 2 * G + s : 2 * G + s + 1],
                                    out_sb[:, col : col + UP],
                                    MULT,
                                    ADD,
                                )

                            def store_group(g):
                                dst = out_d[512 * g : 512 * g + 512, :].rearrange(
                                    "(j r s) u -> r j s u", r=128, s=2
                                )
                                src_sb = out_sb[
                                    :, 256 * g : 256 * g + 256
                                ].rearrange("r (j s u) -> r j s u", j=2, s=2)
                                nc.sync.dma_start(dst, src_sb)

                            nbank = [0]

                            def next_bank():
                                nbank[0] = (nbank[0] + 1) % (n_strips - 1)
                                return nbank[0]

                            # Correction-unit order: tail strips' groups
                            # first (their scale work finished earliest), then
                            # walk backwards so each store's units complete in
                            # sequence; the last store is the smallest chunk
                            # still waiting on the DVE stream's tail.
                            for s in range(2):
                                corr_unit(n_groups - 2, s, next_bank())
                            emit_scale(si)  # final strip: full M in one pass
                            for G in (4, 5):
                                for s in range(2):
                                    corr_unit(G, s, next_bank())
                            # rows 1024..2048 ready: one 256KiB store
                            dst = out_d[1024:2048, :].rearrange(
                                "(j r s) u -> r j s u", r=128, s=2
                            )
                            nc.sync.dma_start(
                                dst,
                                out_sb[:, 512:1024].rearrange(
                                    "r (j s u) -> r j s u", j=4, s=2
                                ),
                            )
                            for G in (2, 3):
                                for s in range(2):
                                    corr_unit(G, s, next_bank())
                            store_group(1)
                            for G in (0, 1):
                                for s in range(2):
                                    corr_unit(G, s, next_bank())
                            store_group(0)

    nc.compile()
    return nc


_NC_CACHE = {}


def _get_nc():
    if "nc" not in _NC_CACHE:
        _NC_CACHE["nc"] = build()
    return _NC_CACHE["nc"]


def kernel(Ans, X, weight, bias):
    Ans = np.ascontiguousarray(Ans, dtype=np.float32)
    X = np.ascontiguousarray(X, dtype=np.float32)
    weight = np.ascontiguousarray(weight, dtype=np.float32)
    bias = np.ascontiguousarray(bias, dtype=np.float32)

    nc = _get_nc()
    in_maps = [
        {"a_in": Ans[b], "x_in": X[b], "w_in": weight, "b_in": bias}
        for b in range(Ans.shape[0])
    ]
    res = run_bass_kernel_spmd(nc, in_maps, core_ids=list(range(len(in_maps))))
    return np.stack([r["out"] for r in res.results], axis=0)


# revision 28
# speedup vs baseline: 1.6701x; 1.6161x over previous
"""GCNConv Trainium2 kernel.

Per (b, p) slice of Ans [B, P, n, n] the reference computes
    deg[m]  = sum_i A[i, m]                 (column sums)
    dhat    = 1 / (sqrt(deg) + eps)
    L       = diag(dhat) (diag(deg) - A) diag(dhat)
    out_bp  = h_p @ L          where h_p = ((X W)^T)[16p:16p+16, :]
With eps dropped (a ~3e-9 relative shift, far below the f32r matmul
noise), deg*dhat^2 == 1 exactly, so the diagonal term collapses:
    out[c, m] = (XW)[m, c] + bias[c] + dhat[m] * (gn @ A)[c, m]
with gn = -(XW)^T * dhat (broadcast along c).  t1 = XW + bias is a
setup-time constant; the per-strip critical chain is only
deg -> sqrt -> transpose -> (-1/sq) -> gn.  A streams from HBM exactly
once in column strips; column sums are computed from SBUF while it
loads, and the main matmul re-reads it from SBUF.

Output rows are staged in SBUF in a row-pair layout (partition r of
256-row group G holds rows 256G+2r and 256G+2r+1) so the store DMAs
move 512-byte contiguous chunks (256B chunks pay a 2x DMA penalty).
t1 is built in the same pair layout at setup time via strided-weight
matmuls.

Sharding: core b <- batch b (8 cores).  weight/bias are replicated; each
core gets Ans[b] ([4, 2048, 2048]) and X[b].  No collectives.

Matmuls over A run in float32r (relaxed fp32, full PE rate; output
free-dim kept >= 256 to stay at the 1 cycle/row rate); the tiny X@W /
broadcast matmuls run in exact fp32.  A loads as column strips
[512,512,512,256,256]; partial matmuls are emitted per (output strip,
row block) as soon as their dependencies (tiles + that column strip's
degree/dhat) are satisfied, so only the last 256 columns' worth of work
trails the final DMA.  deg accumulators are double-buffered in PSUM so
consecutive strips' column sums overlap, and in the endgame the last
strip's deg matmuls are ordered ahead of its backlog matmuls so the
scale chain starts as early as possible.
"""

import numpy as np

import concourse.bacc as bacc
import concourse.mybir as mybir
import concourse.tile as tile
from concourse.bass_utils import run_bass_kernel_spmd
from concourse.masks import make_identity
from concourse.tile_rust import add_dep_helper

F32 = mybir.dt.float32
BF16 = mybir.dt.bfloat16
MULT = mybir.AluOpType.mult
ADD = mybir.AluOpType.add

U = 64
UP = 16  # U // P

# build-time instruction labels for trace attribution (no program effect)
MM_LABELS = {}


def _lab(bi, label):
    MM_LABELS[bi.ins.name] = label
    return bi


def build(n=2048, n_slices=4, a_bufs=14):
    """Build the per-core SPMD program.

    n: graph size (multiple of 512), n_slices: number of P slices per core.
    """
    assert n % 512 == 0
    n_blocks = n // 128  # 128-row blocks (also m-blocks)
    n_groups = n // 256  # 256-row groups (store pair-layout units)

    nc = bacc.Bacc("TRN2", target_bir_lowering=False, debug=False)

    a_in = nc.dram_tensor("a_in", [n_slices, n, n], F32, kind="ExternalInput")
    x_in = nc.dram_tensor("x_in", [n, U], F32, kind="ExternalInput")
    w_in = nc.dram_tensor("w_in", [U, U], F32, kind="ExternalInput")
    b_in = nc.dram_tensor("b_in", [U], F32, kind="ExternalInput")
    out_d = nc.dram_tensor("out", [n, U], F32, kind="ExternalOutput")

    with tile.TileContext(nc) as tc:
        with (
            tc.tile_pool(name="consts", bufs=1) as consts,
            tc.tile_pool(name="work", bufs=2) as work,
            tc.tile_pool(name="apool", bufs=a_bufs) as apool,
        ):
            identity = consts.tile([128, 128], F32)
            make_identity(nc, identity[:])
            ones_col = consts.tile([128, 1], F32)
            nc.vector.memset(ones_col[:], 1.0)
            ones_bf = consts.tile([128, 1], BF16)
            nc.vector.tensor_copy(ones_bf[:], ones_col[:])
            ones_row = consts.tile([1, 128], F32)
            nc.vector.memset(ones_row[:], 1.0)
            # Issue the first A strip's DMA ahead of the setup loads so
            # the big stream starts immediately (XW isn't needed for ~15us).
            # A is cast to bf16 during the (gpsimd) DMA: the matmuls over A
            # already run relaxed, the error budget allows it, and the
            # SBUF-side transfer and footprint halve.
            pre_at = apool.tile([128, 16, 512], BF16, tag="A512", bufs=4, name="at_0_0")
            nc.gpsimd.dma_start(
                pre_at[:],
                a_in[0, 0:2048, 0:512].rearrange("(j r) c -> r j c", r=128),
            )

            # X loaded r-major (partition r holds rows 16r..16r+15) so the
            # DMA moves 4KiB contiguous chunks; the block-layout fixup
            # happens in the setup transposes below.
            xtile = consts.tile([128, n_blocks, U], F32)
            nc.sync.dma_start(
                xtile[:], x_in[:].rearrange("(r j) u -> r j u", j=n_blocks)
            )
            w_sb = consts.tile([U, U], F32)
            nc.sync.dma_start(w_sb[:], w_in[:])
            bias_row = consts.tile([1, U], F32)
            nc.sync.dma_start(bias_row[:], b_in[:].unsqueeze(0))

            # xw_sb column block kb holds (X @ W)[128*kb : 128*kb+128, :]
            # (standard block layout, feeds the gn scaling).
            # t1p column group G holds (XW + bias) rows 256G+2r+s in the
            # row-pair layout: t1p[r, 128G + 64s + u].
            xw_sb = consts.tile([128, n_blocks * U], F32)
            t1p = consts.tile([128, n_blocks * U], F32)
            bias_t = consts.tile([128, U], F32)
            # out staging, same pair layout as t1p
            out_sb = consts.tile([128, n_blocks * U], F32)
            # X^T staging: xts_wide[u, m] = X[m, u]
            xts_wide = work.tile([U, n], F32, tag="xts", bufs=1)

            with tc.tile_pool(name="psetup", bufs=2, space="PSUM") as psetup:
                # bias broadcast across partitions: ones_row^T @ bias_row
                pb = psetup.tile([128, U], F32, tag="pb")
                nc.tensor.matmul(pb[:], ones_row[:], bias_row[:], start=True, stop=True)
                nc.vector.tensor_copy(bias_t[:], pb[:])
                # X^T assembly: transposing the r-major chunk j gives X^T
                # columns {16r+j}; a stride-16 copy scatters them into place.
                xtv = xts_wide[:].rearrange("u (r j) -> u j r", j=n_blocks)
                for j in range(n_blocks):
                    pxt = psetup.tile([U, 128], F32, tag="pxt")
                    nc.tensor.transpose(pxt[:], xtile[:, j], identity[:])
                    nc.vector.tensor_copy(xtv[:, j], pxt[:])
                for kb in range(n_blocks):
                    pxw = psetup.tile([128, U], F32, tag="pxw")
                    nc.tensor.matmul(
                        pxw[:],
                        xts_wide[:, 128 * kb : 128 * kb + 128],
                        w_sb[:],
                        start=True,
                        stop=True,
                    )
                    nc.vector.tensor_copy(xw_sb[:, U * kb : U * kb + U], pxw[:])
                # pair-layout XW + bias: for 256-row group G, parity s the
                # stationary operand is X^T columns 256G+2x+s (x = 0..127),
                # so the matmul output partition r holds row 256G+2r+s.
                for G in range(n_groups):
                    xv = xts_wide[:, 256 * G : 256 * G + 256].rearrange(
                        "u (x s) -> u s x", s=2
                    )
                    for s in range(2):
                        pxw = psetup.tile([128, U], F32, tag="pxw")
                        nc.tensor.matmul(
                            pxw[:], xv[:, s], w_sb[:], start=True, stop=True
                        )
                        nc.vector.tensor_tensor(
                            t1p[:, 128 * G + 64 * s : 128 * G + 64 * s + 64],
                            pxw[:],
                            bias_t[:],
                            ADD,
                        )

            with tc.tile_pool(name="pmain", bufs=2, space="PSUM") as pmain:
                # Column strips; the last strips are narrower so only a small
                # amount of deg/matmul work depends on the final DMAs.  All
                # widths stay >= 256 to keep f32r matmuls at full rate.
                if n >= 2048:
                    widths = [512] * (n // 512 - 1) + [256, 256]
                else:
                    widths = [512] * (n // 512)
                offs = [sum(widths[:i]) for i in range(len(widths))]
                n_strip_list = list(zip(offs, widths))
                n_quads = n // 512  # 512-row groups
                n_strips = len(n_strip_list)

                for p in range(n_slices):
                    # ndhat: column kb holds -1/sqrt(deg) for rows 128kb+r
                    # (standard layout, feeds gn).  dhat_pr: column 2G+s
                    # holds +1/sqrt(deg) for rows 256G+2r+s (pair layout,
                    # feeds the output stt).
                    ndhat = work.tile([128, n_blocks], F32, tag="ndhat")
                    dhat_pr = work.tile([128, n_groups * 2], F32, tag="dhat_pr")
                    gT = work.tile([128, n_blocks * UP], BF16, tag="gT")
                    atiles = []
                    banks = [
                        pmain.tile(
                            [UP, 512], F32, tag=f"pmmb{bi}", bufs=1,
                            name=f"pmmb_{p}_{bi}",
                        )
                        for bi in range(n_strips)
                    ]

                    def pmm_view(t):
                        return banks[t][:, 0 : n_strip_list[t][1]]

                    started = [False] * n_strips
                    emitted = [0] * n_strips
                    scaled = [False] * n_strips
                    # In the last slice, strips other than the final one close
                    # their accumulation at block 13 so their scale work runs
                    # while the final strip still streams; blocks 14/15 are
                    # applied later as an A-stationary correction.
                    last_slice = p == n_slices - 1
                    stop_at = [
                        n_blocks - 2 if last_slice and t < n_strips - 1 else n_blocks
                        for t in range(n_strips)
                    ]

                    def emit_mm(t, nb):
                        # pmm_t += gT[block nb].T @ A[rows nb, strip t cols]
                        emitted[t] += 1
                        mm = _lab(nc.tensor.matmul(
                            pmm_view(t),
                            gT[:, UP * nb : UP * nb + UP],
                            atiles[t][:, nb],
                            start=not started[t],
                            stop=(emitted[t] == stop_at[t]),
                        ), f"mm_p{p}_t{t}_nb{nb}")
                        started[t] = True
                        return mm

                    pmt_count = [0]

                    def emit_scale(t):
                        # out strip t: out = t1 + dhat * Mneg^T  (gT carries
                        # the -1/sqrt(deg) factor, so Mneg = -g @ A), built
                        # per 256-row group G and row parity s in the pair
                        # layout.
                        off, w = n_strip_list[t]
                        msb = work.tile([UP, 512], F32, tag="msb", bufs=5, name=f"msb_{p}_{t}")
                        nc.scalar.copy(msb[0:UP, 0:w], pmm_view(t))
                        for gl in range(w // 256):
                            G = off // 256 + gl
                            mv = msb[0:UP, 256 * gl : 256 * gl + 256].rearrange(
                                "c (x s) -> c s x", s=2
                            )
                            for s in range(2):
                                k = pmt_count[0]
                                pmt_count[0] += 1
                                ptag, pbufs = ("ptr", 1) if k % 3 == 0 else ("pdeg", 2)
                                pmt = pmain.tile(
                                    [128, UP], F32, tag=ptag, bufs=pbufs,
                                    name=f"pmt_{p}_{t}_{gl}_{s}",
                                )
                                _lab(nc.tensor.transpose(
                                    pmt[:], mv[:, s], identity[0:UP, 0:UP]
                                ), f"pmtT_p{p}_t{t}_{gl}_{s}")
                                col = 128 * G + 64 * s + UP * p
                                nc.vector.scalar_tensor_tensor(
                                    out_sb[:, col : col + UP],
                                    pmt[:],
                                    dhat_pr[:, 2 * G + s : 2 * G + s + 1],
                                    t1p[:, col : col + UP],
                                    MULT,
                                    ADD,
                                )

                    ready_blocks = []
                    for si, (off, w) in enumerate(n_strip_list):
                        last_strip = si == n_strips - 1
                        endgame = last_strip and p == n_slices - 1
                        if p == 0 and si == 0:
                            at = pre_at
                        else:
                            at = apool.tile(
                                [128, n_blocks, w], BF16, tag=f"A{w}",
                                bufs=(4 if w == 512 else 3),
                                name=f"at_{p}_{si}",
                            )
                            src = a_in[p, :, off : off + w].rearrange(
                                "(j r) c -> r j c", r=128
                            )
                            if endgame:
                                # split the final row chunks so the last
                                # column-sum matmuls wait on 64KiB pieces only
                                nc.gpsimd.dma_start(
                                    at[:, 0 : n_blocks - 4], src[:, 0 : n_blocks - 4]
                                )
                                for jj in range(4):
                                    j = n_blocks - 4 + jj
                                    nc.gpsimd.dma_start(
                                        at[:, j : j + 1], src[:, j : j + 1]
                                    )
                            else:
                                nc.gpsimd.dma_start(at[:], src)
                        atiles.append(at)

                        # Column sums directly in column layout: for each
                        # 128-column chunk, deg_chunk[m] = A[:, chunk]^T @ ones
                        # with the A piece as the (cost-free) stationary
                        # operand and a single ones column moving.  pdeg
                        # columns 0..nj hold sqrt-input in standard layout
                        # (partition r of column c = deg[off+128c+r]); columns
                        # nj..2nj hold the row-pair layout (deg[off+256gl+2r+s]
                        # at column nj+2gl+s).  The accumulation groups on the
                        # shared tile serialize in emission order, which also
                        # pins the PE order deg-first in the endgame.
                        nj = w // 128
                        with tc.high_priority():
                            pdeg = pmain.tile(
                                [128, 2 * nj], F32, tag="pdeg", bufs=2,
                                padded_shape=[128, UP],
                                name=f"pdeg_{p}_{si}",
                            )

                        def deg_group(col, view_fn, label):
                            with tc.high_priority():
                                for i in range(16):
                                    _lab(nc.tensor.matmul(
                                        pdeg[:, col : col + 1],
                                        view_fn(i),
                                        ones_bf[:],
                                        start=(i == 0),
                                        stop=(i == 15),
                                    ), f"{label}_i{i}")

                        def std_view(c):
                            return lambda i: at[:, i, 128 * c : 128 * c + 128]

                        def pair_view(gl, s):
                            def f(i):
                                return at[
                                    :, i, 256 * gl : 256 * gl + 256
                                ].rearrange("r (x s) -> r s x", s=2)[:, s]

                            return f

                        if endgame:
                            # interleave the (tiny) deg groups with the
                            # strip's backlog matmuls in quad arrival order
                            deg_group(0, std_view(0), f"degS_p{p}_s{si}_c0")
                            for nb in ready_blocks:
                                emit_mm(si, nb)
                            for c in range(1, nj):
                                deg_group(c, std_view(c), f"degS_p{p}_s{si}_c{c}")
                            for gl in range(w // 256):
                                for s in range(2):
                                    deg_group(
                                        nj + 2 * gl + s, pair_view(gl, s),
                                        f"degP_p{p}_s{si}_g{gl}_{s}",
                                    )
                        else:
                            for c in range(nj):
                                deg_group(c, std_view(c), f"degS_p{p}_s{si}_c{c}")
                            for gl in range(w // 256):
                                for s in range(2):
                                    deg_group(
                                        nj + 2 * gl + s, pair_view(gl, s),
                                        f"degP_p{p}_s{si}_g{gl}_{s}",
                                    )
                        with tc.high_priority():
                            sq_cols = work.tile(
                                [128, 2 * nj], F32, tag="sq_cols",
                                padded_shape=[128, UP],
                                name=f"sq_cols_{p}_{si}",
                            )
                            nc.scalar.sqrt(sq_cols[:], pdeg[:])
                            b0 = off // 128
                            cs = slice(b0, b0 + nj)
                            # dhat = +1/sqrt(deg) gates gn: emit its consumers
                            # (the gn muls) before the pair-layout reciprocal
                            # so DVE order favours the critical path.
                            nc.vector.reciprocal(ndhat[:, cs], sq_cols[:, 0:nj])
                            new_blocks = list(range(b0, b0 + nj))
                            for kb in new_blocks:
                                nc.vector.tensor_scalar_mul(
                                    gT[:, UP * kb : UP * kb + UP],
                                    xw_sb[:, U * kb + UP * p : U * kb + UP * p + UP],
                                    ndhat[:, kb : kb + 1],
                                )
                        # pair-layout -1/sqrt(deg) (off the critical
                        # path; only the late stt reads it)
                        g0 = off // 256
                        pv = dhat_pr[:, 2 * g0 : 2 * g0 + w // 128]
                        nc.vector.reciprocal(pv, sq_cols[:, nj : 2 * nj])
                        nc.vector.tensor_scalar_mul(pv, pv, -1.0)

                        # Partial matmuls that just became ready.  The new
                        # strip's backlog (old gT blocks x new tiles) only
                        # needs the tiles, so emit it before the matmuls
                        # gated on this strip's deg chain.
                        if not endgame:
                            for nb in ready_blocks:
                                emit_mm(si, nb)
                        if endgame:
                            # only the final strip still accumulates the last
                            # two blocks in its main (gn-stationary) group
                            for nb in new_blocks:
                                emit_mm(si, nb)
                        elif last_strip:
                            for t in [si, si - 1] + list(range(si - 1)):
                                for nb in new_blocks:
                                    emit_mm(t, nb)
                        else:
                            for t in range(si):
                                for nb in new_blocks:
                                    emit_mm(t, nb)
                            for nb in new_blocks:
                                emit_mm(si, nb)
                        ready_blocks += new_blocks
                        if not endgame:
                            with tc.high_priority():
                                for t in range(n_strips):
                                    if emitted[t] == stop_at[t] and not scaled[t]:
                                        scaled[t] = True
                                        emit_scale(t)

                        if endgame:
                            # Correction pass: add dhat * (gn[14:16] @ A)^T for
                            # strips 0..3, computed per 256-row pair group with
                            # the A piece as the (cost-free) stationary operand
                            # so the output lands directly in the pair layout.
                            # PSUM comes from the now-free main banks.
                            def corr_unit(G, s, bank):
                                t = next(
                                    tt
                                    for tt, (o, ww) in enumerate(n_strip_list)
                                    if o <= 256 * G < o + ww
                                )
                                o, ww = n_strip_list[t]
                                gl = (256 * G - o) // 256
                                corr = pmain.tile(
                                    [128, UP], F32, tag=f"pmmb{bank}", bufs=1,
                                    name=f"corr_{G}_{s}",
                                )
                                for k, i in enumerate((n_blocks - 2, n_blocks - 1)):
                                    lhs = atiles[t][
                                        :, i, 256 * gl : 256 * gl + 256
                                    ].rearrange("r (x s) -> r s x", s=2)[:, s]
                                    _lab(nc.tensor.matmul(
                                        corr[:],
                                        lhs,
                                        gT[:, UP * i : UP * i + UP],
                                        start=(k == 0),
                                        stop=(k == 1),
                                    ), f"corr_p{p}_G{G}_s{s}_i{i}")
                                col = 128 * G + 64 * s + UP * p
                                nc.vector.scalar_tensor_tensor(
                                    out_sb[:, col : col + UP],
                                    corr[:],
                                    dhat_pr[:, 2 * G + s : 2 * G + s + 1],
                                    out_sb[:, col : col + UP],
                                    MULT,
                                    ADD,
                                )

                            def store_group(g):
                                dst = out_d[512 * g : 512 * g + 512, :].rearrange(
                                    "(j r s) u -> r j s u", r=128, s=2
                                )
                                src_sb = out_sb[
                                    :, 256 * g : 256 * g + 256
                                ].rearrange("r (j s u) -> r j s u", j=2, s=2)
                                nc.sync.dma_start(dst, src_sb)

                            nbank = [0]

                            def next_bank():
                                nbank[0] = (nbank[0] + 1) % (n_strips - 1)
                                return nbank[0]

                            # Correction-unit order: tail strips' groups
                            # first (their scale work finished earliest), then
                            # walk backwards so each store's units complete in
                            # sequence; the last store is the smallest chunk
                            # still waiting on the DVE stream's tail.
                            for s in range(2):
                                corr_unit(n_groups - 2, s, next_bank())
                            emit_scale(si)  # final strip: full M in one pass
                            for G in (4, 5):
                                for s in range(2):
                                    corr_unit(G, s, next_bank())
                            # rows 1024..2048 ready: one 256KiB store
                            dst = out_d[1024:2048, :].rearrange(
                                "(j r s) u -> r j s u", r=128, s=2
                            )
                            nc.sync.dma_start(
                                dst,
                                out_sb[:, 512:1024].rearrange(
                                    "r (j s u) -> r j s u", j=4, s=2
                                ),
                            )
                            for G in (2, 3):
                                for s in range(2):
                                    corr_unit(G, s, next_bank())
                            store_group(1)
                            for G in (0, 1):
                                for s in range(2):
                                    corr_unit(G, s, next_bank())
                            store_group(0)

    nc.compile()
    return nc


_NC_CACHE = {}


def _get_nc():
    if "nc" not in _NC_CACHE:
        _NC_CACHE["nc"] = build()
    return _NC_CACHE["nc"]


def kernel(Ans, X, weight, bias):
    Ans = np.ascontiguousarray(Ans, dtype=np.float32)
    X = np.ascontiguousarray(X, dtype=np.float32)
    weight = np.ascontiguousarray(weight, dtype=np.float32)
    bias = np.ascontiguousarray(bias, dtype=np.float32)

    nc = _get_nc()
    in_maps = [
        {"a_in": Ans[b], "x_in": X[b], "w_in": weight, "b_in": bias}
        for b in range(Ans.shape[0])
    ]
    res = run_bass_kernel_spmd(nc, in_maps, core_ids=list(range(len(in_maps))))
    return np.stack([r["out"] for r in res.results], axis=0)


# revision 33
# speedup vs baseline: 1.9124x; 1.1451x over previous
"""GCNConv Trainium2 kernel.

Per (b, p) slice of Ans [B, P, n, n] the reference computes
    deg[m]  = sum_i A[i, m]                 (column sums)
    dhat    = 1 / (sqrt(deg) + eps)
    L       = diag(dhat) (diag(deg) - A) diag(dhat)
    out_bp  = h_p @ L          where h_p = ((X W)^T)[16p:16p+16, :]
With eps dropped (a ~3e-9 relative shift, far below the matmul noise),
deg*dhat^2 == 1 exactly, so the diagonal term collapses:
    out[c, m] = (XW)[m, c] + bias[c] + dhat[m] * (gn @ A)[c, m]
with gn = -(XW)^T * dhat.  t1 = XW + bias is a setup-time constant.

Key structure:
- A is cast to bf16 during the (gpsimd/SWDGE) DMA: the products over A
  are relaxed anyway and the error budget is wide, while the SBUF-side
  transfer and footprint halve.  One [128, 16, w] tile per column strip
  (widths 512,512,512,256,256), double-slice buffering so the stream
  never stalls on tile reuse.
- Column sums run with the A pieces as the (stationary) matmul operand
  and a single ones column moving, producing deg directly in both the
  standard column layout (feeds gn) and the row-pair layout (feeds the
  output scaling), eliminating all transpose round-trips from the
  per-strip chain: deg -> sqrt -> reciprocal -> gn.
- Main matmuls keep gn stationary, accumulating M = gn @ A per strip in
  [16, w] PSUM banks.  In the last slice all strips but the final one
  close their accumulation at block 11; the mid-stream scale pass
  (PSUM copy -> pair-layout PE transposes -> stt with dhat) runs while
  the final strips still stream, and the last four blocks are applied
  afterwards as an A-stationary correction directly in the row-pair
  layout ([128, 16] PSUM chunks rotating through the freed main banks),
  so the post-stream tail is only: deg tail -> sqrt -> reciprocal -> gn
  -> correction matmuls -> stt accumulate -> stores.
- Output is staged in SBUF in a row-pair layout (partition r of 256-row
  group G holds rows 256G+2r and 256G+2r+1) so the store DMAs move
  512-byte contiguous chunks; t1 is built in the same layout at setup
  time via strided-weight matmuls.  X is loaded r-major (4KiB chunks)
  and repacked with transposes + strided copies.
- The endgame static PE order is pinned with ordering-only deps so the
  near-free column-sum matmuls always precede the final strip's backlog.

Sharding: core b <- batch b (8 cores).  weight/bias are replicated; each
core gets Ans[b] ([4, 2048, 2048]) and X[b].  No collectives.
"""

import numpy as np

import concourse.bacc as bacc
import concourse.mybir as mybir
import concourse.tile as tile
from concourse.bass_utils import run_bass_kernel_spmd
from concourse.masks import make_identity
from concourse.tile_rust import add_dep_helper

F32 = mybir.dt.float32
BF16 = mybir.dt.bfloat16
MULT = mybir.AluOpType.mult
ADD = mybir.AluOpType.add

U = 64
UP = 16  # U // P

# build-time instruction labels for trace attribution (no program effect)
MM_LABELS = {}


def _lab(bi, label):
    MM_LABELS[bi.ins.name] = label
    return bi


def build(n=2048, n_slices=4, a_bufs=14):
    """Build the per-core SPMD program.

    n: graph size (multiple of 512), n_slices: number of P slices per core.
    """
    assert n % 512 == 0
    n_blocks = n // 128  # 128-row blocks (also m-blocks)
    n_groups = n // 256  # 256-row groups (store pair-layout units)

    nc = bacc.Bacc("TRN2", target_bir_lowering=False, debug=False)

    a_in = nc.dram_tensor("a_in", [n_slices, n, n], F32, kind="ExternalInput")
    x_in = nc.dram_tensor("x_in", [n, U], F32, kind="ExternalInput")
    w_in = nc.dram_tensor("w_in", [U, U], F32, kind="ExternalInput")
    b_in = nc.dram_tensor("b_in", [U], F32, kind="ExternalInput")
    out_d = nc.dram_tensor("out", [n, U], F32, kind="ExternalOutput")

    with tile.TileContext(nc) as tc:
        with (
            tc.tile_pool(name="consts", bufs=1) as consts,
            tc.tile_pool(name="work", bufs=2) as work,
            tc.tile_pool(name="apool", bufs=a_bufs) as apool,
        ):
            identity = consts.tile([128, 128], F32)
            make_identity(nc, identity[:])
            ones_col = consts.tile([128, 1], F32)
            nc.vector.memset(ones_col[:], 1.0)
            ones_bf = consts.tile([128, 1], BF16)
            nc.vector.tensor_copy(ones_bf[:], ones_col[:])
            ones_row = consts.tile([1, 128], F32)
            nc.vector.memset(ones_row[:], 1.0)
            # Issue the first A strip's DMA ahead of the setup loads so
            # the big stream starts immediately (XW isn't needed for ~15us).
            # A is cast to bf16 during the (gpsimd) DMA: the matmuls over A
            # already run relaxed, the error budget allows it, and the
            # SBUF-side transfer and footprint halve.
            pre_at = apool.tile([128, 16, 512], BF16, tag="A512", bufs=7, name="at_0_0")
            nc.gpsimd.dma_start(
                pre_at[:],
                a_in[0, 0:2048, 0:512].rearrange("(j r) c -> r j c", r=128),
            )

            # X loaded r-major (partition r holds rows 16r..16r+15) so the
            # DMA moves 4KiB contiguous chunks; the block-layout fixup
            # happens in the setup transposes below.
            xtile = consts.tile([128, n_blocks, U], F32)
            nc.sync.dma_start(
                xtile[:], x_in[:].rearrange("(r j) u -> r j u", j=n_blocks)
            )
            w_sb = consts.tile([U, U], F32)
            nc.sync.dma_start(w_sb[:], w_in[:])
            bias_row = consts.tile([1, U], F32)
            nc.sync.dma_start(bias_row[:], b_in[:].unsqueeze(0))

            # xw_sb column block kb holds (X @ W)[128*kb : 128*kb+128, :]
            # (standard block layout, feeds the gn scaling).
            # t1p column group G holds (XW + bias) rows 256G+2r+s in the
            # row-pair layout: t1p[r, 128G + 64s + u].
            xw_sb = consts.tile([128, n_blocks * U], F32)
            t1p = consts.tile([128, n_blocks * U], F32)
            bias_t = consts.tile([128, U], F32)
            # out staging, same pair layout as t1p
            out_sb = consts.tile([128, n_blocks * U], F32)
            # X^T staging: xts_wide[u, m] = X[m, u]
            xts_wide = work.tile([U, n], F32, tag="xts", bufs=1)

            with tc.tile_pool(name="psetup", bufs=2, space="PSUM") as psetup:
                # bias broadcast across partitions: ones_row^T @ bias_row
                pb = psetup.tile([128, U], F32, tag="pb")
                nc.tensor.matmul(pb[:], ones_row[:], bias_row[:], start=True, stop=True)
                nc.vector.tensor_copy(bias_t[:], pb[:])
                # X^T assembly: transposing the r-major chunk j gives X^T
                # columns {16r+j}; a stride-16 copy scatters them into place.
                xtv = xts_wide[:].rearrange("u (r j) -> u j r", j=n_blocks)
                for j in range(n_blocks):
                    pxt = psetup.tile([U, 128], F32, tag="pxt")
                    nc.tensor.transpose(pxt[:], xtile[:, j], identity[:])
                    nc.vector.tensor_copy(xtv[:, j], pxt[:])
                for kb in range(n_blocks):
                    pxw = psetup.tile([128, U], F32, tag="pxw")
                    nc.tensor.matmul(
                        pxw[:],
                        xts_wide[:, 128 * kb : 128 * kb + 128],
                        w_sb[:],
                        start=True,
                        stop=True,
                    )
                    nc.vector.tensor_copy(xw_sb[:, U * kb : U * kb + U], pxw[:])
                # pair-layout XW + bias: for 256-row group G, parity s the
                # stationary operand is X^T columns 256G+2x+s (x = 0..127),
                # so the matmul output partition r holds row 256G+2r+s.
                for G in range(n_groups):
                    xv = xts_wide[:, 256 * G : 256 * G + 256].rearrange(
                        "u (x s) -> u s x", s=2
                    )
                    for s in range(2):
                        pxw = psetup.tile([128, U], F32, tag="pxw")
                        nc.tensor.matmul(
                            pxw[:], xv[:, s], w_sb[:], start=True, stop=True
                        )
                        nc.vector.tensor_tensor(
                            t1p[:, 128 * G + 64 * s : 128 * G + 64 * s + 64],
                            pxw[:],
                            bias_t[:],
                            ADD,
                        )

            with tc.tile_pool(name="pmain", bufs=2, space="PSUM") as pmain:
                # Column strips; the last strips are narrower so only a small
                # amount of deg/matmul work depends on the final DMAs.  All
                # widths stay >= 256 to keep f32r matmuls at full rate.
                if n >= 2048:
                    widths = [512] * (n // 512 - 1) + [256, 256]
                else:
                    widths = [512] * (n // 512)
                offs = [sum(widths[:i]) for i in range(len(widths))]
                n_strip_list = list(zip(offs, widths))
                n_quads = n // 512  # 512-row groups
                n_strips = len(n_strip_list)

                for p in range(n_slices):
                    # ndhat: column kb holds -1/sqrt(deg) for rows 128kb+r
                    # (standard layout, feeds gn).  dhat_pr: column 2G+s
                    # holds +1/sqrt(deg) for rows 256G+2r+s (pair layout,
                    # feeds the output stt).
                    ndhat = work.tile([128, n_blocks], F32, tag="ndhat")
                    dhat_pr = work.tile([128, n_groups * 2], F32, tag="dhat_pr")
                    gT = work.tile([128, n_blocks * UP], BF16, tag="gT")
                    atiles = []
                    banks = [
                        pmain.tile(
                            [UP, 512], F32, tag=f"pmmb{bi}", bufs=1,
                            name=f"pmmb_{p}_{bi}",
                        )
                        for bi in range(n_strips)
                    ]

                    def pmm_view(t):
                        return banks[t][:, 0 : n_strip_list[t][1]]

                    started = [False] * n_strips
                    emitted = [0] * n_strips
                    scaled = [False] * n_strips
                    # In the last slice, strips other than the final one close
                    # their accumulation at block 13 so their scale work runs
                    # while the final strip still streams; blocks 14/15 are
                    # applied later as an A-stationary correction.
                    last_slice = p == n_slices - 1
                    stop_at = [
                        n_blocks - 4 if last_slice and t < n_strips - 1 else n_blocks
                        for t in range(n_strips)
                    ]

                    def emit_mm(t, nb):
                        # pmm_t += gT[block nb].T @ A[rows nb, strip t cols]
                        emitted[t] += 1
                        mm = _lab(nc.tensor.matmul(
                            pmm_view(t),
                            gT[:, UP * nb : UP * nb + UP],
                            atiles[t][:, nb],
                            start=not started[t],
                            stop=(emitted[t] == stop_at[t]),
                        ), f"mm_p{p}_t{t}_nb{nb}")
                        started[t] = True
                        return mm

                    pmt_count = [0]

                    def emit_scale(t, after=None):
                        # out strip t: out = t1 + dhat * Mneg^T  (gT carries
                        # the -1/sqrt(deg) factor, so Mneg = -g @ A), built
                        # per 256-row group G and row parity s in the pair
                        # layout.
                        off, w = n_strip_list[t]
                        msb = work.tile([UP, 512], F32, tag="msb", bufs=5, name=f"msb_{p}_{t}")
                        nc.scalar.copy(msb[0:UP, 0:w], pmm_view(t))
                        for gl in range(w // 256):
                            G = off // 256 + gl
                            mv = msb[0:UP, 256 * gl : 256 * gl + 256].rearrange(
                                "c (x s) -> c s x", s=2
                            )
                            for s in range(2):
                                k = pmt_count[0]
                                pmt_count[0] += 1
                                pmt = pmain.tile(
                                    [128, UP], F32, tag="pmtb", bufs=2,
                                    name=f"pmt_{p}_{t}_{gl}_{s}",
                                )
                                tr = _lab(nc.tensor.transpose(
                                    pmt[:], mv[:, s], identity[0:UP, 0:UP]
                                ), f"pmtT_p{p}_t{t}_{gl}_{s}")
                                if after is not None:
                                    add_dep_helper(
                                        tr.ins, after.ins, sync=False,
                                        reason="scale transposes after endgame deg",
                                    )
                                col = 128 * G + 64 * s + UP * p
                                nc.vector.scalar_tensor_tensor(
                                    out_sb[:, col : col + UP],
                                    pmt[:],
                                    dhat_pr[:, 2 * G + s : 2 * G + s + 1],
                                    t1p[:, col : col + UP],
                                    MULT,
                                    ADD,
                                )

                    ready_blocks = []
                    for si, (off, w) in enumerate(n_strip_list):
                        last_strip = si == n_strips - 1
                        endgame = last_strip and p == n_slices - 1
                        if p == 0 and si == 0:
                            at = pre_at
                        else:
                            at = apool.tile(
                                [128, n_blocks, w], BF16, tag=f"A{w}",
                                bufs=(7 if w == 512 else 5),
                                name=f"at_{p}_{si}",
                            )
                            src = a_in[p, :, off : off + w].rearrange(
                                "(j r) c -> r j c", r=128
                            )
                            if endgame:
                                # split the final row chunks so the last
                                # column-sum matmuls wait on 64KiB pieces only
                                nc.gpsimd.dma_start(
                                    at[:, 0 : n_blocks - 4], src[:, 0 : n_blocks - 4]
                                )
                                for jj in range(4):
                                    j = n_blocks - 4 + jj
                                    nc.gpsimd.dma_start(
                                        at[:, j : j + 1], src[:, j : j + 1]
                                    )
                            else:
                                nc.gpsimd.dma_start(at[:], src)
                        atiles.append(at)

                        # Column sums directly in column layout: for each
                        # 128-column chunk, deg_chunk[m] = A[:, chunk]^T @ ones
                        # with the A piece as the (cost-free) stationary
                        # operand and a single ones column moving.  pdeg
                        # columns 0..nj hold sqrt-input in standard layout
                        # (partition r of column c = deg[off+128c+r]); columns
                        # nj..2nj hold the row-pair layout (deg[off+256gl+2r+s]
                        # at column nj+2gl+s).  The accumulation groups on the
                        # shared tile serialize in emission order, which also
                        # pins the PE order deg-first in the endgame.
                        nj = w // 128
                        with tc.high_priority():
                            pdeg = pmain.tile(
                                [128, 2 * nj], F32, tag="pdeg", bufs=1,
                                padded_shape=[128, UP],
                                name=f"pdeg_{p}_{si}",
                            )

                        last_deg = [None]

                        def deg_group(col, view_fn, label):
                            with tc.high_priority():
                                for i in range(16):
                                    last_deg[0] = _lab(nc.tensor.matmul(
                                        pdeg[:, col : col + 1],
                                        view_fn(i),
                                        ones_bf[:],
                                        start=(i == 0),
                                        stop=(i == 15),
                                    ), f"{label}_i{i}")

                        def std_view(c):
                            return lambda i: at[:, i, 128 * c : 128 * c + 128]

                        def pair_view(gl, s):
                            def f(i):
                                return at[
                                    :, i, 256 * gl : 256 * gl + 256
                                ].rearrange("r (x s) -> r s x", s=2)[:, s]

                            return f

                        for c in range(nj):
                            deg_group(c, std_view(c), f"degS_p{p}_s{si}_c{c}")
                        for gl in range(w // 256):
                            for s in range(2):
                                deg_group(
                                    nj + 2 * gl + s, pair_view(gl, s),
                                    f"degP_p{p}_s{si}_g{gl}_{s}",
                                )
                        if endgame:
                            # the (near-free) deg groups must precede the
                            # strip's backlog and the deferred strip-3 scale
                            # in the static PE order so the scale chain starts
                            # right after the last byte
                            with tc.high_priority():
                                scaled[si - 1] = True
                                emit_scale(si - 1, after=last_deg[0])
                            for nb in ready_blocks:
                                mm = emit_mm(si, nb)
                                add_dep_helper(
                                    mm.ins, last_deg[0].ins, sync=False,
                                    reason="endgame: deg before backlog",
                                )
                        with tc.high_priority():
                            sq_cols = work.tile(
                                [128, 2 * nj], F32, tag="sq_cols",
                                padded_shape=[128, UP],
                                name=f"sq_cols_{p}_{si}",
                            )
                            nc.scalar.sqrt(sq_cols[:], pdeg[:])
                            b0 = off // 128
                            cs = slice(b0, b0 + nj)
                            # dhat = +1/sqrt(deg) gates gn: emit its consumers
                            # (the gn muls) before the pair-layout reciprocal
                            # so DVE order favours the critical path.
                            nc.vector.reciprocal(ndhat[:, cs], sq_cols[:, 0:nj])
                            new_blocks = list(range(b0, b0 + nj))
                            for kb in new_blocks:
                                nc.vector.tensor_scalar_mul(
                                    gT[:, UP * kb : UP * kb + UP],
                                    xw_sb[:, U * kb + UP * p : U * kb + UP * p + UP],
                                    ndhat[:, kb : kb + 1],
                                )
                        # pair-layout -1/sqrt(deg) (off the critical
                        # path; only the late stt reads it)
                        g0 = off // 256
                        pv = dhat_pr[:, 2 * g0 : 2 * g0 + w // 128]
                        nc.vector.reciprocal(pv, sq_cols[:, nj : 2 * nj])
                        nc.vector.tensor_scalar_mul(pv, pv, -1.0)

                        # Partial matmuls that just became ready.  The new
                        # strip's backlog (old gT blocks x new tiles) only
                        # needs the tiles, so emit it before the matmuls
                        # gated on this strip's deg chain.
                        if not endgame:
                            for nb in ready_blocks:
                                emit_mm(si, nb)
                        if endgame:
                            # only the final strip still accumulates the last
                            # blocks in its main (gn-stationary) group
                            for nb in new_blocks:
                                emit_mm(si, nb)
                        elif last_strip:
                            for t in [si, si - 1] + list(range(si - 1)):
                                for nb in new_blocks:
                                    if nb < stop_at[t]:
                                        emit_mm(t, nb)
                        else:
                            for t in range(si):
                                for nb in new_blocks:
                                    if nb < stop_at[t]:
                                        emit_mm(t, nb)
                            for nb in new_blocks:
                                if nb < stop_at[si]:
                                    emit_mm(si, nb)
                        ready_blocks += new_blocks
                        if not endgame:
                            with tc.high_priority():
                                for t in range(n_strips):
                                    if emitted[t] == stop_at[t] and not scaled[t]:
                                        if last_slice and t == n_strips - 2:
                                            # deferred into the endgame so its
                                            # PE transposes order after the
                                            # final strip's column sums
                                            continue
                                        scaled[t] = True
                                        emit_scale(t)

                        if endgame:
                            # Correction pass: add dhat * (gn[14:16] @ A)^T for
                            # strips 0..3, computed per 256-row pair group with
                            # the A piece as the (cost-free) stationary operand
                            # so the output lands directly in the pair layout.
                            # PSUM comes from the now-free main banks.
                            def corr_unit(G, s, bank):
                                t = next(
                                    tt
                                    for tt, (o, ww) in enumerate(n_strip_list)
                                    if o <= 256 * G < o + ww
                                )
                                o, ww = n_strip_list[t]
                                gl = (256 * G - o) // 256
                                corr = pmain.tile(
                                    [128, UP], F32, tag=f"pmmb{bank}", bufs=1,
                                    name=f"corr_{G}_{s}",
                                )
                                for k, i in enumerate(range(n_blocks - 4, n_blocks)):
                                    lhs = atiles[t][
                                        :, i, 256 * gl : 256 * gl + 256
                                    ].rearrange("r (x s) -> r s x", s=2)[:, s]
                                    _lab(nc.tensor.matmul(
                                        corr[:],
                                        lhs,
                                        gT[:, UP * i : UP * i + UP],
                                        start=(k == 0),
                                        stop=(k == 3),
                                    ), f"corr_p{p}_G{G}_s{s}_i{i}")
                                col = 128 * G + 64 * s + UP * p
                                nc.vector.scalar_tensor_tensor(
                                    out_sb[:, col : col + UP],
                                    corr[:],
                                    dhat_pr[:, 2 * G + s : 2 * G + s + 1],
                                    out_sb[:, col : col + UP],
                                    MULT,
                                    ADD,
                                )

                            def store_group(g):
                                dst = out_d[512 * g : 512 * g + 512, :].rearrange(
                                    "(j r s) u -> r j s u", r=128, s=2
                                )
                                src_sb = out_sb[
                                    :, 256 * g : 256 * g + 256
                                ].rearrange("r (j s u) -> r j s u", j=2, s=2)
                                nc.sync.dma_start(dst, src_sb)

                            nbank = [0]

                            def next_bank():
                                nbank[0] = (nbank[0] + 1) % (n_strips - 1)
                                return nbank[0]

                            # Correction-unit order: tail strips' groups
                            # first (their scale work finished earliest), then
                            # walk backwards so each store's units complete in
                            # sequence; the last store is the smallest chunk
                            # still waiting on the DVE stream's tail.
                            for s in range(2):
                                corr_unit(n_groups - 2, s, next_bank())
                            emit_scale(si)  # final strip: full M in one pass
                            for G in (4, 5):
                                for s in range(2):
                                    corr_unit(G, s, next_bank())
                            # rows 1024..2048 ready: one 256KiB store
                            dst = out_d[1024:2048, :].rearrange(
                                "(j r s) u -> r j s u", r=128, s=2
                            )
                            nc.sync.dma_start(
                                dst,
                                out_sb[:, 512:1024].rearrange(
                                    "r (j s u) -> r j s u", j=4, s=2
                                ),
                            )
                            for G in (2, 3):
                                for s in range(2):
                                    corr_unit(G, s, next_bank())
                            store_group(1)
                            for G in (0, 1):
                                for s in range(2):
                                    corr_unit(G, s, next_bank())
                            store_group(0)

    nc.compile()
    return nc


_NC_CACHE = {}


def _get_nc():
    if "nc" not in _NC_CACHE:
        _NC_CACHE["nc"] = build()
    return _NC_CACHE["nc"]


def kernel(Ans, X, weight, bias):
    Ans = np.ascontiguousarray(Ans, dtype=np.float32)
    X = np.ascontiguousarray(X, dtype=np.float32)
    weight = np.ascontiguousarray(weight, dtype=np.float32)
    bias = np.ascontiguousarray(bias, dtype=np.float32)

    nc = _get_nc()
    in_maps = [
        {"a_in": Ans[b], "x_in": X[b], "w_in": weight, "b_in": bias}
        for b in range(Ans.shape[0])
    ]
    res = run_bass_kernel_spmd(nc, in_maps, core_ids=list(range(len(in_maps))))
    return np.stack([r["out"] for r in res.results], axis=0)


# revision 36
# speedup vs baseline: 2.2249x; 1.1634x over previous
"""GCNConv Trainium2 kernel.

Per (b, p) slice of Ans [B, P, n, n] the reference computes
    deg[m]  = sum_i A[i, m]                 (column sums)
    dhat    = 1 / (sqrt(deg) + eps)
    L       = diag(dhat) (diag(deg) - A) diag(dhat)
    out_bp  = h_p @ L          where h_p = ((X W)^T)[16p:16p+16, :]
With eps dropped (a ~3e-9 relative shift, far below the matmul noise),
deg*dhat^2 == 1 exactly, so the diagonal term collapses:
    out[c, m] = (XW)[m, c] + bias[c] + dhat[m] * (gn @ A)[c, m]
with gn = -(XW)^T * dhat.  t1 = XW + bias is a setup-time constant.

Key structure:
- A is cast to fp8 (e4m3) during the (gpsimd/SWDGE) DMA: the products
  over A are relaxed anyway and the fp8 rounding, scaled by dhat against
  the exact-f32 t1 term, lands ~25x inside the error budget, while the
  SBUF-side transfer time and footprint drop 4x vs f32.  gn is scaled by
  64 to sit in fp8e4m3's normal range (the output stt compensates).  One
  [128, 16, 512] tile per column strip (4 strips of 512, keeping every
  DMA chunk at the 512-byte full-rate boundary), three slices of tile
  buffering so the stream runs ahead of the (now PE-bound) matmuls.
- Column sums run with the A pieces as the (stationary) matmul operand
  and a single ones column moving, producing deg directly in both the
  standard column layout (feeds gn) and the row-pair layout (feeds the
  output scaling), eliminating all transpose round-trips from the
  per-strip chain: deg -> sqrt -> reciprocal -> gn.
- Main matmuls keep gn stationary, accumulating M = gn @ A per strip in
  [16, w] PSUM banks.  In the last slice all strips but the final one
  close their accumulation at block 11; the mid-stream scale pass
  (PSUM copy -> pair-layout PE transposes -> stt with dhat) runs while
  the final strips still stream, and the last four blocks are applied
  afterwards as an A-stationary correction directly in the row-pair
  layout ([128, 16] PSUM chunks rotating through the freed main banks),
  so the post-stream tail is only: deg tail -> sqrt -> reciprocal -> gn
  -> correction matmuls -> stt accumulate -> stores.
- Output is staged in SBUF in a row-pair layout (partition r of 256-row
  group G holds rows 256G+2r and 256G+2r+1) so the store DMAs move
  512-byte contiguous chunks; t1 is built in the same layout at setup
  time via strided-weight matmuls.  X is loaded r-major (4KiB chunks)
  and repacked with transposes + strided copies.
- The endgame static PE order is pinned with ordering-only deps so the
  near-free column-sum matmuls always precede the final strip's backlog.

Sharding: core b <- batch b (8 cores).  weight/bias are replicated; each
core gets Ans[b] ([4, 2048, 2048]) and X[b].  No collectives.
"""

import numpy as np

import concourse.bacc as bacc
import concourse.mybir as mybir
import concourse.tile as tile
from concourse.bass_utils import run_bass_kernel_spmd
from concourse.masks import make_identity
from concourse.tile_rust import add_dep_helper

F32 = mybir.dt.float32
BF16 = mybir.dt.bfloat16
FP8 = mybir.dt.float8e4
GSCALE = 64.0  # keeps gn in fp8e4m3's normal range; stt compensates
MULT = mybir.AluOpType.mult
ADD = mybir.AluOpType.add

U = 64
UP = 16  # U // P

# build-time instruction labels for trace attribution (no program effect)
MM_LABELS = {}


def _lab(bi, label):
    MM_LABELS[bi.ins.name] = label
    return bi


def build(n=2048, n_slices=4, a_bufs=14):
    """Build the per-core SPMD program.

    n: graph size (multiple of 512), n_slices: number of P slices per core.
    """
    assert n % 512 == 0
    n_blocks = n // 128  # 128-row blocks (also m-blocks)
    n_groups = n // 256  # 256-row groups (store pair-layout units)

    nc = bacc.Bacc("TRN2", target_bir_lowering=False, debug=False)

    a_in = nc.dram_tensor("a_in", [n_slices, n, n], F32, kind="ExternalInput")
    x_in = nc.dram_tensor("x_in", [n, U], F32, kind="ExternalInput")
    w_in = nc.dram_tensor("w_in", [U, U], F32, kind="ExternalInput")
    b_in = nc.dram_tensor("b_in", [U], F32, kind="ExternalInput")
    out_d = nc.dram_tensor("out", [n, U], F32, kind="ExternalOutput")

    with tile.TileContext(nc) as tc:
        with (
            tc.tile_pool(name="consts", bufs=1) as consts,
            tc.tile_pool(name="work", bufs=2) as work,
            tc.tile_pool(name="apool", bufs=a_bufs) as apool,
        ):
            identity = consts.tile([128, 128], F32)
            make_identity(nc, identity[:])
            ones_col = consts.tile([128, 1], F32)
            nc.vector.memset(ones_col[:], 1.0)
            ones_bf = consts.tile([128, 1], FP8)
            nc.vector.tensor_copy(ones_bf[:], ones_col[:])
            ones_row = consts.tile([1, 128], F32)
            nc.vector.memset(ones_row[:], 1.0)
            # Issue the first A strip's DMA ahead of the setup loads so
            # the big stream starts immediately (XW isn't needed for ~15us).
            # A is cast to bf16 during the (gpsimd) DMA: the matmuls over A
            # already run relaxed, the error budget allows it, and the
            # SBUF-side transfer and footprint halve.
            pre_at = apool.tile([128, 16, 512], FP8, tag="A512", bufs=13, name="at_0_0")
            nc.gpsimd.dma_start(
                pre_at[:],
                a_in[0, 0:2048, 0:512].rearrange("(j r) c -> r j c", r=128),
            )

            # X loaded r-major (partition r holds rows 16r..16r+15) so the
            # DMA moves 4KiB contiguous chunks; the block-layout fixup
            # happens in the setup transposes below.
            xtile = consts.tile([128, n_blocks, U], F32)
            nc.sync.dma_start(
                xtile[:], x_in[:].rearrange("(r j) u -> r j u", j=n_blocks)
            )
            w_sb = consts.tile([U, U], F32)
            nc.sync.dma_start(w_sb[:], w_in[:])
            bias_row = consts.tile([1, U], F32)
            nc.sync.dma_start(bias_row[:], b_in[:].unsqueeze(0))

            # xw_sb column block kb holds (X @ W)[128*kb : 128*kb+128, :]
            # (standard block layout, feeds the gn scaling).
            # t1p column group G holds (XW + bias) rows 256G+2r+s in the
            # row-pair layout: t1p[r, 128G + 64s + u].
            xw_sb = consts.tile([128, n_blocks * U], F32)
            t1p = consts.tile([128, n_blocks * U], F32)
            bias_t = consts.tile([128, U], F32)
            # out staging, same pair layout as t1p
            out_sb = consts.tile([128, n_blocks * U], F32)
            # X^T staging: xts_wide[u, m] = X[m, u]
            xts_wide = work.tile([U, n], F32, tag="xts", bufs=1)

            with tc.tile_pool(name="psetup", bufs=2, space="PSUM") as psetup:
                # bias broadcast across partitions: ones_row^T @ bias_row
                pb = psetup.tile([128, U], F32, tag="pb")
                nc.tensor.matmul(pb[:], ones_row[:], bias_row[:], start=True, stop=True)
                nc.vector.tensor_copy(bias_t[:], pb[:])
                # X^T assembly: transposing the r-major chunk j gives X^T
                # columns {16r+j}; a stride-16 copy scatters them into place.
                xtv = xts_wide[:].rearrange("u (r j) -> u j r", j=n_blocks)
                for j in range(n_blocks):
                    pxt = psetup.tile([U, 128], F32, tag="pxt")
                    nc.tensor.transpose(pxt[:], xtile[:, j], identity[:])
                    nc.vector.tensor_copy(xtv[:, j], pxt[:])
                for kb in range(n_blocks):
                    pxw = psetup.tile([128, U], F32, tag="pxw")
                    nc.tensor.matmul(
                        pxw[:],
                        xts_wide[:, 128 * kb : 128 * kb + 128],
                        w_sb[:],
                        start=True,
                        stop=True,
                    )
                    nc.vector.tensor_copy(xw_sb[:, U * kb : U * kb + U], pxw[:])
                # pair-layout XW + bias: for 256-row group G, parity s the
                # stationary operand is X^T columns 256G+2x+s (x = 0..127),
                # so the matmul output partition r holds row 256G+2r+s.
                for G in range(n_groups):
                    xv = xts_wide[:, 256 * G : 256 * G + 256].rearrange(
                        "u (x s) -> u s x", s=2
                    )
                    for s in range(2):
                        pxw = psetup.tile([128, U], F32, tag="pxw")
                        nc.tensor.matmul(
                            pxw[:], xv[:, s], w_sb[:], start=True, stop=True
                        )
                        nc.vector.tensor_tensor(
                            t1p[:, 128 * G + 64 * s : 128 * G + 64 * s + 64],
                            pxw[:],
                            bias_t[:],
                            ADD,
                        )

            with tc.tile_pool(name="pmain", bufs=2, space="PSUM") as pmain:
                # Column strips; the last strips are narrower so only a small
                # amount of deg/matmul work depends on the final DMAs.  All
                # widths stay >= 256 to keep f32r matmuls at full rate.
                widths = [512] * (n // 512)
                offs = [sum(widths[:i]) for i in range(len(widths))]
                n_strip_list = list(zip(offs, widths))
                n_quads = n // 512  # 512-row groups
                n_strips = len(n_strip_list)

                for p in range(n_slices):
                    # ndhat: column kb holds -1/sqrt(deg) for rows 128kb+r
                    # (standard layout, feeds gn).  dhat_pr: column 2G+s
                    # holds +1/sqrt(deg) for rows 256G+2r+s (pair layout,
                    # feeds the output stt).
                    ndhat = work.tile([128, n_blocks], F32, tag="ndhat")
                    dhat_pr = work.tile([128, n_groups * 2], F32, tag="dhat_pr")
                    gT = work.tile([128, n_blocks * UP], FP8, tag="gT")
                    atiles = []
                    banks = [
                        pmain.tile(
                            [UP, 512], F32, tag=f"pmmb{bi}", bufs=1,
                            name=f"pmmb_{p}_{bi}",
                        )
                        for bi in range(n_strips)
                    ]

                    def pmm_view(t):
                        return banks[t][:, 0 : n_strip_list[t][1]]

                    started = [False] * n_strips
                    emitted = [0] * n_strips
                    scaled = [False] * n_strips
                    # In the last slice, strips other than the final one close
                    # their accumulation at block 13 so their scale work runs
                    # while the final strip still streams; blocks 14/15 are
                    # applied later as an A-stationary correction.
                    last_slice = p == n_slices - 1
                    stop_at = [
                        n_blocks - 4 if last_slice and t < n_strips - 1 else n_blocks
                        for t in range(n_strips)
                    ]

                    def emit_mm(t, nb):
                        # pmm_t += gT[block nb].T @ A[rows nb, strip t cols]
                        emitted[t] += 1
                        mm = _lab(nc.tensor.matmul(
                            pmm_view(t),
                            gT[:, UP * nb : UP * nb + UP],
                            atiles[t][:, nb],
                            start=not started[t],
                            stop=(emitted[t] == stop_at[t]),
                        ), f"mm_p{p}_t{t}_nb{nb}")
                        started[t] = True
                        return mm

                    pmt_count = [0]

                    def emit_scale(t, after=None):
                        # out strip t: out = t1 + dhat * Mneg^T  (gT carries
                        # the -1/sqrt(deg) factor, so Mneg = -g @ A), built
                        # per 256-row group G and row parity s in the pair
                        # layout.
                        off, w = n_strip_list[t]
                        msb = work.tile([UP, 512], F32, tag="msb", bufs=5, name=f"msb_{p}_{t}")
                        nc.scalar.copy(msb[0:UP, 0:w], pmm_view(t))
                        for gl in range(w // 256):
                            G = off // 256 + gl
                            mv = msb[0:UP, 256 * gl : 256 * gl + 256].rearrange(
                                "c (x s) -> c s x", s=2
                            )
                            for s in range(2):
                                k = pmt_count[0]
                                pmt_count[0] += 1
                                pmt = pmain.tile(
                                    [128, UP], F32, tag="pmtb", bufs=3,
                                    name=f"pmt_{p}_{t}_{gl}_{s}",
                                )
                                tr = _lab(nc.tensor.transpose(
                                    pmt[:], mv[:, s], identity[0:UP, 0:UP]
                                ), f"pmtT_p{p}_t{t}_{gl}_{s}")
                                if after is not None:
                                    add_dep_helper(
                                        tr.ins, after.ins, sync=False,
                                        reason="scale transposes after endgame deg",
                                    )
                                col = 128 * G + 64 * s + UP * p
                                nc.vector.scalar_tensor_tensor(
                                    out_sb[:, col : col + UP],
                                    pmt[:],
                                    dhat_pr[:, 2 * G + s : 2 * G + s + 1],
                                    t1p[:, col : col + UP],
                                    MULT,
                                    ADD,
                                )

                    ready_blocks = []
                    for si, (off, w) in enumerate(n_strip_list):
                        last_strip = si == n_strips - 1
                        endgame = last_strip and p == n_slices - 1
                        if p == 0 and si == 0:
                            at = pre_at
                        else:
                            at = apool.tile(
                                [128, n_blocks, w], FP8, tag=f"A{w}",
                                bufs=13,
                                name=f"at_{p}_{si}",
                            )
                            src = a_in[p, :, off : off + w].rearrange(
                                "(j r) c -> r j c", r=128
                            )
                            if endgame:
                                # split the final row chunks so the last
                                # column-sum matmuls wait on 64KiB pieces only
                                nc.gpsimd.dma_start(
                                    at[:, 0 : n_blocks - 4], src[:, 0 : n_blocks - 4]
                                )
                                for jj in range(4):
                                    j = n_blocks - 4 + jj
                                    nc.gpsimd.dma_start(
                                        at[:, j : j + 1], src[:, j : j + 1]
                                    )
                            else:
                                nc.gpsimd.dma_start(at[:], src)
                        atiles.append(at)

                        # Column sums directly in column layout: for each
                        # 128-column chunk, deg_chunk[m] = A[:, chunk]^T @ ones
                        # with the A piece as the (cost-free) stationary
                        # operand and a single ones column moving.  pdeg
                        # columns 0..nj hold sqrt-input in standard layout
                        # (partition r of column c = deg[off+128c+r]); columns
                        # nj..2nj hold the row-pair layout (deg[off+256gl+2r+s]
                        # at column nj+2gl+s).  The accumulation groups on the
                        # shared tile serialize in emission order, which also
                        # pins the PE order deg-first in the endgame.
                        nj = w // 128
                        with tc.high_priority():
                            pdeg = pmain.tile(
                                [128, 2 * nj], F32, tag="pdeg", bufs=1,
                                padded_shape=[128, UP],
                                name=f"pdeg_{p}_{si}",
                            )

                        last_deg = [None]

                        def deg_group(col, view_fn, label):
                            with tc.high_priority():
                                for i in range(16):
                                    last_deg[0] = _lab(nc.tensor.matmul(
                                        pdeg[:, col : col + 1],
                                        view_fn(i),
                                        ones_bf[:],
                                        start=(i == 0),
                                        stop=(i == 15),
                                    ), f"{label}_i{i}")

                        def std_view(c):
                            return lambda i: at[:, i, 128 * c : 128 * c + 128]

                        def pair_view(gl, s):
                            def f(i):
                                return at[
                                    :, i, 256 * gl : 256 * gl + 256
                                ].rearrange("r (x s) -> r s x", s=2)[:, s]

                            return f

                        for c in range(nj):
                            deg_group(c, std_view(c), f"degS_p{p}_s{si}_c{c}")
                        for gl in range(w // 256):
                            for s in range(2):
                                deg_group(
                                    nj + 2 * gl + s, pair_view(gl, s),
                                    f"degP_p{p}_s{si}_g{gl}_{s}",
                                )
                        if endgame:
                            # the (near-free) deg groups must precede the
                            # strip's backlog and the deferred strip-3 scale
                            # in the static PE order so the scale chain starts
                            # right after the last byte
                            with tc.high_priority():
                                scaled[si - 1] = True
                                emit_scale(si - 1, after=last_deg[0])
                            for nb in ready_blocks:
                                mm = emit_mm(si, nb)
                                add_dep_helper(
                                    mm.ins, last_deg[0].ins, sync=False,
                                    reason="endgame: deg before backlog",
                                )
                        with tc.high_priority():
                            sq_cols = work.tile(
                                [128, 2 * nj], F32, tag="sq_cols",
                                padded_shape=[128, UP],
                                name=f"sq_cols_{p}_{si}",
                            )
                            nc.scalar.sqrt(sq_cols[:], pdeg[:])
                            b0 = off // 128
                            cs = slice(b0, b0 + nj)
                            # dhat = +1/sqrt(deg) gates gn: emit its consumers
                            # (the gn muls) before the pair-layout reciprocal
                            # so DVE order favours the critical path.
                            nc.vector.reciprocal(ndhat[:, cs], sq_cols[:, 0:nj])
                            new_blocks = list(range(b0, b0 + nj))
                            for kb in new_blocks:
                                nc.vector.tensor_scalar(
                                    gT[:, UP * kb : UP * kb + UP],
                                    xw_sb[:, U * kb + UP * p : U * kb + UP * p + UP],
                                    ndhat[:, kb : kb + 1],
                                    GSCALE,
                                    MULT,
                                    MULT,
                                )
                        # pair-layout -1/sqrt(deg) (off the critical
                        # path; only the late stt reads it)
                        g0 = off // 256
                        pv = dhat_pr[:, 2 * g0 : 2 * g0 + w // 128]
                        nc.vector.reciprocal(pv, sq_cols[:, nj : 2 * nj])
                        nc.vector.tensor_scalar_mul(pv, pv, -1.0 / GSCALE)

                        # Partial matmuls that just became ready.  The new
                        # strip's backlog (old gT blocks x new tiles) only
                        # needs the tiles, so emit it before the matmuls
                        # gated on this strip's deg chain.
                        if not endgame:
                            for nb in ready_blocks:
                                emit_mm(si, nb)
                        if endgame:
                            # only the final strip still accumulates the last
                            # blocks in its main (gn-stationary) group
                            for nb in new_blocks:
                                emit_mm(si, nb)
                        elif last_strip:
                            for t in [si, si - 1] + list(range(si - 1)):
                                for nb in new_blocks:
                                    if nb < stop_at[t]:
                                        emit_mm(t, nb)
                        else:
                            for t in range(si):
                                for nb in new_blocks:
                                    if nb < stop_at[t]:
                                        emit_mm(t, nb)
                            for nb in new_blocks:
                                if nb < stop_at[si]:
                                    emit_mm(si, nb)
                        ready_blocks += new_blocks
                        if not endgame:
                            with tc.high_priority():
                                for t in range(n_strips):
                                    if emitted[t] == stop_at[t] and not scaled[t]:
                                        if last_slice and t == n_strips - 2:
                                            # deferred into the endgame so its
                                            # PE transposes order after the
                                            # final strip's column sums
                                            continue
                                        scaled[t] = True
                                        emit_scale(t)

                        if endgame:
                            # Correction pass: add dhat * (gn[14:16] @ A)^T for
                            # strips 0..3, computed per 256-row pair group with
                            # the A piece as the (cost-free) stationary operand
                            # so the output lands directly in the pair layout.
                            # PSUM comes from the now-free main banks.
                            def corr_unit(G, s, bank):
                                t = next(
                                    tt
                                    for tt, (o, ww) in enumerate(n_strip_list)
                                    if o <= 256 * G < o + ww
                                )
                                o, ww = n_strip_list[t]
                                gl = (256 * G - o) // 256
                                corr = pmain.tile(
                                    [128, UP], F32, tag=f"pmmb{bank}", bufs=1,
                                    name=f"corr_{G}_{s}",
                                )
                                for k, i in enumerate(range(n_blocks - 4, n_blocks)):
                                    lhs = atiles[t][
                                        :, i, 256 * gl : 256 * gl + 256
                                    ].rearrange("r (x s) -> r s x", s=2)[:, s]
                                    _lab(nc.tensor.matmul(
                                        corr[:],
                                        lhs,
                                        gT[:, UP * i : UP * i + UP],
                                        start=(k == 0),
                                        stop=(k == 3),
                                    ), f"corr_p{p}_G{G}_s{s}_i{i}")
                                col = 128 * G + 64 * s + UP * p
                                nc.vector.scalar_tensor_tensor(
                                    out_sb[:, col : col + UP],
                                    corr[:],
                                    dhat_pr[:, 2 * G + s : 2 * G + s + 1],
                                    out_sb[:, col : col + UP],
                                    MULT,
                                    ADD,
                                )

                            def store_group(g):
                                dst = out_d[512 * g : 512 * g + 512, :].rearrange(
                                    "(j r s) u -> r j s u", r=128, s=2
                                )
                                src_sb = out_sb[
                                    :, 256 * g : 256 * g + 256
                                ].rearrange("r (j s u) -> r j s u", j=2, s=2)
                                nc.sync.dma_start(dst, src_sb)

                            nbank = [0]

                            def next_bank():
                                nbank[0] = (nbank[0] + 1) % (n_strips - 1)
                                return nbank[0]

                            # The final strip's own scale covers its two
                            # groups in full; the correction covers the rest.
                            emit_scale(si)
                            store_group(n // 512 - 1)
                            for G in (4, 5):
                                for s in range(2):
                                    corr_unit(G, s, next_bank())
                            store_group(2)
                            for G in (2, 3):
                                for s in range(2):
                                    corr_unit(G, s, next_bank())
                            store_group(1)
                            for G in (0, 1):
                                for s in range(2):
                                    corr_unit(G, s, next_bank())
                            store_group(0)

    nc.compile()
    return nc


_NC_CACHE = {}


def _get_nc():
    if "nc" not in _NC_CACHE:
        _NC_CACHE["nc"] = build()
    return _NC_CACHE["nc"]


def kernel(Ans, X, weight, bias):
    Ans = np.ascontiguousarray(Ans, dtype=np.float32)
    X = np.ascontiguousarray(X, dtype=np.float32)
    weight = np.ascontiguousarray(weight, dtype=np.float32)
    bias = np.ascontiguousarray(bias, dtype=np.float32)

    nc = _get_nc()
    in_maps = [
        {"a_in": Ans[b], "x_in": X[b], "w_in": weight, "b_in": bias}
        for b in range(Ans.shape[0])
    ]
    res = run_bass_kernel_spmd(nc, in_maps, core_ids=list(range(len(in_maps))))
    return np.stack([r["out"] for r in res.results], axis=0)


# revision 38
# speedup vs baseline: 2.3252x; 1.0451x over previous
"""GCNConv Trainium2 kernel.

Per (b, p) slice of Ans [B, P, n, n] the reference computes
    deg[m]  = sum_i A[i, m]                 (column sums)
    dhat    = 1 / (sqrt(deg) + eps)
    L       = diag(dhat) (diag(deg) - A) diag(dhat)
    out_bp  = h_p @ L          where h_p = ((X W)^T)[16p:16p+16, :]
With eps dropped (a ~3e-9 relative shift, far below the matmul noise),
deg*dhat^2 == 1 exactly, so the diagonal term collapses:
    out[c, m] = (XW)[m, c] + bias[c] + dhat[m] * (gn @ A)[c, m]
with gn = -(XW)^T * dhat.  t1 = XW + bias is a setup-time constant.

Key structure:
- A is cast to fp8 (e4m3) during the (gpsimd/SWDGE) DMA: the products
  over A are relaxed anyway and the fp8 rounding, scaled by dhat against
  the exact-f32 t1 term, lands ~25x inside the error budget, while the
  SBUF-side transfer time and footprint drop 4x vs f32.  gn is scaled by
  64 to sit in fp8e4m3's normal range (the output stt compensates).  One
  [128, 16, 512] tile per column strip (4 strips of 512, keeping every
  DMA chunk at the 512-byte full-rate boundary), three slices of tile
  buffering so the stream runs ahead of the (now PE-bound) matmuls.
- Column sums run with the A pieces as the (stationary) matmul operand
  and a single ones column moving, producing deg directly in both the
  standard column layout (feeds gn) and the row-pair layout (feeds the
  output scaling), eliminating all transpose round-trips from the
  per-strip chain: deg -> sqrt -> reciprocal -> gn.
- Main matmuls keep gn stationary, accumulating M = gn @ A per strip in
  [16, w] PSUM banks.  In the last slice all strips but the final one
  close their accumulation at block 11; the mid-stream scale pass
  (PSUM copy -> pair-layout PE transposes -> stt with dhat) runs while
  the final strips still stream, and the last four blocks are applied
  afterwards as an A-stationary correction directly in the row-pair
  layout ([128, 16] PSUM chunks rotating through the freed main banks),
  so the post-stream tail is only: deg tail -> sqrt -> reciprocal -> gn
  -> correction matmuls -> stt accumulate -> stores.
- Output is staged in SBUF in a row-pair layout (partition r of 256-row
  group G holds rows 256G+2r and 256G+2r+1) so the store DMAs move
  512-byte contiguous chunks; t1 is built in the same layout at setup
  time via strided-weight matmuls.  X is loaded r-major (4KiB chunks)
  and repacked with transposes + strided copies.
- The endgame static PE order is pinned with ordering-only deps so the
  near-free column-sum matmuls always precede the final strip's backlog.

Sharding: core b <- batch b (8 cores).  weight/bias are replicated; each
core gets Ans[b] ([4, 2048, 2048]) and X[b].  No collectives.
"""

import numpy as np

import concourse.bacc as bacc
import concourse.mybir as mybir
import concourse.tile as tile
from concourse.bass_utils import run_bass_kernel_spmd
from concourse.masks import make_identity
from concourse.tile_rust import add_dep_helper

F32 = mybir.dt.float32
BF16 = mybir.dt.bfloat16
FP8 = mybir.dt.float8e4
GSCALE = 64.0  # keeps gn in fp8e4m3's normal range; stt compensates
MULT = mybir.AluOpType.mult
ADD = mybir.AluOpType.add

U = 64
UP = 16  # U // P

# build-time instruction labels for trace attribution (no program effect)
MM_LABELS = {}


def _lab(bi, label):
    MM_LABELS[bi.ins.name] = label
    return bi


def build(n=2048, n_slices=4, a_bufs=14):
    """Build the per-core SPMD program.

    n: graph size (multiple of 512), n_slices: number of P slices per core.
    """
    assert n % 512 == 0
    n_blocks = n // 128  # 128-row blocks (also m-blocks)
    n_groups = n // 256  # 256-row groups (store pair-layout units)

    nc = bacc.Bacc("TRN2", target_bir_lowering=False, debug=False)

    a_in = nc.dram_tensor("a_in", [n_slices, n, n], F32, kind="ExternalInput")
    x_in = nc.dram_tensor("x_in", [n, U], F32, kind="ExternalInput")
    w_in = nc.dram_tensor("w_in", [U, U], F32, kind="ExternalInput")
    b_in = nc.dram_tensor("b_in", [U], F32, kind="ExternalInput")
    out_d = nc.dram_tensor("out", [n, U], F32, kind="ExternalOutput")

    with tile.TileContext(nc) as tc:
        with (
            tc.tile_pool(name="consts", bufs=1) as consts,
            tc.tile_pool(name="work", bufs=2) as work,
            tc.tile_pool(name="apool", bufs=a_bufs) as apool,
        ):
            identity = consts.tile([128, 128], F32)
            make_identity(nc, identity[:])
            ones_col = consts.tile([128, 1], F32)
            nc.vector.memset(ones_col[:], 1.0)
            ones_bf = consts.tile([128, 1], FP8)
            nc.vector.tensor_copy(ones_bf[:], ones_col[:])
            ones_row = consts.tile([1, 128], F32)
            nc.vector.memset(ones_row[:], 1.0)
            # Issue the first A strip's DMA ahead of the setup loads so
            # the big stream starts immediately (XW isn't needed for ~15us).
            # A is cast to bf16 during the (gpsimd) DMA: the matmuls over A
            # already run relaxed, the error budget allows it, and the
            # SBUF-side transfer and footprint halve.
            pre_at = apool.tile([128, 16, 512], FP8, tag="A512", bufs=13, name="at_0_0")
            nc.gpsimd.dma_start(
                pre_at[:],
                a_in[0, 0:2048, 0:512].rearrange("(j r) c -> r j c", r=128),
            )

            # X loaded r-major (partition r holds rows 16r..16r+15) so the
            # DMA moves 4KiB contiguous chunks; the block-layout fixup
            # happens in the setup transposes below.
            xtile = consts.tile([128, n_blocks, U], F32)
            nc.sync.dma_start(
                xtile[:], x_in[:].rearrange("(r j) u -> r j u", j=n_blocks)
            )
            w_sb = consts.tile([U, U], F32)
            nc.sync.dma_start(w_sb[:], w_in[:])
            bias_row = consts.tile([1, U], F32)
            nc.sync.dma_start(bias_row[:], b_in[:].unsqueeze(0))

            # xw_sb column block kb holds (X @ W)[128*kb : 128*kb+128, :]
            # (standard block layout, feeds the gn scaling).
            # t1p column group G holds (XW + bias) rows 256G+2r+s in the
            # row-pair layout: t1p[r, 128G + 64s + u].
            xw_sb = consts.tile([128, n_blocks * U], F32)
            t1p = consts.tile([128, n_blocks * U], F32)
            bias_t = consts.tile([128, U], F32)
            # out staging, same pair layout as t1p
            out_sb = consts.tile([128, n_blocks * U], F32)
            # X^T staging: xts_wide[u, m] = X[m, u]
            xts_wide = work.tile([U, n], F32, tag="xts", bufs=1)

            with tc.tile_pool(name="psetup", bufs=2, space="PSUM") as psetup:
                # bias broadcast across partitions: ones_row^T @ bias_row
                pb = psetup.tile([128, U], F32, tag="pb", bufs=1)
                nc.tensor.matmul(pb[:], ones_row[:], bias_row[:], start=True, stop=True)
                nc.vector.tensor_copy(bias_t[:], pb[:])
                # X^T assembly: transposing the r-major chunk j gives X^T
                # columns {16r+j}; a stride-16 copy scatters them into place.
                xtv = xts_wide[:].rearrange("u (r j) -> u j r", j=n_blocks)
                for j in range(n_blocks):
                    pxt = psetup.tile([U, 128], F32, tag="pxt", bufs=3)
                    nc.tensor.transpose(pxt[:], xtile[:, j], identity[:])
                    nc.vector.tensor_copy(xtv[:, j], pxt[:])
                for kb in range(n_blocks):
                    pxw = psetup.tile([128, U], F32, tag="pxw", bufs=4)
                    nc.tensor.matmul(
                        pxw[:],
                        xts_wide[:, 128 * kb : 128 * kb + 128],
                        w_sb[:],
                        start=True,
                        stop=True,
                    )
                    nc.vector.tensor_copy(xw_sb[:, U * kb : U * kb + U], pxw[:])
                # pair-layout XW + bias: for 256-row group G, parity s the
                # stationary operand is X^T columns 256G+2x+s (x = 0..127),
                # so the matmul output partition r holds row 256G+2r+s.
                for G in range(n_groups):
                    xv = xts_wide[:, 256 * G : 256 * G + 256].rearrange(
                        "u (x s) -> u s x", s=2
                    )
                    for s in range(2):
                        pxw = psetup.tile([128, U], F32, tag="pxw", bufs=4)
                        nc.tensor.matmul(
                            pxw[:], xv[:, s], w_sb[:], start=True, stop=True
                        )
                        nc.vector.tensor_tensor(
                            t1p[:, 128 * G + 64 * s : 128 * G + 64 * s + 64],
                            pxw[:],
                            bias_t[:],
                            ADD,
                        )

            with tc.tile_pool(name="pmain", bufs=2, space="PSUM") as pmain:
                # Column strips; the last strips are narrower so only a small
                # amount of deg/matmul work depends on the final DMAs.  All
                # widths stay >= 256 to keep f32r matmuls at full rate.
                widths = [512] * (n // 512)
                offs = [sum(widths[:i]) for i in range(len(widths))]
                n_strip_list = list(zip(offs, widths))
                n_quads = n // 512  # 512-row groups
                n_strips = len(n_strip_list)

                for p in range(n_slices):
                    # ndhat: column kb holds -1/sqrt(deg) for rows 128kb+r
                    # (standard layout, feeds gn).  dhat_pr: column 2G+s
                    # holds +1/sqrt(deg) for rows 256G+2r+s (pair layout,
                    # feeds the output stt).
                    ndhat = work.tile([128, n_blocks], F32, tag="ndhat")
                    dhat_pr = work.tile([128, n_groups * 2], F32, tag="dhat_pr")
                    gT = work.tile([128, n_blocks * UP], FP8, tag="gT")
                    atiles = []
                    banks = [
                        pmain.tile(
                            [UP, 512], F32, tag=f"pmmb{bi}", bufs=1,
                            name=f"pmmb_{p}_{bi}",
                        )
                        for bi in range(n_strips)
                    ]

                    def pmm_view(t):
                        return banks[t][:, 0 : n_strip_list[t][1]]

                    started = [False] * n_strips
                    emitted = [0] * n_strips
                    scaled = [False] * n_strips
                    # In the last slice, strips other than the final one close
                    # their accumulation at block 13 so their scale work runs
                    # while the final strip still streams; blocks 14/15 are
                    # applied later as an A-stationary correction.
                    last_slice = p == n_slices - 1
                    stop_at = [
                        n_blocks - 4 if last_slice and t < n_strips - 1 else n_blocks
                        for t in range(n_strips)
                    ]

                    def emit_mm(t, nb):
                        # pmm_t += gT[block nb].T @ A[rows nb, strip t cols]
                        emitted[t] += 1
                        mm = _lab(nc.tensor.matmul(
                            pmm_view(t),
                            gT[:, UP * nb : UP * nb + UP],
                            atiles[t][:, nb],
                            start=not started[t],
                            stop=(emitted[t] == stop_at[t]),
                        ), f"mm_p{p}_t{t}_nb{nb}")
                        started[t] = True
                        return mm

                    pmt_count = [0]

                    def emit_scale(t, after=None):
                        # out strip t: out = t1 + dhat * Mneg^T  (gT carries
                        # the -1/sqrt(deg) factor, so Mneg = -g @ A), built
                        # per 256-row group G and row parity s in the pair
                        # layout.
                        off, w = n_strip_list[t]
                        msb = work.tile([UP, 512], F32, tag="msb", bufs=5, name=f"msb_{p}_{t}")
                        nc.scalar.copy(msb[0:UP, 0:w], pmm_view(t))
                        for gl in range(w // 256):
                            G = off // 256 + gl
                            mv = msb[0:UP, 256 * gl : 256 * gl + 256].rearrange(
                                "c (x s) -> c s x", s=2
                            )
                            for s in range(2):
                                k = pmt_count[0]
                                pmt_count[0] += 1
                                pmt = pmain.tile(
                                    [128, UP], F32, tag="pmtb", bufs=3,
                                    name=f"pmt_{p}_{t}_{gl}_{s}",
                                )
                                tr = _lab(nc.tensor.transpose(
                                    pmt[:], mv[:, s], identity[0:UP, 0:UP]
                                ), f"pmtT_p{p}_t{t}_{gl}_{s}")
                                if after is not None:
                                    add_dep_helper(
                                        tr.ins, after.ins, sync=False,
                                        reason="scale transposes after endgame deg",
                                    )
                                col = 128 * G + 64 * s + UP * p
                                nc.vector.scalar_tensor_tensor(
                                    out_sb[:, col : col + UP],
                                    pmt[:],
                                    dhat_pr[:, 2 * G + s : 2 * G + s + 1],
                                    t1p[:, col : col + UP],
                                    MULT,
                                    ADD,
                                )

                    ready_blocks = []
                    for si, (off, w) in enumerate(n_strip_list):
                        last_strip = si == n_strips - 1
                        endgame = last_strip and p == n_slices - 1
                        if p == 0 and si == 0:
                            at = pre_at
                        else:
                            at = apool.tile(
                                [128, n_blocks, w], FP8, tag=f"A{w}",
                                bufs=13,
                                name=f"at_{p}_{si}",
                            )
                            src = a_in[p, :, off : off + w].rearrange(
                                "(j r) c -> r j c", r=128
                            )
                            if endgame:
                                # split the final row chunks so the last
                                # column-sum matmuls wait on 64KiB pieces only
                                nc.gpsimd.dma_start(
                                    at[:, 0 : n_blocks - 4], src[:, 0 : n_blocks - 4]
                                )
                                for jj in range(4):
                                    j = n_blocks - 4 + jj
                                    nc.gpsimd.dma_start(
                                        at[:, j : j + 1], src[:, j : j + 1]
                                    )
                            else:
                                nc.gpsimd.dma_start(at[:], src)
                        atiles.append(at)

                        # Column sums directly in column layout: for each
                        # 128-column chunk, deg_chunk[m] = A[:, chunk]^T @ ones
                        # with the A piece as the (cost-free) stationary
                        # operand and a single ones column moving.  pdeg
                        # columns 0..nj hold sqrt-input in standard layout
                        # (partition r of column c = deg[off+128c+r]); columns
                        # nj..2nj hold the row-pair layout (deg[off+256gl+2r+s]
                        # at column nj+2gl+s).  The accumulation groups on the
                        # shared tile serialize in emission order, which also
                        # pins the PE order deg-first in the endgame.
                        nj = w // 128
                        with tc.high_priority():
                            pdeg = pmain.tile(
                                [128, 2 * nj], F32, tag="pdeg", bufs=1,
                                padded_shape=[128, UP],
                                name=f"pdeg_{p}_{si}",
                            )

                        last_deg = [None]

                        def deg_group(col, view_fn, label):
                            with tc.high_priority():
                                for i in range(16):
                                    last_deg[0] = _lab(nc.tensor.matmul(
                                        pdeg[:, col : col + 1],
                                        view_fn(i),
                                        ones_bf[:],
                                        start=(i == 0),
                                        stop=(i == 15),
                                    ), f"{label}_i{i}")

                        def std_view(c):
                            return lambda i: at[:, i, 128 * c : 128 * c + 128]

                        def pair_view(gl, s):
                            def f(i):
                                return at[
                                    :, i, 256 * gl : 256 * gl + 256
                                ].rearrange("r (x s) -> r s x", s=2)[:, s]

                            return f

                        for c in range(nj):
                            deg_group(c, std_view(c), f"degS_p{p}_s{si}_c{c}")
                        for gl in range(w // 256):
                            for s in range(2):
                                deg_group(
                                    nj + 2 * gl + s, pair_view(gl, s),
                                    f"degP_p{p}_s{si}_g{gl}_{s}",
                                )
                        if endgame:
                            # the (near-free) deg groups must precede the
                            # strip's backlog and the deferred strip-3 scale
                            # in the static PE order so the scale chain starts
                            # right after the last byte
                            with tc.high_priority():
                                scaled[si - 1] = True
                                emit_scale(si - 1, after=last_deg[0])
                            for nb in ready_blocks:
                                mm = emit_mm(si, nb)
                                add_dep_helper(
                                    mm.ins, last_deg[0].ins, sync=False,
                                    reason="endgame: deg before backlog",
                                )
                        with tc.high_priority():
                            sq_cols = work.tile(
                                [128, 2 * nj], F32, tag="sq_cols",
                                padded_shape=[128, UP],
                                name=f"sq_cols_{p}_{si}",
                            )
                            nc.scalar.sqrt(sq_cols[:], pdeg[:])
                            b0 = off // 128
                            cs = slice(b0, b0 + nj)
                            # dhat = +1/sqrt(deg) gates gn: emit its consumers
                            # (the gn muls) before the pair-layout reciprocal
                            # so DVE order favours the critical path.
                            nc.vector.reciprocal(ndhat[:, cs], sq_cols[:, 0:nj])
                            new_blocks = list(range(b0, b0 + nj))
                            for kb in new_blocks:
                                nc.vector.tensor_scalar(
                                    gT[:, UP * kb : UP * kb + UP],
                                    xw_sb[:, U * kb + UP * p : U * kb + UP * p + UP],
                                    ndhat[:, kb : kb + 1],
                                    GSCALE,
                                    MULT,
                                    MULT,
                                )
                        # pair-layout -1/sqrt(deg) (off the critical
                        # path; only the late stt reads it)
                        g0 = off // 256
                        pv = dhat_pr[:, 2 * g0 : 2 * g0 + w // 128]
                        nc.vector.reciprocal(pv, sq_cols[:, nj : 2 * nj])
                        nc.vector.tensor_scalar_mul(pv, pv, -1.0 / GSCALE)

                        # Partial matmuls that just became ready.  The new
                        # strip's backlog (old gT blocks x new tiles) only
                        # needs the tiles, so emit it before the matmuls
                        # gated on this strip's deg chain.
                        if not endgame:
                            for nb in ready_blocks:
                                emit_mm(si, nb)
                        if endgame:
                            # only the final strip still accumulates the last
                            # blocks in its main (gn-stationary) group
                            for nb in new_blocks:
                                emit_mm(si, nb)
                        elif last_strip:
                            for t in [si, si - 1] + list(range(si - 1)):
                                for nb in new_blocks:
                                    if nb < stop_at[t]:
                                        emit_mm(t, nb)
                        else:
                            for t in range(si):
                                for nb in new_blocks:
                                    if nb < stop_at[t]:
                                        emit_mm(t, nb)
                            for nb in new_blocks:
                                if nb < stop_at[si]:
                                    emit_mm(si, nb)
                        ready_blocks += new_blocks
                        if not endgame:
                            with tc.high_priority():
                                for t in range(n_strips):
                                    if emitted[t] == stop_at[t] and not scaled[t]:
                                        if last_slice and t == n_strips - 2:
                                            # deferred into the endgame so its
                                            # PE transposes order after the
                                            # final strip's column sums
                                            continue
                                        scaled[t] = True
                                        emit_scale(t)

                        if endgame:
                            # Correction pass: add dhat * (gn[14:16] @ A)^T for
                            # strips 0..3, computed per 256-row pair group with
                            # the A piece as the (cost-free) stationary operand
                            # so the output lands directly in the pair layout.
                            # PSUM comes from the now-free main banks.
                            def corr_unit(G, s, bank):
                                t = next(
                                    tt
                                    for tt, (o, ww) in enumerate(n_strip_list)
                                    if o <= 256 * G < o + ww
                                )
                                o, ww = n_strip_list[t]
                                gl = (256 * G - o) // 256
                                corr = pmain.tile(
                                    [128, UP], F32, tag=f"pmmb{bank}", bufs=1,
                                    name=f"corr_{G}_{s}",
                                )
                                for k, i in enumerate(range(n_blocks - 4, n_blocks)):
                                    lhs = atiles[t][
                                        :, i, 256 * gl : 256 * gl + 256
                                    ].rearrange("r (x s) -> r s x", s=2)[:, s]
                                    _lab(nc.tensor.matmul(
                                        corr[:],
                                        lhs,
                                        gT[:, UP * i : UP * i + UP],
                                        start=(k == 0),
                                        stop=(k == 3),
                                    ), f"corr_p{p}_G{G}_s{s}_i{i}")
                                col = 128 * G + 64 * s + UP * p
                                nc.vector.scalar_tensor_tensor(
                                    out_sb[:, col : col + UP],
                                    corr[:],
                                    dhat_pr[:, 2 * G + s : 2 * G + s + 1],
                                    out_sb[:, col : col + UP],
                                    MULT,
                                    ADD,
                                )

                            def store_group(g):
                                dst = out_d[512 * g : 512 * g + 512, :].rearrange(
                                    "(j r s) u -> r j s u", r=128, s=2
                                )
                                src_sb = out_sb[
                                    :, 256 * g : 256 * g + 256
                                ].rearrange("r (j s u) -> r j s u", j=2, s=2)
                                nc.sync.dma_start(dst, src_sb)

                            nbank = [0]

                            def next_bank():
                                nbank[0] = (nbank[0] + 1) % (n_strips - 1)
                                return nbank[0]

                            # The final strip's own scale covers its two
                            # groups in full; the correction covers the rest.
                            emit_scale(si)
                            store_group(n // 512 - 1)
                            for G in (4, 5):
                                for s in range(2):
                                    corr_unit(G, s, next_bank())
                            store_group(2)
                            for G in (2, 3):
                                for s in range(2):
                                    corr_unit(G, s, next_bank())
                            store_group(1)
                            for G in (0, 1):
                                for s in range(2):
                                    corr_unit(G, s, next_bank())
                            store_group(0)

    nc.compile()
    return nc


_NC_CACHE = {}


def _get_nc():
    if "nc" not in _NC_CACHE:
        _NC_CACHE["nc"] = build()
    return _NC_CACHE["nc"]


def kernel(Ans, X, weight, bias):
    Ans = np.ascontiguousarray(Ans, dtype=np.float32)
    X = np.ascontiguousarray(X, dtype=np.float32)
    weight = np.ascontiguousarray(weight, dtype=np.float32)
    bias = np.ascontiguousarray(bias, dtype=np.float32)

    nc = _get_nc()
    in_maps = [
        {"a_in": Ans[b], "x_in": X[b], "w_in": weight, "b_in": bias}
        for b in range(Ans.shape[0])
    ]
    res = run_bass_kernel_spmd(nc, in_maps, core_ids=list(range(len(in_maps))))
    return np.stack([r["out"] for r in res.results], axis=0)


# revision 40
# speedup vs baseline: 3.1975x; 1.3751x over previous
"""GCNConv Trainium2 kernel.

Per (b, p) slice of Ans [B, P, n, n] the reference computes
    deg[m]  = sum_i A[i, m]                 (column sums)
    dhat    = 1 / (sqrt(deg) + eps)
    L       = diag(dhat) (diag(deg) - A) diag(dhat)
    out_bp  = h_p @ L          where h_p = ((X W)^T)[16p:16p+16, :]
With eps dropped (a ~3e-9 relative shift, far below the matmul noise),
deg*dhat^2 == 1 exactly, so the diagonal term collapses:
    out[c, m] = (XW)[m, c] + bias[c] + dhat[m] * (gn @ A)[c, m]
with gn = -(XW)^T * dhat.  t1 = XW + bias is a setup-time constant.

Key structure:
- A is cast to fp8 (e4m3) during the (gpsimd/SWDGE) DMA: the products
  over A are relaxed anyway and the fp8 rounding, scaled by dhat against
  the exact-f32 t1 term, lands ~25x inside the error budget, while the
  SBUF-side transfer time and footprint drop 4x vs f32.  gn is scaled by
  64 to sit in fp8e4m3's normal range (the output stt compensates).  One
  [128, 16, 512] tile per column strip (4 strips of 512, keeping every
  DMA chunk at the 512-byte full-rate boundary), three slices of tile
  buffering so the stream runs ahead of the (now PE-bound) matmuls.
- Column sums run with the A pieces as the (stationary) matmul operand
  and a single ones column moving, producing deg directly in both the
  standard column layout (feeds gn) and the row-pair layout (feeds the
  output scaling), eliminating all transpose round-trips from the
  per-strip chain: deg -> sqrt -> reciprocal -> gn.
- Main matmuls keep gn stationary, accumulating M = gn @ A per strip in
  [16, w] PSUM banks.  In the last slice all strips but the final one
  close their accumulation at block 11; the mid-stream scale pass
  (PSUM copy -> pair-layout PE transposes -> stt with dhat) runs while
  the final strips still stream, and the last four blocks are applied
  afterwards as an A-stationary correction directly in the row-pair
  layout ([128, 16] PSUM chunks rotating through the freed main banks),
  so the post-stream tail is only: deg tail -> sqrt -> reciprocal -> gn
  -> correction matmuls -> stt accumulate -> stores.
- Output is staged in SBUF in a row-pair layout (partition r of 256-row
  group G holds rows 256G+2r and 256G+2r+1) so the store DMAs move
  512-byte contiguous chunks; t1 is built in the same layout at setup
  time via strided-weight matmuls.  X is loaded r-major (4KiB chunks)
  and repacked with transposes + strided copies.
- The endgame static PE order is pinned with ordering-only deps so the
  near-free column-sum matmuls always precede the final strip's backlog.

Sharding: core b <- batch b (8 cores).  weight/bias are replicated; each
core gets Ans[b] ([4, 2048, 2048]) and X[b].  No collectives.
"""

import numpy as np

import concourse.bacc as bacc
import concourse.mybir as mybir
import concourse.tile as tile
from concourse.bass_utils import run_bass_kernel_spmd
from concourse.masks import make_identity
from concourse.tile_rust import add_dep_helper

F32 = mybir.dt.float32
BF16 = mybir.dt.bfloat16
FP8 = mybir.dt.float8e4
GSCALE = 64.0  # keeps gn in fp8e4m3's normal range; stt compensates
MULT = mybir.AluOpType.mult
ADD = mybir.AluOpType.add

U = 64
UP = 16  # U // P

# build-time instruction labels for trace attribution (no program effect)
MM_LABELS = {}


def _lab(bi, label):
    MM_LABELS[bi.ins.name] = label
    return bi


def build(n=2048, n_slices=4, a_bufs=14):
    """Build the per-core SPMD program.

    n: graph size (multiple of 512), n_slices: number of P slices per core.
    """
    assert n % 512 == 0
    n_blocks = n // 128  # 128-row blocks (also m-blocks)
    n_groups = n // 256  # 256-row groups (store pair-layout units)

    nc = bacc.Bacc("TRN2", target_bir_lowering=False, debug=False)

    a_in = nc.dram_tensor("a_in", [n_slices, n, n], F32, kind="ExternalInput")
    x_in = nc.dram_tensor("x_in", [n, U], F32, kind="ExternalInput")
    w_in = nc.dram_tensor("w_in", [U, U], F32, kind="ExternalInput")
    b_in = nc.dram_tensor("b_in", [U], F32, kind="ExternalInput")
    out_d = nc.dram_tensor("out", [n, U], F32, kind="ExternalOutput")

    with tile.TileContext(nc) as tc:
        with (
            tc.tile_pool(name="consts", bufs=1) as consts,
            tc.tile_pool(name="work", bufs=2) as work,
            tc.tile_pool(name="apool", bufs=a_bufs) as apool,
        ):
            identity = consts.tile([128, 128], F32)
            make_identity(nc, identity[:])
            ones_col = consts.tile([128, 1], F32)
            nc.vector.memset(ones_col[:], 1.0)
            ones_bf = consts.tile([128, 1], FP8)
            nc.vector.tensor_copy(ones_bf[:], ones_col[:])
            ones_row = consts.tile([1, 128], F32)
            nc.vector.memset(ones_row[:], 1.0)
            # Issue the first A strip's DMA ahead of the setup loads so
            # the big stream starts immediately (XW isn't needed for ~15us).
            # A is cast to bf16 during the (gpsimd) DMA: the matmuls over A
            # already run relaxed, the error budget allows it, and the
            # SBUF-side transfer and footprint halve.
            pre_at = apool.tile([128, 16, 512], FP8, tag="A512", bufs=13, name="at_0_0")
            nc.gpsimd.dma_start(
                pre_at[:],
                a_in[0, 0:2048, 0:512].rearrange("(j r) c -> r j c", r=128),
            )

            # X loaded r-major (partition r holds rows 16r..16r+15) so the
            # DMA moves 4KiB contiguous chunks; the block-layout fixup
            # happens in the setup transposes below.
            xtile = consts.tile([128, n_blocks, U], F32)
            nc.sync.dma_start(
                xtile[:], x_in[:].rearrange("(r j) u -> r j u", j=n_blocks)
            )
            w_sb = consts.tile([U, U], F32)
            nc.sync.dma_start(w_sb[:], w_in[:])
            bias_row = consts.tile([1, U], F32)
            nc.sync.dma_start(bias_row[:], b_in[:].unsqueeze(0))

            # xw_sb column block kb holds (X @ W)[128*kb : 128*kb+128, :]
            # (standard block layout, feeds the gn scaling).
            # t1p column group G holds (XW + bias) rows 256G+2r+s in the
            # row-pair layout: t1p[r, 128G + 64s + u].
            xw_sb = consts.tile([128, n_blocks * U], F32)
            t1p = consts.tile([128, n_blocks * U], F32)
            bias_t = consts.tile([128, U], F32)
            # out staging, same pair layout as t1p
            out_sb = consts.tile([128, n_blocks * U], F32)
            # X^T staging: xts_wide[u, m] = X[m, u]
            xts_wide = work.tile([U, n], F32, tag="xts", bufs=1)

            with tc.tile_pool(name="psetup", bufs=2, space="PSUM") as psetup:
                # bias broadcast across partitions: ones_row^T @ bias_row
                pb = psetup.tile([128, U], F32, tag="pb", bufs=1)
                nc.tensor.matmul(pb[:], ones_row[:], bias_row[:], start=True, stop=True)
                nc.vector.tensor_copy(bias_t[:], pb[:])
                # X^T assembly: transposing the r-major chunk j gives X^T
                # columns {16r+j}; a stride-16 copy scatters them into place.
                xtv = xts_wide[:].rearrange("u (r j) -> u j r", j=n_blocks)
                for j in range(n_blocks):
                    pxt = psetup.tile([U, 128], F32, tag="pxt", bufs=3)
                    nc.tensor.transpose(pxt[:], xtile[:, j], identity[:])
                    nc.vector.tensor_copy(xtv[:, j], pxt[:])
                for kb in range(n_blocks):
                    pxw = psetup.tile([128, U], F32, tag="pxw", bufs=4)
                    nc.tensor.matmul(
                        pxw[:],
                        xts_wide[:, 128 * kb : 128 * kb + 128],
                        w_sb[:],
                        start=True,
                        stop=True,
                    )
                    nc.vector.tensor_copy(xw_sb[:, U * kb : U * kb + U], pxw[:])
                # pair-layout XW + bias: for 256-row group G, parity s the
                # stationary operand is X^T columns 256G+2x+s (x = 0..127),
                # so the matmul output partition r holds row 256G+2r+s.
                for G in range(n_groups):
                    xv = xts_wide[:, 256 * G : 256 * G + 256].rearrange(
                        "u (x s) -> u s x", s=2
                    )
                    for s in range(2):
                        pxw = psetup.tile([128, U], F32, tag="pxw", bufs=4)
                        nc.tensor.matmul(
                            pxw[:], xv[:, s], w_sb[:], start=True, stop=True
                        )
                        nc.vector.tensor_tensor(
                            t1p[:, 128 * G + 64 * s : 128 * G + 64 * s + 64],
                            pxw[:],
                            bias_t[:],
                            ADD,
                        )

            with tc.tile_pool(name="pmain", bufs=2, space="PSUM") as pmain:
                # Column strips; the last strips are narrower so only a small
                # amount of deg/matmul work depends on the final DMAs.  All
                # widths stay >= 256 to keep f32r matmuls at full rate.
                widths = [512] * (n // 512)
                offs = [sum(widths[:i]) for i in range(len(widths))]
                n_strip_list = list(zip(offs, widths))
                n_quads = n // 512  # 512-row groups
                n_strips = len(n_strip_list)

                for p in range(n_slices):
                    # ndhat: column kb holds -1/sqrt(deg) for rows 128kb+r
                    # (standard layout, feeds gn).  dhat_pr: column 2G+s
                    # holds +1/sqrt(deg) for rows 256G+2r+s (pair layout,
                    # feeds the output stt).
                    ndhat = work.tile([128, n_blocks], F32, tag="ndhat")
                    dhat_pr = work.tile([128, n_groups * 2], F32, tag="dhat_pr")
                    gT = work.tile([128, n_blocks * UP], FP8, tag="gT")
                    atiles = []
                    banks = [
                        pmain.tile(
                            [UP, 512], F32, tag=f"pmmb{bi}", bufs=1,
                            name=f"pmmb_{p}_{bi}",
                        )
                        for bi in range(n_strips)
                    ]

                    def pmm_view(t):
                        return banks[t][:, 0 : n_strip_list[t][1]]

                    started = [False] * n_strips
                    emitted = [0] * n_strips
                    scaled = [False] * n_strips
                    # In the last slice, strips other than the final one close
                    # their accumulation at block 13 so their scale work runs
                    # while the final strip still streams; blocks 14/15 are
                    # applied later as an A-stationary correction.
                    last_slice = p == n_slices - 1
                    njb = n_blocks // 2
                    stop_at = [
                        njb - 2 if last_slice and t < n_strips - 1 else njb
                        for t in range(n_strips)
                    ]

                    def emit_mm(t, jb):
                        # pmm_t += gT[blocks 2jb,2jb+1].T @ A[those rows,
                        # strip t cols] -- DoubleRow packs the two adjacent
                        # 128-row blocks as the fp8 weight pair, contracting
                        # 256 rows per matmul at half the cycle cost.
                        emitted[t] += 1
                        mm = _lab(nc.tensor.matmul(
                            pmm_view(t),
                            gT[:, 2 * UP * jb : 2 * UP * jb + 2 * UP].rearrange(
                                "r (ko u) -> r ko u", ko=2
                            ),
                            atiles[t][:, 2 * jb : 2 * jb + 2],
                            start=not started[t],
                            stop=(emitted[t] == stop_at[t]),
                            perf_mode=mybir.MatmulPerfMode.DoubleRow,
                        ), f"mm_p{p}_t{t}_jb{jb}")
                        started[t] = True
                        return mm

                    pmt_count = [0]

                    def emit_scale(t, after=None):
                        # out strip t: out = t1 + dhat * Mneg^T  (gT carries
                        # the -1/sqrt(deg) factor, so Mneg = -g @ A), built
                        # per 256-row group G and row parity s in the pair
                        # layout.
                        off, w = n_strip_list[t]
                        msb = work.tile([UP, 512], F32, tag="msb", bufs=5, name=f"msb_{p}_{t}")
                        nc.scalar.copy(msb[0:UP, 0:w], pmm_view(t))
                        for gl in range(w // 256):
                            G = off // 256 + gl
                            mv = msb[0:UP, 256 * gl : 256 * gl + 256].rearrange(
                                "c (x s) -> c s x", s=2
                            )
                            for s in range(2):
                                k = pmt_count[0]
                                pmt_count[0] += 1
                                pmt = pmain.tile(
                                    [128, UP], F32, tag="pmtb", bufs=3,
                                    name=f"pmt_{p}_{t}_{gl}_{s}",
                                )
                                tr = _lab(nc.tensor.transpose(
                                    pmt[:], mv[:, s], identity[0:UP, 0:UP]
                                ), f"pmtT_p{p}_t{t}_{gl}_{s}")
                                if after is not None:
                                    add_dep_helper(
                                        tr.ins, after.ins, sync=False,
                                        reason="scale transposes after endgame deg",
                                    )
                                col = 128 * G + 64 * s + UP * p
                                nc.vector.scalar_tensor_tensor(
                                    out_sb[:, col : col + UP],
                                    pmt[:],
                                    dhat_pr[:, 2 * G + s : 2 * G + s + 1],
                                    t1p[:, col : col + UP],
                                    MULT,
                                    ADD,
                                )

                    ready_blocks = []
                    for si, (off, w) in enumerate(n_strip_list):
                        last_strip = si == n_strips - 1
                        endgame = last_strip and p == n_slices - 1
                        if p == 0 and si == 0:
                            at = pre_at
                        else:
                            at = apool.tile(
                                [128, n_blocks, w], FP8, tag=f"A{w}",
                                bufs=13,
                                name=f"at_{p}_{si}",
                            )
                            src = a_in[p, :, off : off + w].rearrange(
                                "(j r) c -> r j c", r=128
                            )
                            if endgame:
                                # split the final row chunks so the last
                                # column-sum matmuls wait on 64KiB pieces only
                                nc.gpsimd.dma_start(
                                    at[:, 0 : n_blocks - 4], src[:, 0 : n_blocks - 4]
                                )
                                for jj in range(4):
                                    j = n_blocks - 4 + jj
                                    nc.gpsimd.dma_start(
                                        at[:, j : j + 1], src[:, j : j + 1]
                                    )
                            else:
                                nc.gpsimd.dma_start(at[:], src)
                        atiles.append(at)

                        # Column sums directly in column layout: for each
                        # 128-column chunk, deg_chunk[m] = A[:, chunk]^T @ ones
                        # with the A piece as the (cost-free) stationary
                        # operand and a single ones column moving.  pdeg
                        # columns 0..nj hold sqrt-input in standard layout
                        # (partition r of column c = deg[off+128c+r]); columns
                        # nj..2nj hold the row-pair layout (deg[off+256gl+2r+s]
                        # at column nj+2gl+s).  The accumulation groups on the
                        # shared tile serialize in emission order, which also
                        # pins the PE order deg-first in the endgame.
                        nj = w // 128
                        with tc.high_priority():
                            pdeg = pmain.tile(
                                [128, 2 * nj], F32, tag="pdeg", bufs=1,
                                padded_shape=[128, UP],
                                name=f"pdeg_{p}_{si}",
                            )

                        last_deg = [None]

                        def deg_group(col, view_fn, label):
                            with tc.high_priority():
                                for i in range(16):
                                    last_deg[0] = _lab(nc.tensor.matmul(
                                        pdeg[:, col : col + 1],
                                        view_fn(i),
                                        ones_bf[:],
                                        start=(i == 0),
                                        stop=(i == 15),
                                    ), f"{label}_i{i}")

                        def std_view(c):
                            return lambda i: at[:, i, 128 * c : 128 * c + 128]

                        def pair_view(gl, s):
                            def f(i):
                                return at[
                                    :, i, 256 * gl : 256 * gl + 256
                                ].rearrange("r (x s) -> r s x", s=2)[:, s]

                            return f

                        for c in range(nj):
                            deg_group(c, std_view(c), f"degS_p{p}_s{si}_c{c}")
                        for gl in range(w // 256):
                            for s in range(2):
                                deg_group(
                                    nj + 2 * gl + s, pair_view(gl, s),
                                    f"degP_p{p}_s{si}_g{gl}_{s}",
                                )
                        if endgame:
                            # the (near-free) deg groups must precede the
                            # strip's backlog and the deferred strip-3 scale
                            # in the static PE order so the scale chain starts
                            # right after the last byte
                            with tc.high_priority():
                                scaled[si - 1] = True
                                emit_scale(si - 1, after=last_deg[0])
                            for nb in ready_blocks:
                                mm = emit_mm(si, nb)
                                add_dep_helper(
                                    mm.ins, last_deg[0].ins, sync=False,
                                    reason="endgame: deg before backlog",
                                )
                        with tc.high_priority():
                            sq_cols = work.tile(
                                [128, 2 * nj], F32, tag="sq_cols",
                                padded_shape=[128, UP],
                                name=f"sq_cols_{p}_{si}",
                            )
                            nc.scalar.sqrt(sq_cols[:], pdeg[:])
                            b0 = off // 128
                            cs = slice(b0, b0 + nj)
                            # dhat = +1/sqrt(deg) gates gn: emit its consumers
                            # (the gn muls) before the pair-layout reciprocal
                            # so DVE order favours the critical path.
                            nc.vector.reciprocal(ndhat[:, cs], sq_cols[:, 0:nj])
                            new_blocks = list(range(b0, b0 + nj))
                            mm_jbs = [2 * si, 2 * si + 1]
                            for kb in new_blocks:
                                nc.vector.tensor_scalar(
                                    gT[:, UP * kb : UP * kb + UP],
                                    xw_sb[:, U * kb + UP * p : U * kb + UP * p + UP],
                                    ndhat[:, kb : kb + 1],
                                    GSCALE,
                                    MULT,
                                    MULT,
                                )
                        # pair-layout -1/sqrt(deg) (off the critical
                        # path; only the late stt reads it)
                        g0 = off // 256
                        pv = dhat_pr[:, 2 * g0 : 2 * g0 + w // 128]
                        nc.vector.reciprocal(pv, sq_cols[:, nj : 2 * nj])
                        nc.vector.tensor_scalar_mul(pv, pv, -1.0 / GSCALE)

                        # Partial matmuls that just became ready.  The new
                        # strip's backlog (old gT blocks x new tiles) only
                        # needs the tiles, so emit it before the matmuls
                        # gated on this strip's deg chain.
                        if not endgame:
                            for nb in ready_blocks:
                                emit_mm(si, nb)
                        if endgame:
                            # only the final strip still accumulates the last
                            # blocks in its main (gn-stationary) group
                            for jb in mm_jbs:
                                emit_mm(si, jb)
                        elif last_strip:
                            for t in [si, si - 1] + list(range(si - 1)):
                                for jb in mm_jbs:
                                    if jb < stop_at[t]:
                                        emit_mm(t, jb)
                        else:
                            for t in range(si):
                                for jb in mm_jbs:
                                    if jb < stop_at[t]:
                                        emit_mm(t, jb)
                            for jb in mm_jbs:
                                if jb < stop_at[si]:
                                    emit_mm(si, jb)
                        ready_blocks += mm_jbs
                        if not endgame:
                            with tc.high_priority():
                                for t in range(n_strips):
                                    if emitted[t] == stop_at[t] and not scaled[t]:
                                        if last_slice and t == n_strips - 2:
                                            # deferred into the endgame so its
                                            # PE transposes order after the
                                            # final strip's column sums
                                            continue
                                        scaled[t] = True
                                        emit_scale(t)

                        if endgame:
                            # Correction pass: add dhat * (gn[14:16] @ A)^T for
                            # strips 0..3, computed per 256-row pair group with
                            # the A piece as the (cost-free) stationary operand
                            # so the output lands directly in the pair layout.
                            # PSUM comes from the now-free main banks.
                            def corr_unit(G, s, bank):
                                t = next(
                                    tt
                                    for tt, (o, ww) in enumerate(n_strip_list)
                                    if o <= 256 * G < o + ww
                                )
                                o, ww = n_strip_list[t]
                                gl = (256 * G - o) // 256
                                corr = pmain.tile(
                                    [128, UP], F32, tag=f"pmmb{bank}", bufs=1,
                                    name=f"corr_{G}_{s}",
                                )
                                for k, i in enumerate(range(n_blocks - 4, n_blocks)):
                                    lhs = atiles[t][
                                        :, i, 256 * gl : 256 * gl + 256
                                    ].rearrange("r (x s) -> r s x", s=2)[:, s]
                                    _lab(nc.tensor.matmul(
                                        corr[:],
                                        lhs,
                                        gT[:, UP * i : UP * i + UP],
                                        start=(k == 0),
                                        stop=(k == 3),
                                    ), f"corr_p{p}_G{G}_s{s}_i{i}")
                                col = 128 * G + 64 * s + UP * p
                                nc.vector.scalar_tensor_tensor(
                                    out_sb[:, col : col + UP],
                                    corr[:],
                                    dhat_pr[:, 2 * G + s : 2 * G + s + 1],
                                    out_sb[:, col : col + UP],
                                    MULT,
                                    ADD,
                                )

                            def store_group(g):
                                dst = out_d[512 * g : 512 * g + 512, :].rearrange(
                                    "(j r s) u -> r j s u", r=128, s=2
                                )
                                src_sb = out_sb[
                                    :, 256 * g : 256 * g + 256
                                ].rearrange("r (j s u) -> r j s u", j=2, s=2)
                                nc.sync.dma_start(dst, src_sb)

                            nbank = [0]

                            def next_bank():
                                nbank[0] = (nbank[0] + 1) % (n_strips - 1)
                                return nbank[0]

                            # The final strip's own scale covers its two
                            # groups in full; the correction covers the rest.
                            emit_scale(si)
                            store_group(n // 512 - 1)
                            for G in (4, 5):
                                for s in range(2):
                                    corr_unit(G, s, next_bank())
                            store_group(2)
                            for G in (2, 3):
                                for s in range(2):
                                    corr_unit(G, s, next_bank())
                            store_group(1)
                            for G in (0, 1):
                                for s in range(2):
                                    corr_unit(G, s, next_bank())
                            store_group(0)

    nc.compile()
    return nc


_NC_CACHE = {}


def _get_nc():
    if "nc" not in _NC_CACHE:
        _NC_CACHE["nc"] = build()
    return _NC_CACHE["nc"]


def kernel(Ans, X, weight, bias):
    Ans = np.ascontiguousarray(Ans, dtype=np.float32)
    X = np.ascontiguousarray(X, dtype=np.float32)
    weight = np.ascontiguousarray(weight, dtype=np.float32)
    bias = np.ascontiguousarray(bias, dtype=np.float32)

    nc = _get_nc()
    in_maps = [
        {"a_in": Ans[b], "x_in": X[b], "w_in": weight, "b_in": bias}
        for b in range(Ans.shape[0])
    ]
    res = run_bass_kernel_spmd(nc, in_maps, core_ids=list(range(len(in_maps))))
    return np.stack([r["out"] for r in res.results], axis=0)


# revision 47
# speedup vs baseline: 3.2245x; 1.0084x over previous
"""GCNConv Trainium2 kernel.

Per (b, p) slice of Ans [B, P, n, n] the reference computes
    deg[m]  = sum_i A[i, m]                 (column sums)
    dhat    = 1 / (sqrt(deg) + eps)
    L       = diag(dhat) (diag(deg) - A) diag(dhat)
    out_bp  = h_p @ L          where h_p = ((X W)^T)[16p:16p+16, :]
With eps dropped (a ~3e-9 relative shift, far below the matmul noise),
deg*dhat^2 == 1 exactly, so the diagonal term collapses:
    out[c, m] = (XW)[m, c] + bias[c] + dhat[m] * (gn @ A)[c, m]
with gn = -(XW)^T * dhat.  t1 = XW + bias is a setup-time constant.

Key structure:
- A is cast to fp8 (e4m3) during the (gpsimd/SWDGE) DMA: the products
  over A are relaxed anyway and the fp8 rounding, scaled by dhat against
  the exact-f32 t1 term, lands ~25x inside the error budget, while the
  SBUF-side transfer time and footprint drop 4x vs f32.  gn is scaled by
  64 to sit in fp8e4m3's normal range (the output stt compensates).  One
  [128, 16, 512] tile per column strip (4 strips of 512, keeping every
  DMA chunk at the 512-byte full-rate boundary), three slices of tile
  buffering so the stream runs ahead of the (now PE-bound) matmuls.
- Column sums run with the A pieces as the (stationary) matmul operand
  and a single ones column moving, producing deg directly in both the
  standard column layout (feeds gn) and the row-pair layout (feeds the
  output scaling), eliminating all transpose round-trips from the
  per-strip chain: deg -> sqrt -> reciprocal -> gn.
- Main matmuls keep gn stationary, accumulating M = gn @ A per strip in
  [16, w] PSUM banks.  In the last slice all strips but the final one
  close their accumulation at block 11; the mid-stream scale pass
  (PSUM copy -> pair-layout PE transposes -> stt with dhat) runs while
  the final strips still stream, and the last four blocks are applied
  afterwards as an A-stationary correction directly in the row-pair
  layout ([128, 16] PSUM chunks rotating through the freed main banks),
  so the post-stream tail is only: deg tail -> sqrt -> reciprocal -> gn
  -> correction matmuls -> stt accumulate -> stores.
- Output is staged in SBUF in a row-pair layout (partition r of 256-row
  group G holds rows 256G+2r and 256G+2r+1) so the store DMAs move
  512-byte contiguous chunks; t1 is built in the same layout at setup
  time via strided-weight matmuls.  X is loaded r-major (4KiB chunks)
  and repacked with transposes + strided copies.
- The endgame static PE order is pinned with ordering-only deps so the
  near-free column-sum matmuls always precede the final strip's backlog.

Sharding: core b <- batch b (8 cores).  weight/bias are replicated; each
core gets Ans[b] ([4, 2048, 2048]) and X[b].  No collectives.
"""

import numpy as np

import concourse.bacc as bacc
import concourse.mybir as mybir
import concourse.tile as tile
from concourse.bass_utils import run_bass_kernel_spmd
from concourse.masks import make_identity
from concourse.tile_rust import add_dep_helper

F32 = mybir.dt.float32
BF16 = mybir.dt.bfloat16
FP8 = mybir.dt.float8e4
GSCALE = 64.0  # keeps gn in fp8e4m3's normal range; stt compensates
MULT = mybir.AluOpType.mult
ADD = mybir.AluOpType.add

U = 64
UP = 16  # U // P

# build-time instruction labels for trace attribution (no program effect)
MM_LABELS = {}


def _lab(bi, label):
    MM_LABELS[bi.ins.name] = label
    return bi


def build(n=2048, n_slices=4, a_bufs=14):
    """Build the per-core SPMD program.

    n: graph size (multiple of 512), n_slices: number of P slices per core.
    """
    assert n % 512 == 0
    n_blocks = n // 128  # 128-row blocks (also m-blocks)
    n_groups = n // 256  # 256-row groups (store pair-layout units)

    nc = bacc.Bacc("TRN2", target_bir_lowering=False, debug=False)

    a_in = nc.dram_tensor("a_in", [n_slices, n, n], F32, kind="ExternalInput")
    x_in = nc.dram_tensor("x_in", [n, U], F32, kind="ExternalInput")
    w_in = nc.dram_tensor("w_in", [U, U], F32, kind="ExternalInput")
    b_in = nc.dram_tensor("b_in", [U], F32, kind="ExternalInput")
    out_d = nc.dram_tensor("out", [n, U], F32, kind="ExternalOutput")

    with tile.TileContext(nc) as tc:
        with (
            tc.tile_pool(name="consts", bufs=1) as consts,
            tc.tile_pool(name="work", bufs=2) as work,
            tc.tile_pool(name="apool", bufs=a_bufs) as apool,
        ):
            identity = consts.tile([128, 128], F32)
            make_identity(nc, identity[:])
            ones_col = consts.tile([128, 1], F32)
            nc.vector.memset(ones_col[:], 1.0)
            ones_bf = consts.tile([128, 1], FP8)
            nc.vector.tensor_copy(ones_bf[:], ones_col[:])
            ones_row = consts.tile([1, 128], F32)
            nc.vector.memset(ones_row[:], 1.0)
            # Issue the first A strip's DMA ahead of the setup loads so
            # the big stream starts immediately (XW isn't needed for ~15us).
            # A is cast to bf16 during the (gpsimd) DMA: the matmuls over A
            # already run relaxed, the error budget allows it, and the
            # SBUF-side transfer and footprint halve.
            pre_at = apool.tile([128, 16, 512], FP8, tag="A512", bufs=13, name="at_0_0")
            nc.gpsimd.dma_start(
                pre_at[:],
                a_in[0, 0:2048, 0:512].rearrange("(j r) c -> r j c", r=128),
            )

            # X loaded r-major (partition r holds rows 16r..16r+15) so the
            # DMA moves 4KiB contiguous chunks; the block-layout fixup
            # happens in the setup transposes below.
            xtile = consts.tile([128, n_blocks, U], F32)
            nc.sync.dma_start(
                xtile[:], x_in[:].rearrange("(r j) u -> r j u", j=n_blocks)
            )
            w_sb = consts.tile([U, U], F32)
            nc.sync.dma_start(w_sb[:], w_in[:])
            bias_row = consts.tile([1, U], F32)
            nc.sync.dma_start(bias_row[:], b_in[:].unsqueeze(0))

            # xw_sb column block kb holds (X @ W)[128*kb : 128*kb+128, :]
            # (standard block layout, feeds the gn scaling).
            # t1p column group G holds (XW + bias) rows 256G+2r+s in the
            # row-pair layout: t1p[r, 128G + 64s + u].
            xw_sb = consts.tile([128, n_blocks * U], F32)
            t1p = consts.tile([128, n_blocks * U], F32)
            bias_t = consts.tile([128, U], F32)
            # out staging, same pair layout as t1p
            out_sb = consts.tile([128, n_blocks * U], F32)
            # X^T staging: xts_wide[u, m] = X[m, u]
            xts_wide = work.tile([U, n], F32, tag="xts", bufs=1)

            with tc.tile_pool(name="psetup", bufs=2, space="PSUM") as psetup:
                # bias broadcast across partitions: ones_row^T @ bias_row
                pb = psetup.tile([128, U], F32, tag="pb", bufs=1)
                nc.tensor.matmul(pb[:], ones_row[:], bias_row[:], start=True, stop=True)
                nc.vector.tensor_copy(bias_t[:], pb[:])
                # X^T assembly: transposing the r-major chunk j gives X^T
                # columns {16r+j}; a stride-16 copy scatters them into place.
                xtv = xts_wide[:].rearrange("u (r j) -> u j r", j=n_blocks)
                for j in range(n_blocks):
                    pxt = psetup.tile([U, 128], F32, tag="pxt", bufs=3)
                    nc.tensor.transpose(pxt[:], xtile[:, j], identity[:])
                    nc.vector.tensor_copy(xtv[:, j], pxt[:])
                for kb in range(n_blocks):
                    pxw = psetup.tile([128, U], F32, tag="pxw", bufs=4)
                    nc.tensor.matmul(
                        pxw[:],
                        xts_wide[:, 128 * kb : 128 * kb + 128],
                        w_sb[:],
                        start=True,
                        stop=True,
                    )
                    nc.vector.tensor_copy(xw_sb[:, U * kb : U * kb + U], pxw[:])
                # pair-layout XW + bias: for 256-row group G, parity s the
                # stationary operand is X^T columns 256G+2x+s (x = 0..127),
                # so the matmul output partition r holds row 256G+2r+s.
                for G in range(n_groups):
                    xv = xts_wide[:, 256 * G : 256 * G + 256].rearrange(
                        "u (x s) -> u s x", s=2
                    )
                    for s in range(2):
                        pxw = psetup.tile([128, U], F32, tag="pxw", bufs=4)
                        nc.tensor.matmul(
                            pxw[:], xv[:, s], w_sb[:], start=True, stop=True
                        )
                        nc.vector.tensor_tensor(
                            t1p[:, 128 * G + 64 * s : 128 * G + 64 * s + 64],
                            pxw[:],
                            bias_t[:],
                            ADD,
                        )

            with tc.tile_pool(name="pmain", bufs=2, space="PSUM") as pmain:
                # Column strips; the last strips are narrower so only a small
                # amount of deg/matmul work depends on the final DMAs.  All
                # widths stay >= 256 to keep f32r matmuls at full rate.
                widths = [512] * (n // 512)
                offs = [sum(widths[:i]) for i in range(len(widths))]
                n_strip_list = list(zip(offs, widths))
                n_quads = n // 512  # 512-row groups
                n_strips = len(n_strip_list)

                for p in range(n_slices):
                    # ndhat: column kb holds -1/sqrt(deg) for rows 128kb+r
                    # (standard layout, feeds gn).  dhat_pr: column 2G+s
                    # holds +1/sqrt(deg) for rows 256G+2r+s (pair layout,
                    # feeds the output stt).
                    ndhat = work.tile([128, n_blocks], F32, tag="ndhat")
                    dhat_pr = work.tile([128, n_groups * 2], F32, tag="dhat_pr")
                    gT = work.tile([128, n_blocks * UP], FP8, tag="gT")
                    atiles = []
                    banks = [
                        pmain.tile(
                            [UP, 512], F32, tag=f"pmmb{bi}", bufs=1,
                            name=f"pmmb_{p}_{bi}",
                        )
                        for bi in range(n_strips)
                    ]

                    def pmm_view(t):
                        return banks[t][:, 0 : n_strip_list[t][1]]

                    started = [False] * n_strips
                    emitted = [0] * n_strips
                    scaled = [False] * n_strips
                    # In the last slice, strips other than the final one close
                    # their accumulation at block 13 so their scale work runs
                    # while the final strip still streams; blocks 14/15 are
                    # applied later as an A-stationary correction.
                    last_slice = p == n_slices - 1
                    njb = n_blocks // 2
                    stop_at = [
                        njb - 2 if last_slice and t < n_strips - 1 else njb
                        for t in range(n_strips)
                    ]

                    def emit_mm(t, jb):
                        # pmm_t += gT[blocks 2jb,2jb+1].T @ A[those rows,
                        # strip t cols] -- DoubleRow packs the two adjacent
                        # 128-row blocks as the fp8 weight pair, contracting
                        # 256 rows per matmul at half the cycle cost.
                        emitted[t] += 1
                        mm = _lab(nc.tensor.matmul(
                            pmm_view(t),
                            gT[:, 2 * UP * jb : 2 * UP * jb + 2 * UP].rearrange(
                                "r (ko u) -> r ko u", ko=2
                            ),
                            atiles[t][:, 2 * jb : 2 * jb + 2],
                            start=not started[t],
                            stop=(emitted[t] == stop_at[t]),
                            perf_mode=mybir.MatmulPerfMode.DoubleRow,
                        ), f"mm_p{p}_t{t}_jb{jb}")
                        started[t] = True
                        return mm

                    pmt_count = [0]

                    def emit_scale(t, after=None):
                        # out strip t: out = t1 + dhat * Mneg^T  (gT carries
                        # the -1/sqrt(deg) factor, so Mneg = -g @ A), built
                        # per 256-row group G and row parity s in the pair
                        # layout.
                        off, w = n_strip_list[t]
                        msb = work.tile([UP, 512], F32, tag="msb", bufs=5, name=f"msb_{p}_{t}")
                        nc.scalar.copy(msb[0:UP, 0:w], pmm_view(t))
                        for gl in range(w // 256):
                            G = off // 256 + gl
                            mv = msb[0:UP, 256 * gl : 256 * gl + 256].rearrange(
                                "c (x s) -> c s x", s=2
                            )
                            for s in range(2):
                                k = pmt_count[0]
                                pmt_count[0] += 1
                                pmt = pmain.tile(
                                    [128, UP], F32, tag="pmtb", bufs=3,
                                    name=f"pmt_{p}_{t}_{gl}_{s}",
                                )
                                tr = _lab(nc.tensor.transpose(
                                    pmt[:], mv[:, s], identity[0:UP, 0:UP]
                                ), f"pmtT_p{p}_t{t}_{gl}_{s}")
                                if after is not None:
                                    add_dep_helper(
                                        tr.ins, after.ins, sync=False,
                                        reason="scale transposes after endgame deg",
                                    )
                                col = 128 * G + 64 * s + UP * p
                                nc.vector.scalar_tensor_tensor(
                                    out_sb[:, col : col + UP],
                                    pmt[:],
                                    dhat_pr[:, 2 * G + s : 2 * G + s + 1],
                                    t1p[:, col : col + UP],
                                    MULT,
                                    ADD,
                                )

                    ready_blocks = []
                    for si, (off, w) in enumerate(n_strip_list):
                        last_strip = si == n_strips - 1
                        endgame = last_strip and p == n_slices - 1
                        if p == 0 and si == 0:
                            at = pre_at
                        else:
                            at = apool.tile(
                                [128, n_blocks, w], FP8, tag=f"A{w}",
                                bufs=13,
                                name=f"at_{p}_{si}",
                            )
                            src = a_in[p, :, off : off + w].rearrange(
                                "(j r) c -> r j c", r=128
                            )
                            if endgame:
                                # split the final row chunks so the last
                                # column-sum matmuls wait on 64KiB pieces only
                                nc.gpsimd.dma_start(
                                    at[:, 0 : n_blocks - 4], src[:, 0 : n_blocks - 4]
                                )
                                for jj in range(4):
                                    j = n_blocks - 4 + jj
                                    nc.gpsimd.dma_start(
                                        at[:, j : j + 1], src[:, j : j + 1]
                                    )
                            else:
                                nc.gpsimd.dma_start(at[:], src)
                        atiles.append(at)

                        # Column sums directly in column layout: for each
                        # 128-column chunk, deg_chunk[m] = A[:, chunk]^T @ ones
                        # with the A piece as the (cost-free) stationary
                        # operand and a single ones column moving.  pdeg
                        # columns 0..nj hold sqrt-input in standard layout
                        # (partition r of column c = deg[off+128c+r]); columns
                        # nj..2nj hold the row-pair layout (deg[off+256gl+2r+s]
                        # at column nj+2gl+s).  The accumulation groups on the
                        # shared tile serialize in emission order, which also
                        # pins the PE order deg-first in the endgame.
                        nj = w // 128
                        with tc.high_priority():
                            # alternate the accumulator between the dedicated
                            # bank and the pmt rotation so consecutive strips'
                            # column-sum chains never serialize on one bank
                            dtag, dbufs = ("pdeg", 1) if si % 2 == 0 else ("pmtb", 3)
                            pdeg = pmain.tile(
                                [128, 2 * nj], F32, tag=dtag, bufs=dbufs,
                                padded_shape=[128, UP],
                                name=f"pdeg_{p}_{si}",
                            )

                        last_deg = [None]

                        def deg_group(col, view_fn, label):
                            with tc.high_priority():
                                for i in range(16):
                                    last_deg[0] = _lab(nc.tensor.matmul(
                                        pdeg[:, col : col + 1],
                                        view_fn(i),
                                        ones_bf[:],
                                        start=(i == 0),
                                        stop=(i == 15),
                                    ), f"{label}_i{i}")

                        def std_view(c):
                            return lambda i: at[:, i, 128 * c : 128 * c + 128]

                        def pair_view(gl, s):
                            def f(i):
                                return at[
                                    :, i, 256 * gl : 256 * gl + 256
                                ].rearrange("r (x s) -> r s x", s=2)[:, s]

                            return f

                        for c in range(nj):
                            deg_group(c, std_view(c), f"degS_p{p}_s{si}_c{c}")
                        for gl in range(w // 256):
                            for s in range(2):
                                deg_group(
                                    nj + 2 * gl + s, pair_view(gl, s),
                                    f"degP_p{p}_s{si}_g{gl}_{s}",
                                )
                        if endgame:
                            # the (near-free) deg groups must precede the
                            # strip's backlog and the deferred strip-3 scale
                            # in the static PE order so the scale chain starts
                            # right after the last byte
                            with tc.high_priority():
                                scaled[si - 1] = True
                                emit_scale(si - 1, after=last_deg[0])
                            for nb in ready_blocks:
                                mm = emit_mm(si, nb)
                                add_dep_helper(
                                    mm.ins, last_deg[0].ins, sync=False,
                                    reason="endgame: deg before backlog",
                                )
                        with tc.high_priority():
                            sq_cols = work.tile(
                                [128, 2 * nj], F32, tag="sq_cols",
                                padded_shape=[128, UP],
                                name=f"sq_cols_{p}_{si}",
                            )
                            nc.scalar.sqrt(sq_cols[:], pdeg[:])
                            b0 = off // 128
                            cs = slice(b0, b0 + nj)
                            # dhat = +1/sqrt(deg) gates gn: emit its consumers
                            # (the gn muls) before the pair-layout reciprocal
                            # so DVE order favours the critical path.
                            nc.vector.reciprocal(ndhat[:, cs], sq_cols[:, 0:nj])
                            new_blocks = list(range(b0, b0 + nj))
                            mm_jbs = [2 * si, 2 * si + 1]
                            for kb in new_blocks:
                                nc.vector.tensor_scalar(
                                    gT[:, UP * kb : UP * kb + UP],
                                    xw_sb[:, U * kb + UP * p : U * kb + UP * p + UP],
                                    ndhat[:, kb : kb + 1],
                                    GSCALE,
                                    MULT,
                                    MULT,
                                )
                        # pair-layout -1/sqrt(deg) (off the critical
                        # path; only the late stt reads it)
                        g0 = off // 256
                        pv = dhat_pr[:, 2 * g0 : 2 * g0 + w // 128]
                        nc.vector.reciprocal(pv, sq_cols[:, nj : 2 * nj])
                        nc.vector.tensor_scalar_mul(pv, pv, -1.0 / GSCALE)

                        # Partial matmuls that just became ready.  The new
                        # strip's backlog (old gT blocks x new tiles) only
                        # needs the tiles, so emit it before the matmuls
                        # gated on this strip's deg chain.
                        if not endgame:
                            for nb in ready_blocks:
                                emit_mm(si, nb)
                        if endgame:
                            # only the final strip still accumulates the last
                            # blocks in its main (gn-stationary) group
                            for jb in mm_jbs:
                                emit_mm(si, jb)
                        elif last_strip:
                            for t in [si, si - 1] + list(range(si - 1)):
                                for jb in mm_jbs:
                                    if jb < stop_at[t]:
                                        emit_mm(t, jb)
                        else:
                            for t in range(si):
                                for jb in mm_jbs:
                                    if jb < stop_at[t]:
                                        emit_mm(t, jb)
                            for jb in mm_jbs:
                                if jb < stop_at[si]:
                                    emit_mm(si, jb)
                        ready_blocks += mm_jbs
                        if not endgame:
                            with tc.high_priority():
                                for t in range(n_strips):
                                    if emitted[t] == stop_at[t] and not scaled[t]:
                                        if last_slice and t == n_strips - 2:
                                            # deferred into the endgame so its
                                            # PE transposes order after the
                                            # final strip's column sums
                                            continue
                                        scaled[t] = True
                                        emit_scale(t)

                        if endgame:
                            # Correction pass: add dhat * (gn[14:16] @ A)^T for
                            # strips 0..3, computed per 256-row pair group with
                            # the A piece as the (cost-free) stationary operand
                            # so the output lands directly in the pair layout.
                            # PSUM comes from the now-free main banks.
                            def corr_unit(G, s, bank):
                                t = next(
                                    tt
                                    for tt, (o, ww) in enumerate(n_strip_list)
                                    if o <= 256 * G < o + ww
                                )
                                o, ww = n_strip_list[t]
                                gl = (256 * G - o) // 256
                                corr = pmain.tile(
                                    [128, UP], F32, tag=f"pmmb{bank}", bufs=1,
                                    name=f"corr_{G}_{s}",
                                )
                                for k, i in enumerate(range(n_blocks - 4, n_blocks)):
                                    lhs = atiles[t][
                                        :, i, 256 * gl : 256 * gl + 256
                                    ].rearrange("r (x s) -> r s x", s=2)[:, s]
                                    _lab(nc.tensor.matmul(
                                        corr[:],
                                        lhs,
                                        gT[:, UP * i : UP * i + UP],
                                        start=(k == 0),
                                        stop=(k == 3),
                                    ), f"corr_p{p}_G{G}_s{s}_i{i}")
                                col = 128 * G + 64 * s + UP * p
                                nc.vector.scalar_tensor_tensor(
                                    out_sb[:, col : col + UP],
                                    corr[:],
                                    dhat_pr[:, 2 * G + s : 2 * G + s + 1],
                                    out_sb[:, col : col + UP],
                                    MULT,
                                    ADD,
                                )

                            def store_group(g):
                                dst = out_d[512 * g : 512 * g + 512, :].rearrange(
                                    "(j r s) u -> r j s u", r=128, s=2
                                )
                                src_sb = out_sb[
                                    :, 256 * g : 256 * g + 256
                                ].rearrange("r (j s u) -> r j s u", j=2, s=2)
                                nc.sync.dma_start(dst, src_sb)

                            nbank = [0]

                            def next_bank():
                                nbank[0] = (nbank[0] + 1) % (n_strips - 1)
                                return nbank[0]

                            # The final strip's own scale covers its two
                            # groups in full; the correction covers the rest.
                            emit_scale(si)
                            store_group(n // 512 - 1)
                            for G in (4, 5):
                                for s in range(2):
                                    corr_unit(G, s, next_bank())
                            store_group(2)
                            for G in (2, 3):
                                for s in range(2):
                                    corr_unit(G, s, next_bank())
                            store_group(1)
                            for G in (0, 1):
                                for s in range(2):
                                    corr_unit(G, s, next_bank())
                            store_group(0)

    nc.compile()
    return nc


_NC_CACHE = {}


def _get_nc():
    if "nc" not in _NC_CACHE:
        _NC_CACHE["nc"] = build()
    return _NC_CACHE["nc"]


def kernel(Ans, X, weight, bias):
    Ans = np.ascontiguousarray(Ans, dtype=np.float32)
    X = np.ascontiguousarray(X, dtype=np.float32)
    weight = np.ascontiguousarray(weight, dtype=np.float32)
    bias = np.ascontiguousarray(bias, dtype=np.float32)

    nc = _get_nc()
    in_maps = [
        {"a_in": Ans[b], "x_in": X[b], "w_in": weight, "b_in": bias}
        for b in range(Ans.shape[0])
    ]
    res = run_bass_kernel_spmd(nc, in_maps, core_ids=list(range(len(in_maps))))
    return np.stack([r["out"] for r in res.results], axis=0)
